# revision 1
# baseline (speedup 1.0000x reference)
"""Trainium2 Bass kernel for nn_CrossTowerCausalModel.

Data-parallel over graphs: each of the 8 NeuronCores handles 128 graphs
(128*32 = 4096 nodes, 128*64 = 8192 edges). Weights/embeddings replicated.

Device activation layout is "transposed" (layout B): hT[feature, node] with
the 768 feature dim split into 6 chunks of 128 partitions. Weight matrices
[in, out] then serve directly as matmul lhsT (stationary) operands.

Receptive-field restriction: the GNN output h_graph is only read at 2 nodes
per graph (c, t). Host permutes each graph's 32 node slots so that
  slot 0 = c, slot 1 = t (filler if c == t),
  slots [0, C2) contain S2 = {c,t} U in({c,t})  (in-neighbors),
so layer 3 only computes slots {0,1} and layer 2 only the C2-slot prefix
(C2=16 when the per-graph |S2| max allows; else full 32). Layer 1 must stay
full (its targets feed layer-2 message sources anywhere). This is exact --
every value read downstream is identical to the full computation.

h is stored in bf16 (matmul input dtype), eliminating the per-layer f32->
bf16 copy; the per-layer psum accumulation and relu stay fp32.

Host-side prep (pure index logic + layout, no heavy math):
  * per-graph node permutation (above) -> final gathers h_c / h_t become
    stride-32 strided copies. (c == t graphs fixed up with copy_predicated.)
  * x passed pre-transposed (feature-major) bf16.
  * dense per-graph adjacency as block-diagonal 128x128 AT tiles (layer 1),
    [128, 4*C2] restricted tiles (layer 2), [128, 16] tiles (layer 3).
  * the quirky first-edge/dist logic of the reference (exact int math).
  * speaker/emotion one-hots (16 rows) fused into the input projection.
"""

import numpy as np
import ml_dtypes

B = 1024          # graphs
P = 32            # nodes per graph
N = B * P
H = 768
HC = H // 128     # 6 feature chunks
L = 3
DSEM = 1024
NUM_SPK, NUM_EMO = 9, 7
NCORES = 8
BC = B // NCORES          # graphs per core = 128
NCN = BC * P              # nodes per core = 4096
NT = 8                    # node tiles of 512 per core
GPT = 4                   # groups (of 128 nodes) per node tile

BF16 = ml_dtypes.bfloat16

_cache = {}


def _build_program(C2):
    from contextlib import ExitStack

    import concourse.bacc as bacc
    import concourse.mybir as mybir
    import concourse.tile as tile
    from concourse.masks import make_identity

    f32 = mybir.dt.float32
    bf16 = mybir.dt.bfloat16
    AF = mybir.ActivationFunctionType

    NC2 = BC * C2             # layer-2 packed cols per core
    NT2 = NC2 // 512          # layer-2 node tiles of 512
    RT3 = NC2 // 128          # layer-3 source row tiles
    GP3 = 128 // C2           # graphs per layer-3 source row tile

    nc = bacc.Bacc(
        "TRN2", target_bir_lowering=False, debug=False, num_devices=NCORES
    )

    dram = lambda name, shape, dt: nc.dram_tensor(
        name, shape, dt, kind="ExternalInput"
    ).ap()

    xt = dram("xt", [NT, 128, 8 * 512], bf16)
    oh16 = dram("oh16", [16, NCN], bf16)
    embcat = dram("embcat", [16, H], bf16)
    wsem = dram("wsem", [128, 8 * H], bf16)
    wself = dram("wself", [L, 128, HC * H], bf16)
    wnbr = dram("wnbr", [L, 128, HC * H], bf16)
    atb = dram("atb", [NCN // 128, 128, 128], bf16)
    atb2 = dram("atb2", [NCN // 128, 128, 4 * C2], bf16)
    atb3 = dram("atb3", [RT3, 128, 2 * GP3], bf16)
    cmask = dram("cmask", [128, BC], mybir.dt.uint8)
    ohd = dram("ohd", [P, BC], bf16)
    demb = dram("demb", [P, H], bf16)
    wexpl = dram("wexpl", [128, HC * H], bf16)
    bexpl = dram("bexpl", [128, HC], f32)
    ext = dram("ext", [H, BC], bf16)
    wp1 = dram("wp1", [HC, 128, 36 * 128], bf16)
    bp1 = dram("bp1", [128, HC], f32)
    wp2 = dram("wp2", [128, HC], bf16)
    bp2 = dram("bp2", [1, 1], f32)
    out_ap = nc.dram_tensor("out", [1, BC], f32, kind="ExternalOutput").ap()

    # [C*128, J] dram AP -> [128, C, J] (partition-major chunked view)
    def chunked(ap, J):
        return ap.rearrange("(c p) j -> c p j", p=128).transpose([1, 0, 2])

    # SBUF tile [128, C*J] -> [128, C, J]
    def sb3(t, J):
        return t[:].rearrange("p (c j) -> p c j", j=J)

    with tile.TileContext(nc) as tc, ExitStack() as ctx:
        erpool = ctx.enter_context(tc.tile_pool(name="er", bufs=1))
        cpool = ctx.enter_context(tc.tile_pool(name="const", bufs=1))

        # pools close in LIFO order: hA (after layer 1), then hB (after
        # layer 2); h2 lives until the end.
        h2pool = ctx.enter_context(tc.tile_pool(name="h2", bufs=1))
        hsB = ctx.enter_context(ExitStack())     # closes after layer 2
        hBpool = hsB.enter_context(tc.tile_pool(name="hB", bufs=1))
        hsA = ctx.enter_context(ExitStack())     # closes after layer 1
        hApool = hsA.enter_context(tc.tile_pool(name="hA", bufs=1))

        ident = cpool.tile([128, 128], bf16)
        make_identity(nc, ident)
        cmask_t = cpool.tile([128, BC], mybir.dt.uint8)
        nc.sync.dma_start(cmask_t[:], cmask[:])

        # transposed activations, ping-pong: hA = h_text (layer-1 input),
        # hB = h1 (layer-2 input); each [jc][nt] tile is [128, 512] bf16
        hA = [
            [
                hApool.tile(
                    [128, 512], bf16, tag=f"hA_{jc}_{nt}", name=f"hA_{jc}_{nt}"
                )
                for nt in range(NT)
            ]
            for jc in range(HC)
        ]
        hB = [
            [
                hBpool.tile(
                    [128, 512], bf16, tag=f"hB_{jc}_{nt}", name=f"hB_{jc}_{nt}"
                )
                for nt in range(NT)
            ]
            for jc in range(HC)
        ]
        # layer-2 packed activations: h2T[jc][nt2] is [128, 512] bf16
        h2T = [
            [
                h2pool.tile(
                    [128, 512], bf16, tag=f"h2_{jc}_{nt2}", name=f"h2_{jc}_{nt2}"
                )
                for nt2 in range(NT2)
            ]
            for jc in range(HC)
        ]
        # edge_repr^T, 36 chunks of 128 rows: [h_graph_c, h_text_c, h_graph_t,
        # h_text_t, h_dist, z] each HC chunks wide
        erT = erpool.tile([128, 36 * 128], bf16)

        # ---------------- phase 1: text projection ----------------
        with ExitStack() as p1:
            xtpool = p1.enter_context(tc.tile_pool(name="xt", bufs=3))
            wsem_pool = p1.enter_context(tc.tile_pool(name="wsem", bufs=1))
            oh_pool = p1.enter_context(tc.tile_pool(name="oh16", bufs=3))
            ps_a = p1.enter_context(tc.tile_pool(name="ps_a", bufs=4, space="PSUM"))

            wsem_t = wsem_pool.tile([128, 8 * H], bf16)
            nc.sync.dma_start(wsem_t[:], wsem[:])
            emb_t = wsem_pool.tile([128, H], bf16)
            nc.sync.dma_start(emb_t[:16, :], embcat[:])
            for nt in range(NT):
                oh16_t = oh_pool.tile([128, 512], bf16)
                nc.sync.dma_start(oh16_t[:16, :], oh16[:, nt * 512:][:, :512])
                xt_t = xtpool.tile([128, 8 * 512], bf16)
                nc.sync.dma_start(xt_t[:], xt[nt])
                for jc in range(HC):
                    acc = ps_a.tile([128, 512], f32)
                    for kc in range(8):
                        nc.tensor.matmul(
                            acc[:],
                            wsem_t[:, kc * H + jc * 128:][:, :128],
                            xt_t[:, kc * 512:][:, :512],
                            start=(kc == 0),
                            stop=False,
                        )
                    nc.tensor.matmul(
                        acc[:],
                        emb_t[:16, jc * 128:][:, :128],
                        oh16_t[:16, :],
                        start=False,
                        stop=True,
                    )
                    nc.scalar.activation(hA[jc][nt][:], acc[:], AF.Relu)

            # h_text gathers (chunks 6-11 = h_text_c, 18-23 = h_text_t)
            for jc in range(HC):
                for nt in range(NT):
                    src = hA[jc][nt].rearrange("p (b u) -> p b u", u=P)
                    nc.vector.tensor_copy(
                        erT[:, (6 + jc) * 128 + nt * 16:][:, :16],
                        src[:, :, 0],
                    )
                    nc.vector.tensor_copy(
                        erT[:, (18 + jc) * 128 + nt * 16:][:, :16],
                        src[:, :, 1],
                    )
                nc.vector.copy_predicated(
                    erT[:, (18 + jc) * 128:][:, :BC],
                    cmask_t[:],
                    erT[:, (6 + jc) * 128:][:, :BC],
                )

        # ---------------- phase 2: GNN layer 1 (full) ----------------
        with ExitStack() as p2:
            wpool = p2.enter_context(tc.tile_pool(name="w", bufs=2))
            a1pool = p2.enter_context(tc.tile_pool(name="a1", bufs=1))
            hapool = p2.enter_context(tc.tile_pool(name="ha", bufs=3))
            msgpool = p2.enter_context(tc.tile_pool(name="msg", bufs=2))
            tmppool = p2.enter_context(tc.tile_pool(name="tmp", bufs=3))
            ps_t2 = p2.enter_context(tc.tile_pool(name="ps_t2", bufs=2, space="PSUM"))
            ps_m = p2.enter_context(tc.tile_pool(name="ps_m", bufs=2, space="PSUM"))
            ps_a2 = p2.enter_context(tc.tile_pool(name="ps_a2", bufs=3, space="PSUM"))

            atb_t = a1pool.tile([128, (NCN // 128) * 128], bf16)
            nc.sync.dma_start(sb3(atb_t, 128), atb.transpose([1, 0, 2]))

            ws_t = wpool.tile([128, HC * H], bf16, tag="ws")
            nc.sync.dma_start(ws_t[:], wself[0])
            wn_t = wpool.tile([128, HC * H], bf16, tag="wn")
            nc.sync.dma_start(wn_t[:], wnbr[0])
            for nt in range(NT):
                msg_t = msgpool.tile([128, HC * 512], bf16)
                for jc in range(HC):
                    pst = ps_t2.tile([128, 512], bf16)
                    for g4 in range(GPT):
                        nc.tensor.transpose(
                            pst[:, g4 * 128:][:, :128],
                            hA[jc][nt][:, g4 * 128:][:, :128],
                            ident[:],
                        )
                    ha = hapool.tile([128, 512], bf16)
                    nc.scalar.activation(ha[:], pst[:], AF.Copy)
                    psm = ps_m.tile([128, 512], f32)
                    for g4 in range(GPT):
                        nc.tensor.matmul(
                            psm[:, g4 * 128:][:, :128],
                            ha[:, g4 * 128:][:, :128],
                            atb_t[:, (nt * GPT + g4) * 128:][:, :128],
                            start=True,
                            stop=True,
                        )
                    nc.vector.tensor_copy(msg_t[:, jc * 512:][:, :512], psm[:])
                for jc in range(HC):
                    acc = ps_a2.tile([128, 512], f32)
                    for kc in range(HC):
                        nc.tensor.matmul(
                            acc[:],
                            ws_t[:, kc * H + jc * 128:][:, :128],
                            hA[kc][nt][:],
                            start=(kc == 0),
                            stop=False,
                        )
                    for kc in range(HC):
                        nc.tensor.matmul(
                            acc[:],
                            wn_t[:, kc * H + jc * 128:][:, :128],
                            msg_t[:, kc * 512:][:, :512],
                            start=False,
                            stop=(kc == HC - 1),
                        )
                    tmp = tmppool.tile([128, 512], f32)
                    nc.scalar.activation(tmp[:], acc[:], AF.Relu)
                    nc.vector.tensor_add(
                        out=hB[jc][nt][:], in0=tmp[:], in1=hA[jc][nt][:]
                    )

        # hA (h_text) no longer needed
        hsA.close()

        # ---------------- phase 3: GNN layer 2 (prefix C2) ----------------
        with ExitStack() as p3:
            wpool = p3.enter_context(tc.tile_pool(name="w2", bufs=2))
            a2pool = p3.enter_context(tc.tile_pool(name="a2", bufs=1))
            hapool = p3.enter_context(tc.tile_pool(name="ha2", bufs=3))
            hppool = p3.enter_context(tc.tile_pool(name="hp", bufs=2))
            msgpool = p3.enter_context(tc.tile_pool(name="msg2", bufs=2))
            tmppool = p3.enter_context(tc.tile_pool(name="tmp2", bufs=3))
            ps_t2 = p3.enter_context(tc.tile_pool(name="ps_t3", bufs=2, space="PSUM"))
            ps_m = p3.enter_context(tc.tile_pool(name="ps_m3", bufs=2, space="PSUM"))
            ps_a2 = p3.enter_context(tc.tile_pool(name="ps_a3", bufs=3, space="PSUM"))

            atb2_t = a2pool.tile([128, (NCN // 128) * 4 * C2], bf16)
            nc.sync.dma_start(sb3(atb2_t, 4 * C2), atb2.transpose([1, 0, 2]))

            zpool = p3.enter_context(tc.tile_pool(name="z", bufs=1))
            ps_z = p3.enter_context(tc.tile_pool(name="ps_z", bufs=1, space="PSUM"))

            ws_t = wpool.tile([128, HC * H], bf16, tag="ws2")
            nc.sync.dma_start(ws_t[:], wself[1])
            wn_t = wpool.tile([128, HC * H], bf16, tag="wn2")
            nc.sync.dma_start(wn_t[:], wnbr[1])

            ohd_t = zpool.tile([128, BC], bf16)
            nc.sync.dma_start(ohd_t[:P, :], ohd[:])
            demb_t = zpool.tile([128, H], bf16)
            nc.sync.dma_start(demb_t[:P, :], demb[:])
            bexpl_t = zpool.tile([128, HC], f32)
            nc.sync.dma_start(bexpl_t[:], bexpl[:])
            ext_t = zpool.tile([128, HC * BC], bf16)
            nc.sync.dma_start(sb3(ext_t, BC), chunked(ext, BC))
            wexpl_t = zpool.tile([128, HC * H], bf16)
            nc.sync.dma_start(wexpl_t[:], wexpl[:])

            # h_dist (erT chunks 24-29) and z_teacher (30-35): independent of
            # the GNN; interleave 3 psum-groups per nt2 so drains hide under
            # the layer-2 matmul streams.
            def emit_zdist(zi):
                jc = zi % HC
                if zi < HC:
                    psd = ps_z.tile([128, BC], f32, tag="zz")
                    nc.tensor.matmul(
                        psd[:],
                        demb_t[:P, jc * 128:][:, :128],
                        ohd_t[:P, :],
                        start=True,
                        stop=True,
                    )
                    nc.scalar.activation(
                        erT[:, (24 + jc) * 128:][:, :BC], psd[:], AF.Copy
                    )
                else:
                    psz = ps_z.tile([128, BC], f32, tag="zz")
                    for kc in range(HC):
                        nc.tensor.matmul(
                            psz[:],
                            wexpl_t[:, kc * H + jc * 128:][:, :128],
                            ext_t[:, kc * BC:][:, :BC],
                            start=(kc == 0),
                            stop=(kc == HC - 1),
                        )
                    nc.scalar.activation(
                        erT[:, (30 + jc) * 128:][:, :BC],
                        psz[:],
                        AF.Relu,
                        bias=bexpl_t[:, jc:jc + 1],
                    )
            W2 = 4 * C2  # target cols produced per source row-tile
            SP2 = 512 // (16 * C2)  # source node-tiles feeding one nt2 tile
            for nt2 in range(NT2):
                # message phase over the two full-layout source tiles
                msg_t = msgpool.tile([128, HC * 512], bf16)
                for jc in range(HC):
                    psm = ps_m.tile([128, 512], f32)
                    for half in range(SP2):
                        nt = nt2 * SP2 + half
                        pst = ps_t2.tile([128, 512], bf16)
                        for g4 in range(GPT):
                            nc.tensor.transpose(
                                pst[:, g4 * 128:][:, :128],
                                hB[jc][nt][:, g4 * 128:][:, :128],
                                ident[:],
                            )
                        ha = hapool.tile([128, 512], bf16)
                        nc.scalar.activation(ha[:], pst[:], AF.Copy)
                        for g4 in range(GPT):
                            nc.tensor.matmul(
                                psm[:, (half * GPT + g4) * W2:][:, :W2],
                                ha[:, g4 * 128:][:, :128],
                                atb2_t[:, (nt * GPT + g4) * W2:][:, :W2],
                                start=True,
                                stop=True,
                            )
                    nc.vector.tensor_copy(msg_t[:, jc * 512:][:, :512], psm[:])
                # gather h1 at the C2-prefix of each graph (residual + self rhs)
                hp_t = hppool.tile([128, HC * 512], bf16)
                for jc in range(HC):
                    for half in range(SP2):
                        nt = nt2 * SP2 + half
                        src = hB[jc][nt].rearrange("p (b u) -> p b u", u=P)
                        dst = hp_t[
                            :, jc * 512 + half * (512 // SP2):
                        ][:, :512 // SP2].rearrange("p (b u) -> p b u", u=C2)
                        nc.vector.tensor_copy(dst, src[:, :, :C2])
                for jc in range(HC):
                    acc = ps_a2.tile([128, 512], f32)
                    for kc in range(HC):
                        nc.tensor.matmul(
                            acc[:],
                            ws_t[:, kc * H + jc * 128:][:, :128],
                            hp_t[:, kc * 512:][:, :512],
                            start=(kc == 0),
                            stop=False,
                        )
                    for kc in range(HC):
                        nc.tensor.matmul(
                            acc[:],
                            wn_t[:, kc * H + jc * 128:][:, :128],
                            msg_t[:, kc * 512:][:, :512],
                            start=False,
                            stop=(kc == HC - 1),
                        )
                    tmp = tmppool.tile([128, 512], f32)
                    nc.scalar.activation(tmp[:], acc[:], AF.Relu)
                    nc.vector.tensor_add(
                        out=h2T[jc][nt2][:],
                        in0=tmp[:],
                        in1=hp_t[:, jc * 512:][:, :512],
                    )
                for zi in range(
                    nt2 * 12 // NT2, (nt2 + 1) * 12 // NT2
                ):
                    emit_zdist(zi)

        # hB (h1) no longer needed
        hsB.close()

        # ---------------- phase 4: GNN layer 3 (slots 0,1) + predictor ----
        with ExitStack() as p4:
            ppool = p4.enter_context(tc.tile_pool(name="pred", bufs=1))
            w3pool = p4.enter_context(tc.tile_pool(name="w3", bufs=1))
            a3pool = p4.enter_context(tc.tile_pool(name="a3", bufs=1))
            hapool = p4.enter_context(tc.tile_pool(name="ha3", bufs=3))
            tmppool = p4.enter_context(tc.tile_pool(name="tmp3", bufs=2))
            ps_t2 = p4.enter_context(tc.tile_pool(name="ps_t4", bufs=2, space="PSUM"))
            ps_m = p4.enter_context(tc.tile_pool(name="ps_m4", bufs=1, space="PSUM"))
            ps_c3 = p4.enter_context(tc.tile_pool(name="ps_c3", bufs=2, space="PSUM"))
            ps_p = p4.enter_context(tc.tile_pool(name="ps_p", bufs=3, space="PSUM"))

            # small layer-3 DMAs first -- the 7 MB wp1 prefetch must not
            # block them in the DMA queue (layer 3 needs these immediately;
            # wp1 is consumed ~25 us later by the predictor).
            atb3_t = a3pool.tile([128, RT3 * 2 * GP3], bf16)
            nc.sync.dma_start(sb3(atb3_t, 2 * GP3), atb3.transpose([1, 0, 2]))
            ws3_t = w3pool.tile([128, HC * H], bf16, tag="ws3")
            nc.sync.dma_start(ws3_t[:], wself[2])
            wn3_t = w3pool.tile([128, HC * H], bf16, tag="wn3")
            nc.sync.dma_start(wn3_t[:], wnbr[2])
            bp1_t = ppool.tile([128, HC], f32)
            nc.sync.dma_start(bp1_t[:], bp1[:])
            wp2_t = ppool.tile([128, HC], bf16)
            nc.sync.dma_start(wp2_t[:], wp2[:])
            bp2_t = ppool.tile([1, 1], f32)
            nc.sync.dma_start(bp2_t[:], bp2[:])
            wp1_t = []
            for jc in range(HC):
                w1s = ppool.tile(
                    [128, 36 * 128], bf16, tag=f"wp1_{jc}", name=f"wp1_{jc}"
                )
                nc.sync.dma_start(w1s[:], wp1[jc])
                wp1_t.append(w1s)

            # --- layer 3 message + gather at slots {0,1} ---
            W3 = 2 * GP3  # target cols per source row tile
            msg3_t = ppool.tile([128, HC * 2 * BC], bf16)
            h2p_t = ppool.tile([128, HC * 2 * BC], bf16)
            for jc in range(HC):
                psm = ps_m.tile([128, 2 * BC], f32)
                for nt2 in range(NT2):
                    pst = ps_t2.tile([128, 512], bf16)
                    for g4 in range(GPT):
                        nc.tensor.transpose(
                            pst[:, g4 * 128:][:, :128],
                            h2T[jc][nt2][:, g4 * 128:][:, :128],
                            ident[:],
                        )
                    ha = hapool.tile([128, 512], bf16)
                    nc.scalar.activation(ha[:], pst[:], AF.Copy)
                    for g4 in range(GPT):
                        rt = nt2 * GPT + g4
                        nc.tensor.matmul(
                            psm[:, rt * W3:][:, :W3],
                            ha[:, g4 * 128:][:, :128],
                            atb3_t[:, rt * W3:][:, :W3],
                            start=True,
                            stop=True,
                        )
                nc.vector.tensor_copy(msg3_t[:, jc * 2 * BC:][:, :2 * BC], psm[:])
                PW3 = 2 * BC // NT2
                for nt2 in range(NT2):
                    src = h2T[jc][nt2].rearrange("p (b u) -> p b u", u=C2)
                    dst = h2p_t[
                        :, jc * 2 * BC + nt2 * PW3:
                    ][:, :PW3].rearrange("p (b u) -> p b u", u=2)
                    nc.vector.tensor_copy(dst, src[:, :, :2])

            # --- layer 3 W-matmuls -> h3 -> erT chunks 0-5 (c), 12-17 (t) ---
            for jc in range(HC):
                acc = ps_c3.tile([128, 2 * BC], f32)
                for kc in range(HC):
                    nc.tensor.matmul(
                        acc[:],
                        ws3_t[:, kc * H + jc * 128:][:, :128],
                        h2p_t[:, kc * 2 * BC:][:, :2 * BC],
                        start=(kc == 0),
                        stop=False,
                    )
                for kc in range(HC):
                    nc.tensor.matmul(
                        acc[:],
                        wn3_t[:, kc * H + jc * 128:][:, :128],
                        msg3_t[:, kc * 2 * BC:][:, :2 * BC],
                        start=False,
                        stop=(kc == HC - 1),
                    )
                tmp = tmppool.tile([128, 2 * BC], f32)
                nc.scalar.activation(tmp[:], acc[:], AF.Relu)
                h3 = tmppool.tile([128, 2 * BC], bf16)
                nc.vector.tensor_add(
                    out=h3[:], in0=tmp[:], in1=h2p_t[:, jc * 2 * BC:][:, :2 * BC]
                )
                h3v = h3.rearrange("p (b u) -> p b u", u=2)
                nc.vector.tensor_copy(erT[:, (0 + jc) * 128:][:, :BC], h3v[:, :, 0])
                nc.vector.tensor_copy(erT[:, (12 + jc) * 128:][:, :BC], h3v[:, :, 1])
                nc.vector.copy_predicated(
                    erT[:, (12 + jc) * 128:][:, :BC],
                    cmask_t[:],
                    erT[:, (0 + jc) * 128:][:, :BC],
                )

            hid_t = ppool.tile([128, HC * BC], bf16)
            for jc in range(HC):
                psp = ps_p.tile([128, BC], f32, tag="pp")
                for kc in range(36):
                    nc.tensor.matmul(
                        psp[:],
                        wp1_t[jc][:, kc * 128:][:, :128],
                        erT[:, kc * 128:][:, :128],
                        start=(kc == 0),
                        stop=(kc == 35),
                    )
                nc.scalar.activation(
                    hid_t[:, jc * BC:][:, :BC],
                    psp[:],
                    AF.Relu,
                    bias=bp1_t[:, jc:jc + 1],
                )

            psl = ps_p.tile([128, BC], f32, tag="pp")
            for jc in range(HC):
                nc.tensor.matmul(
                    psl[:1, :],
                    wp2_t[:, jc:jc + 1],
                    hid_t[:, jc * BC:][:, :BC],
                    start=(jc == 0),
                    stop=(jc == HC - 1),
                )
            logit_t = ppool.tile([128, BC], f32)
            nc.vector.tensor_scalar_add(
                out=logit_t[:1, :], in0=psl[:1, :], scalar1=bp2_t[:1, :1]
            )
            nc.sync.dma_start(out_ap[:], logit_t[:1, :])

    nc.compile()
    return nc


def _host_prep(inputs):
    x = np.asarray(inputs["x"], np.float32)
    spk = np.asarray(inputs["speaker_ids"], np.int64)
    emo = np.asarray(inputs["emotion_ids"], np.int64)
    ei = np.asarray(inputs["edge_index"], np.int64)
    tni = np.asarray(inputs["target_node_indices"], np.int64)
    ex = np.asarray(inputs["expl_space_vec"], np.float32)

    E = ei.shape[1]
    edge_src, edge_tgt = ei[0], ei[1]
    c_idx, t_idx = tni[:, 0], tni[:, 1]

    # reference first-edge/dist logic (exact)
    fe = np.full(N, E, np.int64)
    np.minimum.at(fe, edge_src, np.arange(E, dtype=np.int64))

    def first_tgt(q):
        feq = fe[q]
        return np.where(feq < E, edge_tgt[np.minimum(feq, E - 1)], q)

    dist = np.clip(np.abs(first_tgt(c_idx) - first_tgt(t_idx)), 0, P - 1)

    # slot-1 node: t, or a filler distinct from c when c == t
    t_eff = np.where(c_idx == t_idx, (t_idx + 1) % P, t_idx)

    # per-graph in-neighbor sets of {c, t_eff} -> S2 (old coords)
    g_e = edge_src // P
    s_l, t_l = edge_src % P, edge_tgt % P
    innb = np.zeros((B, P, P), bool)
    innb[g_e, t_l, s_l] = True
    sel = np.zeros((B, P), bool)
    bidx = np.arange(B)
    sel[bidx, c_idx] = True
    sel[bidx, t_eff] = True
    S2 = sel.copy()
    S2 |= np.einsum("bts,bt->bs", innb.astype(np.int8), sel.astype(np.int8)) > 0
    s2_max = int(S2.sum(1).max())
    C2 = 16 if s2_max <= 16 else 32

    # per-graph permutation: slot 0 = c, slot 1 = t_eff, S2 within prefix C2
    prio = np.full((B, P), 4, np.int64)
    prio[S2] = 2
    prio[bidx, t_eff] = 1
    prio[bidx, c_idx] = 0
    new2old = np.argsort(prio, axis=1, kind="stable")
    old2new = np.argsort(new2old, axis=1)
    perm_global = (np.arange(B)[:, None] * P + new2old).reshape(-1)

    xtb = np.ascontiguousarray(x[perm_global].T.astype(BF16))  # [DSEM, N]
    spk_new = spk[perm_global]
    emo_new = emo[perm_global]

    oh16 = np.zeros((16, N), BF16)
    oh16[spk_new, np.arange(N)] = 1.0
    oh16[NUM_SPK + emo_new, np.arange(N)] = 1.0

    # adjacency in permuted coords
    s_new = old2new[g_e, s_l]
    t_new = old2new[g_e, t_l]
    A = np.zeros((B, P, P), np.float32)
    np.add.at(A, (g_e, t_new, s_new), 1.0)
    # layer-1 AT tiles: block-diag, 4 graphs per 128x128 tile
    G = B // 4
    atb = np.zeros((G, 128, 128), np.float32)
    Ar = A.reshape(G, 4, P, P)
    for i in range(4):
        atb[:, 32 * i:32 * i + 32, 32 * i:32 * i + 32] = Ar[:, i].transpose(0, 2, 1)
    atb = atb.astype(BF16)
    # layer-2 AT tiles: [tile, 128 src(full layout), 4*C2 tgt(prefix C2)]
    atb2 = np.zeros((G, 128, 4 * C2), np.float32)
    for i in range(4):
        atb2[:, 32 * i:32 * i + 32, C2 * i:C2 * i + C2] = (
            Ar[:, i][:, :C2, :].transpose(0, 2, 1)
        )
    atb2 = atb2.astype(BF16)
    # layer-3 AT tiles: [tile, 128 src(packed C2), 2*gp3 tgt(slots 0,1)]
    gp3 = 128 // C2
    G3 = B // gp3
    atb3 = np.zeros((G3, 128, 2 * gp3), np.float32)
    Ar3 = A.reshape(G3, gp3, P, P)
    for i in range(gp3):
        atb3[:, C2 * i:C2 * i + C2, 2 * i:2 * i + 2] = (
            Ar3[:, i][:, :2, :C2].transpose(0, 2, 1)
        )
    atb3 = atb3.astype(BF16)
    # exactness check: every in-edge of slots {0,1} originates within prefix C2
    assert not A[:, :2, C2:].any()

    cmask = np.tile((c_idx == t_idx).astype(np.uint8)[None, :], (128, 1))

    ohd = np.zeros((P, B), BF16)
    ohd[dist, np.arange(B)] = 1.0

    extT = np.ascontiguousarray(ex.T.astype(BF16))

    embcat = np.concatenate(
        [np.asarray(inputs["spk_emb"], np.float32),
         np.asarray(inputs["emo_emb"], np.float32)], 0
    ).astype(BF16)
    rearr = lambda v: np.ascontiguousarray(
        np.asarray(v, np.float32).reshape(HC, 128).T
    )
    # [K, H] -> [128, (K//128)*H] SBUF-layout slab (contiguous DMA)
    chunk_w = lambda w: np.ascontiguousarray(
        np.asarray(w, np.float32)
        .reshape(-1, 128, w.shape[-1]).transpose(1, 0, 2)
        .reshape(128, -1)
    ).astype(BF16)
    b16 = lambda k: np.asarray(inputs[k], np.float32).astype(BF16)

    shared = dict(
        embcat=embcat,
        wsem=chunk_w(np.asarray(inputs["W_sem"], np.float32)),
        wself=np.stack([
            chunk_w(np.asarray(inputs["gnn_w_self"], np.float32)[l])
            for l in range(L)
        ]),
        wnbr=np.stack([
            chunk_w(np.asarray(inputs["gnn_w_nbr"], np.float32)[l])
            for l in range(L)
        ]),
        demb=b16("dist_emb"),
        wexpl=chunk_w(np.asarray(inputs["W_expl"], np.float32)),
        bexpl=rearr(inputs["b_expl"]),
        wp1=np.ascontiguousarray(
            np.asarray(inputs["W_p1"], np.float32)
            .reshape(36, 128, HC, 128).transpose(2, 1, 0, 3)
            .reshape(HC, 128, 36 * 128)
        ).astype(BF16),
        bp1=rearr(inputs["b_p1"]),
        wp2=rearr(np.asarray(inputs["W_p2"], np.float32)[:, 0]).astype(BF16),
        bp2=np.asarray(inputs["b_p2"], np.float32).reshape(1, 1),
    )

    in_maps = []
    for i in range(NCORES):
        gs = slice(i * BC, (i + 1) * BC)
        ns = slice(i * NCN, (i + 1) * NCN)
        ts = slice(i * (NCN // 128), (i + 1) * (NCN // 128))
        t3 = slice(i * (BC // gp3), (i + 1) * (BC // gp3))
        m = dict(shared)
        m["xt"] = np.ascontiguousarray(
            xtb[:, ns].reshape(8, 128, NT, 512).transpose(2, 1, 0, 3)
            .reshape(NT, 128, 8 * 512)
        )
        m["oh16"] = np.ascontiguousarray(oh16[:, ns])
        m["atb"] = np.ascontiguousarray(atb[ts])
        m["atb2"] = np.ascontiguousarray(atb2[ts])
        m["atb3"] = np.ascontiguousarray(atb3[t3])
        m["cmask"] = np.ascontiguousarray(cmask[:, gs])
        m["ohd"] = np.ascontiguousarray(ohd[:, gs])
        m["ext"] = np.ascontiguousarray(extT[:, gs])
        in_maps.append(m)
    return in_maps, C2


def kernel(**inputs):
    in_maps, C2 = _host_prep(inputs)
    if C2 not in _cache:
        _cache[C2] = _build_program(C2)
    from concourse.bass_utils import run_bass_kernel_spmd

    res = run_bass_kernel_spmd(_cache[C2], in_maps, list(range(NCORES)))
    out = np.concatenate(
        [res.results[i]["out"].reshape(BC) for i in range(NCORES)]
    )
    return out.astype(np.float32)



# revision 2
# speedup vs baseline: 1.0945x; 1.0945x over previous
"""Trainium2 Bass kernel for nn_CrossTowerCausalModel.

Data-parallel over graphs: each of the 8 NeuronCores handles 128 graphs
(128*32 = 4096 nodes, 128*64 = 8192 edges). Weights/embeddings replicated.

Device activation layout is "transposed" (layout B): hT[feature, node] with
the 768 feature dim split into 6 chunks of 128 partitions. Weight matrices
[in, out] then serve directly as matmul lhsT (stationary) operands.

Receptive-field restriction: the GNN output h_graph is only read at 2 nodes
per graph (c, t). Host permutes each graph's 32 node slots so that
  slot 0 = c, slot 1 = t (filler if c == t),
  slots [0, C2) contain T2 = {c,t} U in({c,t}),
  slots [0, C3) contain T3 = T2 U in(T2),
so layer 3 only computes slots {0,1}, layer 2 only the C2-slot prefix
(C2=16), and layer 1 only the C3-slot prefix (C3=24 when the per-graph
|T3| max allows; else 32). Layer-1 messages still read h_text at all 32
slots, so every value read downstream is identical to the full
computation (values at prefix-C3 are exact; layer-2 junk slots beyond T2
lose out-of-prefix sources but are multiplied by structural zeros in A3).

Packed-C3 layout: 16-graph supergroups of 16*C3 = 384 columns (3 full
128-partition blocks), so layer-2 transposes stay block-aligned; graphs
straddling a 128-block boundary are handled with 2-chain psum
accumulation in the layer-2 adjacency matmuls.

h is stored in bf16 (matmul input dtype); per-layer psum accumulation and
relu stay fp32.

Host-side prep (pure index logic + layout, no heavy math):
  * per-graph node permutation (above) -> final gathers h_c / h_t become
    stride-32 strided copies. (c == t graphs fixed up with copy_predicated.)
  * x passed pre-transposed (feature-major) bf16.
  * dense per-graph adjacency as block-diagonal AT tiles: [128, 4*C3]
    (layer 1, full sources -> packed targets), [128, 6*C2] (layer 2,
    packed sources -> packed targets), [128, 2*GP3] (layer 3).
  * the quirky first-edge/dist logic of the reference (exact int math).
  * speaker/emotion one-hots (16 rows) fused into the input projection.
"""

import numpy as np
import ml_dtypes

B = 1024          # graphs
P = 32            # nodes per graph
N = B * P
H = 768
HC = H // 128     # 6 feature chunks
L = 3
DSEM = 1024
NUM_SPK, NUM_EMO = 9, 7
NCORES = 8
BC = B // NCORES          # graphs per core = 128
NCN = BC * P              # nodes per core = 4096
NT = 8                    # node tiles of 512 (= 16 graphs) per core
GPT = 4                   # blocks (of 128 nodes) per node tile
SG = 16                   # graphs per supergroup (= per full node tile)

BF16 = ml_dtypes.bfloat16

_cache = {}


def _a2_plan(C2, C3):
    """Layer-2 adjacency matmul plan for one supergroup (16 graphs).

    Packed rows live at [g*C3, (g+1)*C3) within the 16*C3-row supergroup;
    graphs may straddle 128-row blocks. Returns merged matmul entries
    (block, rhs_col0, ncols, psm_col0, start, stop) with psum 2-chain
    accumulation for straddlers.
    """
    NB = (SG * C3) // 128
    raw = []
    for b in range(NB):
        g0 = (128 * b) // C3
        g1 = min(SG - 1, (128 * b + 127) // C3)
        for g in range(g0, g1 + 1):
            st = C3 * g >= 128 * b             # graph's rows begin here
            sp = C3 * (g + 1) <= 128 * (b + 1)  # graph's rows end here
            raw.append((b, g, st, sp))
    merged = []
    for b, g, st, sp in raw:
        if (merged and merged[-1][0] == b and merged[-1][2] == (st, sp)
                and merged[-1][1][-1] == g - 1):
            merged[-1][1].append(g)
        else:
            merged.append([b, [g], (st, sp)])
    plan = []
    for b, gs, (st, sp) in merged:
        g0 = (128 * b) // C3
        plan.append((b, (gs[0] - g0) * C2, len(gs) * C2, gs[0] * C2, st, sp))
    return plan


def _build_program(C2, C3):
    from contextlib import ExitStack

    import concourse.bacc as bacc
    import concourse.mybir as mybir
    import concourse.tile as tile
    from concourse.masks import make_identity

    f32 = mybir.dt.float32
    bf16 = mybir.dt.bfloat16
    AF = mybir.ActivationFunctionType

    NC2 = BC * C2             # layer-2 packed cols per core
    NT2 = NC2 // 512          # layer-2 node tiles of 512
    RT3 = NC2 // 128          # layer-3 source row tiles
    GP3 = 128 // C2           # graphs per layer-3 source row tile
    NC3 = BC * C3             # layer-1 packed cols per core
    SGW = SG * C3             # packed cols per supergroup
    W1 = 4 * C3               # layer-1 A-matmul target cols per src block
    AW2 = 6 * C2              # layer-2 AT tile col capacity
    NB = SGW // 128           # packed blocks per supergroup
    SGS2 = 512 // (SG * C2)   # supergroups per layer-2 target tile

    nc = bacc.Bacc(
        "TRN2", target_bir_lowering=False, debug=False, num_devices=NCORES
    )

    dram = lambda name, shape, dt: nc.dram_tensor(
        name, shape, dt, kind="ExternalInput"
    ).ap()

    xt = dram("xt", [NT, 128, 8 * 512], bf16)
    oh16 = dram("oh16", [16, NCN], bf16)
    embcat = dram("embcat", [16, H], bf16)
    wsem = dram("wsem", [128, 8 * H], bf16)
    wself = dram("wself", [L, 128, HC * H], bf16)
    wnbr = dram("wnbr", [L, 128, HC * H], bf16)
    atb1 = dram("atb1", [NCN // 128, 128, W1], bf16)
    atb2 = dram("atb2", [NC3 // 128, 128, AW2], bf16)
    atb3 = dram("atb3", [RT3, 128, 2 * GP3], bf16)
    cmask = dram("cmask", [128, BC], mybir.dt.uint8)
    ohd = dram("ohd", [P, BC], bf16)
    demb = dram("demb", [P, H], bf16)
    wexpl = dram("wexpl", [128, HC * H], bf16)
    bexpl = dram("bexpl", [128, HC], f32)
    ext = dram("ext", [H, BC], bf16)
    wp1 = dram("wp1", [HC, 128, 36 * 128], bf16)
    bp1 = dram("bp1", [128, HC], f32)
    wp2 = dram("wp2", [128, HC], bf16)
    bp2 = dram("bp2", [1, 1], f32)
    out_ap = nc.dram_tensor("out", [1, BC], f32, kind="ExternalOutput").ap()

    # [C*128, J] dram AP -> [128, C, J] (partition-major chunked view)
    def chunked(ap, J):
        return ap.rearrange("(c p) j -> c p j", p=128).transpose([1, 0, 2])

    # SBUF tile [128, C*J] -> [128, C, J]
    def sb3(t, J):
        return t[:].rearrange("p (c j) -> p c j", j=J)

    with tile.TileContext(nc) as tc, ExitStack() as ctx:
        erpool = ctx.enter_context(tc.tile_pool(name="er", bufs=1))
        cpool = ctx.enter_context(tc.tile_pool(name="const", bufs=1))

        # pools close in LIFO order: hA (after layer 1), then h1s (after
        # layer 2); h2 lives until the end.
        h2pool = ctx.enter_context(tc.tile_pool(name="h2", bufs=1))
        hsB = ctx.enter_context(ExitStack())     # closes after layer 2
        hBpool = hsB.enter_context(tc.tile_pool(name="hB", bufs=1))
        hsA = ctx.enter_context(ExitStack())     # closes after layer 1
        hApool = hsA.enter_context(tc.tile_pool(name="hA", bufs=1))

        ident = cpool.tile([128, 128], bf16)
        make_identity(nc, ident)
        cmask_t = cpool.tile([128, BC], mybir.dt.uint8)
        nc.sync.dma_start(cmask_t[:], cmask[:])

        # transposed activations: hA = h_text (layer-1 input), full layout,
        # [jc][nt] tiles of [128, 512]; h1s = h1 (layer-2 input), packed-C3
        # slabs of [128, NC3] per jc
        hA = [
            [
                hApool.tile(
                    [128, 512], bf16, tag=f"hA_{jc}_{nt}", name=f"hA_{jc}_{nt}"
                )
                for nt in range(NT)
            ]
            for jc in range(HC)
        ]
        h1s = [
            hBpool.tile([128, NC3], bf16, tag=f"h1_{jc}", name=f"h1_{jc}")
            for jc in range(HC)
        ]
        # layer-2 packed activations: h2T[jc][nt2] is [128, 512] bf16
        h2T = [
            [
                h2pool.tile(
                    [128, 512], bf16, tag=f"h2_{jc}_{nt2}", name=f"h2_{jc}_{nt2}"
                )
                for nt2 in range(NT2)
            ]
            for jc in range(HC)
        ]
        # edge_repr^T, 36 chunks of 128 rows: [h_graph_c, h_text_c, h_graph_t,
        # h_text_t, h_dist, z] each HC chunks wide
        erT = erpool.tile([128, 36 * 128], bf16)

        # ---------------- phase 1: text projection ----------------
        with ExitStack() as p1:
            xtpool = p1.enter_context(tc.tile_pool(name="xt", bufs=3))
            wsem_pool = p1.enter_context(tc.tile_pool(name="wsem", bufs=1))
            oh_pool = p1.enter_context(tc.tile_pool(name="oh16", bufs=3))
            ps_a = p1.enter_context(tc.tile_pool(name="ps_a", bufs=4, space="PSUM"))

            wsem_t = wsem_pool.tile([128, 8 * H], bf16)
            nc.sync.dma_start(wsem_t[:], wsem[:])
            emb_t = wsem_pool.tile([128, H], bf16)
            nc.sync.dma_start(emb_t[:16, :], embcat[:])
            for nt in range(NT):
                oh16_t = oh_pool.tile([128, 512], bf16)
                nc.sync.dma_start(oh16_t[:16, :], oh16[:, nt * 512:][:, :512])
                xt_t = xtpool.tile([128, 8 * 512], bf16)
                nc.sync.dma_start(xt_t[:], xt[nt])
                for jc in range(HC):
                    acc = ps_a.tile([128, 512], f32)
                    for kc in range(8):
                        nc.tensor.matmul(
                            acc[:],
                            wsem_t[:, kc * H + jc * 128:][:, :128],
                            xt_t[:, kc * 512:][:, :512],
                            start=(kc == 0),
                            stop=False,
                        )
                    nc.tensor.matmul(
                        acc[:],
                        emb_t[:16, jc * 128:][:, :128],
                        oh16_t[:16, :],
                        start=False,
                        stop=True,
                    )
                    nc.scalar.activation(hA[jc][nt][:], acc[:], AF.Relu)

            # h_text gathers (chunks 6-11 = h_text_c, 18-23 = h_text_t)
            for jc in range(HC):
                for nt in range(NT):
                    src = hA[jc][nt].rearrange("p (b u) -> p b u", u=P)
                    nc.vector.tensor_copy(
                        erT[:, (6 + jc) * 128 + nt * 16:][:, :16],
                        src[:, :, 0],
                    )
                    nc.vector.tensor_copy(
                        erT[:, (18 + jc) * 128 + nt * 16:][:, :16],
                        src[:, :, 1],
                    )
                nc.vector.copy_predicated(
                    erT[:, (18 + jc) * 128:][:, :BC],
                    cmask_t[:],
                    erT[:, (6 + jc) * 128:][:, :BC],
                )

        # -------- phase 2: GNN layer 1 (full sources -> packed-C3) --------
        with ExitStack() as p2:
            wpool = p2.enter_context(tc.tile_pool(name="w", bufs=2))
            a1pool = p2.enter_context(tc.tile_pool(name="a1", bufs=1))
            hapool = p2.enter_context(tc.tile_pool(name="ha", bufs=3))
            msgpool = p2.enter_context(tc.tile_pool(name="msg", bufs=2))
            tmppool = p2.enter_context(tc.tile_pool(name="tmp", bufs=3))
            ps_t2 = p2.enter_context(tc.tile_pool(name="ps_t2", bufs=2, space="PSUM"))
            ps_m = p2.enter_context(tc.tile_pool(name="ps_m", bufs=2, space="PSUM"))
            ps_a2 = p2.enter_context(tc.tile_pool(name="ps_a2", bufs=3, space="PSUM"))

            atb1_t = a1pool.tile([128, (NCN // 128) * W1], bf16)
            nc.sync.dma_start(sb3(atb1_t, W1), atb1.transpose([1, 0, 2]))

            ws_t = wpool.tile([128, HC * H], bf16, tag="ws")
            nc.sync.dma_start(ws_t[:], wself[0])
            wn_t = wpool.tile([128, HC * H], bf16, tag="wn")
            nc.sync.dma_start(wn_t[:], wnbr[0])
            # full node tile nt == supergroup sg (512 cols = 16 graphs)
            for sg in range(NT):
                msg_t = msgpool.tile([128, HC * SGW], bf16)
                for jc in range(HC):
                    pst = ps_t2.tile([128, 512], bf16)
                    for g4 in range(GPT):
                        nc.tensor.transpose(
                            pst[:, g4 * 128:][:, :128],
                            hA[jc][sg][:, g4 * 128:][:, :128],
                            ident[:],
                        )
                    ha = hapool.tile([128, 512], bf16)
                    nc.scalar.activation(ha[:], pst[:], AF.Copy)
                    psm = ps_m.tile([128, SGW], f32)
                    for g4 in range(GPT):
                        nc.tensor.matmul(
                            psm[:, g4 * W1:][:, :W1],
                            ha[:, g4 * 128:][:, :128],
                            atb1_t[:, (sg * GPT + g4) * W1:][:, :W1],
                            start=True,
                            stop=True,
                        )
                    nc.vector.tensor_copy(msg_t[:, jc * SGW:][:, :SGW], psm[:])
                for jc in range(HC):
                    acc = ps_a2.tile([128, SGW], f32)
                    for kc in range(HC):
                        nc.tensor.matmul(
                            acc[:],
                            ws_t[:, kc * H + jc * 128:][:, :128],
                            hA[kc][sg].rearrange(
                                "p (b u) -> p b u", u=P
                            )[:, :, :C3],
                            start=(kc == 0),
                            stop=False,
                        )
                    for kc in range(HC):
                        nc.tensor.matmul(
                            acc[:],
                            wn_t[:, kc * H + jc * 128:][:, :128],
                            msg_t[:, kc * SGW:][:, :SGW],
                            start=False,
                            stop=(kc == HC - 1),
                        )
                    tmp = tmppool.tile([128, SGW], f32)
                    nc.scalar.activation(tmp[:], acc[:], AF.Relu)
                    nc.vector.tensor_add(
                        out=h1s[jc][:, sg * SGW:][:, :SGW].rearrange(
                            "p (b u) -> p b u", u=C3
                        ),
                        in0=tmp[:].rearrange("p (b u) -> p b u", u=C3),
                        in1=hA[jc][sg].rearrange(
                            "p (b u) -> p b u", u=P
                        )[:, :, :C3],
                    )

        # hA (h_text) no longer needed
        hsA.close()

        # ------- phase 3: GNN layer 2 (packed-C3 sources -> packed-C2) ----
        with ExitStack() as p3:
            wpool = p3.enter_context(tc.tile_pool(name="w2", bufs=2))
            a2pool = p3.enter_context(tc.tile_pool(name="a2", bufs=1))
            hapool = p3.enter_context(tc.tile_pool(name="ha2", bufs=3))
            msgpool = p3.enter_context(tc.tile_pool(name="msg2", bufs=2))
            tmppool = p3.enter_context(tc.tile_pool(name="tmp2", bufs=3))
            ps_t2 = p3.enter_context(tc.tile_pool(name="ps_t3", bufs=2, space="PSUM"))
            ps_m = p3.enter_context(tc.tile_pool(name="ps_m3", bufs=2, space="PSUM"))
            ps_a2 = p3.enter_context(tc.tile_pool(name="ps_a3", bufs=3, space="PSUM"))

            atb2_t = a2pool.tile([128, (NC3 // 128) * AW2], bf16)
            nc.sync.dma_start(sb3(atb2_t, AW2), atb2.transpose([1, 0, 2]))

            zpool = p3.enter_context(tc.tile_pool(name="z", bufs=1))
            ps_z = p3.enter_context(tc.tile_pool(name="ps_z", bufs=1, space="PSUM"))

            ws_t = wpool.tile([128, HC * H], bf16, tag="ws2")
            nc.sync.dma_start(ws_t[:], wself[1])
            wn_t = wpool.tile([128, HC * H], bf16, tag="wn2")
            nc.sync.dma_start(wn_t[:], wnbr[1])

            ohd_t = zpool.tile([128, BC], bf16)
            nc.sync.dma_start(ohd_t[:P, :], ohd[:])
            demb_t = zpool.tile([128, H], bf16)
            nc.sync.dma_start(demb_t[:P, :], demb[:])
            bexpl_t = zpool.tile([128, HC], f32)
            nc.sync.dma_start(bexpl_t[:], bexpl[:])
            ext_t = zpool.tile([128, HC * BC], bf16)
            nc.sync.dma_start(sb3(ext_t, BC), chunked(ext, BC))
            wexpl_t = zpool.tile([128, HC * H], bf16)
            nc.sync.dma_start(wexpl_t[:], wexpl[:])

            # h_dist (erT chunks 24-29) and z_teacher (30-35): independent of
            # the GNN; interleave psum-groups per nt2 so drains hide under
            # the layer-2 matmul streams.
            def emit_zdist(zi):
                jc = zi % HC
                if zi < HC:
                    psd = ps_z.tile([128, BC], f32, tag="zz")
                    nc.tensor.matmul(
                        psd[:],
                        demb_t[:P, jc * 128:][:, :128],
                        ohd_t[:P, :],
                        start=True,
                        stop=True,
                    )
                    nc.scalar.activation(
                        erT[:, (24 + jc) * 128:][:, :BC], psd[:], AF.Copy
                    )
                else:
                    psz = ps_z.tile([128, BC], f32, tag="zz")
                    for kc in range(HC):
                        nc.tensor.matmul(
                            psz[:],
                            wexpl_t[:, kc * H + jc * 128:][:, :128],
                            ext_t[:, kc * BC:][:, :BC],
                            start=(kc == 0),
                            stop=(kc == HC - 1),
                        )
                    nc.scalar.activation(
                        erT[:, (30 + jc) * 128:][:, :BC],
                        psz[:],
                        AF.Relu,
                        bias=bexpl_t[:, jc:jc + 1],
                    )

            plan2 = _a2_plan(C2, C3)
            for nt2 in range(NT2):
                # message phase over the packed-C3 source supergroups
                msg_t = msgpool.tile([128, HC * 512], bf16)
                for jc in range(HC):
                    psm = ps_m.tile([128, 512], f32)
                    for half in range(SGS2):
                        sg = nt2 * SGS2 + half
                        pst = ps_t2.tile([128, NB * 128], bf16)
                        for b in range(NB):
                            nc.tensor.transpose(
                                pst[:, b * 128:][:, :128],
                                h1s[jc][:, sg * SGW + b * 128:][:, :128],
                                ident[:],
                            )
                        ha = hapool.tile([128, NB * 128], bf16)
                        nc.scalar.activation(ha[:], pst[:], AF.Copy)
                        for (b, rc0, ncol, pc0, st, sp) in plan2:
                            nc.tensor.matmul(
                                psm[:, half * SG * C2 + pc0:][:, :ncol],
                                ha[:, b * 128:][:, :128],
                                atb2_t[:, (sg * NB + b) * AW2 + rc0:][:, :ncol],
                                start=st,
                                stop=sp,
                            )
                    nc.vector.tensor_copy(msg_t[:, jc * 512:][:, :512], psm[:])
                # packed-C2 views of h1 (residual + self rhs)
                hpv = lambda kc: h1s[kc][
                    :, nt2 * SGS2 * SGW:
                ][:, :SGS2 * SGW].rearrange("p (b u) -> p b u", u=C3)[:, :, :C2]
                for jc in range(HC):
                    acc = ps_a2.tile([128, 512], f32)
                    for kc in range(HC):
                        nc.tensor.matmul(
                            acc[:],
                            ws_t[:, kc * H + jc * 128:][:, :128],
                            hpv(kc),
                            start=(kc == 0),
                            stop=False,
                        )
                    for kc in range(HC):
                        nc.tensor.matmul(
                            acc[:],
                            wn_t[:, kc * H + jc * 128:][:, :128],
                            msg_t[:, kc * 512:][:, :512],
                            start=False,
                            stop=(kc == HC - 1),
                        )
                    tmp = tmppool.tile([128, 512], f32)
                    nc.scalar.activation(tmp[:], acc[:], AF.Relu)
                    nc.vector.tensor_add(
                        out=h2T[jc][nt2][:].rearrange("p (b u) -> p b u", u=C2),
                        in0=tmp[:].rearrange("p (b u) -> p b u", u=C2),
                        in1=hpv(jc),
                    )
                for zi in range(
                    nt2 * 12 // NT2, (nt2 + 1) * 12 // NT2
                ):
                    emit_zdist(zi)

        # h1s no longer needed
        hsB.close()

        # ---------------- phase 4: GNN layer 3 (slots 0,1) + predictor ----
        with ExitStack() as p4:
            ppool = p4.enter_context(tc.tile_pool(name="pred", bufs=1))
            w3pool = p4.enter_context(tc.tile_pool(name="w3", bufs=1))
            a3pool = p4.enter_context(tc.tile_pool(name="a3", bufs=1))
            hapool = p4.enter_context(tc.tile_pool(name="ha3", bufs=3))
            tmppool = p4.enter_context(tc.tile_pool(name="tmp3", bufs=2))
            ps_t2 = p4.enter_context(tc.tile_pool(name="ps_t4", bufs=2, space="PSUM"))
            ps_m = p4.enter_context(tc.tile_pool(name="ps_m4", bufs=1, space="PSUM"))
            ps_c3 = p4.enter_context(tc.tile_pool(name="ps_c3", bufs=2, space="PSUM"))
            ps_p = p4.enter_context(tc.tile_pool(name="ps_p", bufs=3, space="PSUM"))

            # small layer-3 DMAs first -- the 7 MB wp1 prefetch must not
            # block them in the DMA queue (layer 3 needs these immediately;
            # wp1 is consumed ~25 us later by the predictor).
            atb3_t = a3pool.tile([128, RT3 * 2 * GP3], bf16)
            nc.sync.dma_start(sb3(atb3_t, 2 * GP3), atb3.transpose([1, 0, 2]))
            ws3_t = w3pool.tile([128, HC * H], bf16, tag="ws3")
            nc.sync.dma_start(ws3_t[:], wself[2])
            wn3_t = w3pool.tile([128, HC * H], bf16, tag="wn3")
            nc.sync.dma_start(wn3_t[:], wnbr[2])
            bp1_t = ppool.tile([128, HC], f32)
            nc.sync.dma_start(bp1_t[:], bp1[:])
            wp2_t = ppool.tile([128, HC], bf16)
            nc.sync.dma_start(wp2_t[:], wp2[:])
            bp2_t = ppool.tile([1, 1], f32)
            nc.sync.dma_start(bp2_t[:], bp2[:])
            wp1_t = []
            for jc in range(HC):
                w1s = ppool.tile(
                    [128, 36 * 128], bf16, tag=f"wp1_{jc}", name=f"wp1_{jc}"
                )
                nc.sync.dma_start(w1s[:], wp1[jc])
                wp1_t.append(w1s)

            # --- layer 3 message + gather at slots {0,1} ---
            W3 = 2 * GP3  # target cols per source row tile
            msg3_t = ppool.tile([128, HC * 2 * BC], bf16)
            h2p_t = ppool.tile([128, HC * 2 * BC], bf16)
            for jc in range(HC):
                psm = ps_m.tile([128, 2 * BC], f32)
                for nt2 in range(NT2):
                    pst = ps_t2.tile([128, 512], bf16)
                    for g4 in range(GPT):
                        nc.tensor.transpose(
                            pst[:, g4 * 128:][:, :128],
                            h2T[jc][nt2][:, g4 * 128:][:, :128],
                            ident[:],
                        )
                    ha = hapool.tile([128, 512], bf16)
                    nc.scalar.activation(ha[:], pst[:], AF.Copy)
                    for g4 in range(GPT):
                        rt = nt2 * GPT + g4
                        nc.tensor.matmul(
                            psm[:, rt * W3:][:, :W3],
                            ha[:, g4 * 128:][:, :128],
                            atb3_t[:, rt * W3:][:, :W3],
                            start=True,
                            stop=True,
                        )
                nc.vector.tensor_copy(msg3_t[:, jc * 2 * BC:][:, :2 * BC], psm[:])
                PW3 = 2 * BC // NT2
                for nt2 in range(NT2):
                    src = h2T[jc][nt2].rearrange("p (b u) -> p b u", u=C2)
                    dst = h2p_t[
                        :, jc * 2 * BC + nt2 * PW3:
                    ][:, :PW3].rearrange("p (b u) -> p b u", u=2)
                    nc.vector.tensor_copy(dst, src[:, :, :2])

            # --- layer 3 W-matmuls -> h3 -> erT chunks 0-5 (c), 12-17 (t) ---
            for jc in range(HC):
                acc = ps_c3.tile([128, 2 * BC], f32)
                for kc in range(HC):
                    nc.tensor.matmul(
                        acc[:],
                        ws3_t[:, kc * H + jc * 128:][:, :128],
                        h2p_t[:, kc * 2 * BC:][:, :2 * BC],
                        start=(kc == 0),
                        stop=False,
                    )
                for kc in range(HC):
                    nc.tensor.matmul(
                        acc[:],
                        wn3_t[:, kc * H + jc * 128:][:, :128],
                        msg3_t[:, kc * 2 * BC:][:, :2 * BC],
                        start=False,
                        stop=(kc == HC - 1),
                    )
                tmp = tmppool.tile([128, 2 * BC], f32)
                nc.scalar.activation(tmp[:], acc[:], AF.Relu)
                h3 = tmppool.tile([128, 2 * BC], bf16)
                nc.vector.tensor_add(
                    out=h3[:], in0=tmp[:], in1=h2p_t[:, jc * 2 * BC:][:, :2 * BC]
                )
                h3v = h3.rearrange("p (b u) -> p b u", u=2)
                nc.vector.tensor_copy(erT[:, (0 + jc) * 128:][:, :BC], h3v[:, :, 0])
                nc.vector.tensor_copy(erT[:, (12 + jc) * 128:][:, :BC], h3v[:, :, 1])
                nc.vector.copy_predicated(
                    erT[:, (12 + jc) * 128:][:, :BC],
                    cmask_t[:],
                    erT[:, (0 + jc) * 128:][:, :BC],
                )

            hid_t = ppool.tile([128, HC * BC], bf16)
            for jc in range(HC):
                psp = ps_p.tile([128, BC], f32, tag="pp")
                for kc in range(36):
                    nc.tensor.matmul(
                        psp[:],
                        wp1_t[jc][:, kc * 128:][:, :128],
                        erT[:, kc * 128:][:, :128],
                        start=(kc == 0),
                        stop=(kc == 35),
                    )
                nc.scalar.activation(
                    hid_t[:, jc * BC:][:, :BC],
                    psp[:],
                    AF.Relu,
                    bias=bp1_t[:, jc:jc + 1],
                )

            psl = ps_p.tile([128, BC], f32, tag="pp")
            for jc in range(HC):
                nc.tensor.matmul(
                    psl[:1, :],
                    wp2_t[:, jc:jc + 1],
                    hid_t[:, jc * BC:][:, :BC],
                    start=(jc == 0),
                    stop=(jc == HC - 1),
                )
            logit_t = ppool.tile([128, BC], f32)
            nc.vector.tensor_scalar_add(
                out=logit_t[:1, :], in0=psl[:1, :], scalar1=bp2_t[:1, :1]
            )
            nc.sync.dma_start(out_ap[:], logit_t[:1, :])

    nc.compile()
    return nc


def _host_prep(inputs):
    x = np.asarray(inputs["x"], np.float32)
    spk = np.asarray(inputs["speaker_ids"], np.int64)
    emo = np.asarray(inputs["emotion_ids"], np.int64)
    ei = np.asarray(inputs["edge_index"], np.int64)
    tni = np.asarray(inputs["target_node_indices"], np.int64)
    ex = np.asarray(inputs["expl_space_vec"], np.float32)

    E = ei.shape[1]
    edge_src, edge_tgt = ei[0], ei[1]
    c_idx, t_idx = tni[:, 0], tni[:, 1]

    # reference first-edge/dist logic (exact)
    fe = np.full(N, E, np.int64)
    np.minimum.at(fe, edge_src, np.arange(E, dtype=np.int64))

    def first_tgt(q):
        feq = fe[q]
        return np.where(feq < E, edge_tgt[np.minimum(feq, E - 1)], q)

    dist = np.clip(np.abs(first_tgt(c_idx) - first_tgt(t_idx)), 0, P - 1)

    # slot-1 node: t, or a filler distinct from c when c == t
    t_eff = np.where(c_idx == t_idx, (t_idx + 1) % P, t_idx)

    # per-graph receptive-field sets (old coords):
    # T2 = {c,t} U in({c,t}); T3 = T2 U in(T2)
    g_e = edge_src // P
    s_l, t_l = edge_src % P, edge_tgt % P
    innb = np.zeros((B, P, P), np.int8)
    innb[g_e, t_l, s_l] = 1
    sel = np.zeros((B, P), bool)
    bidx = np.arange(B)
    sel[bidx, c_idx] = True
    sel[bidx, t_eff] = True
    grow = lambda X: X | (np.einsum("bts,bt->bs", innb, X.astype(np.int8)) > 0)
    S2 = grow(sel)
    S3 = grow(S2)
    s2_max = int(S2.sum(1).max())
    s3_max = int(S3.sum(1).max())
    C2 = 16 if s2_max <= 16 else 32
    C3 = max(C2, 24 if s3_max <= 24 else 32)

    # per-graph permutation: slot 0 = c, slot 1 = t_eff, T2 within prefix
    # C2, T3 within prefix C3
    prio = np.full((B, P), 8, np.int64)
    prio[S3] = 3
    prio[S2] = 2
    prio[bidx, t_eff] = 1
    prio[bidx, c_idx] = 0
    new2old = np.argsort(prio, axis=1, kind="stable")
    old2new = np.argsort(new2old, axis=1)
    perm_global = (np.arange(B)[:, None] * P + new2old).reshape(-1)

    xtb = np.ascontiguousarray(x[perm_global].T.astype(BF16))  # [DSEM, N]
    spk_new = spk[perm_global]
    emo_new = emo[perm_global]

    oh16 = np.zeros((16, N), BF16)
    oh16[spk_new, np.arange(N)] = 1.0
    oh16[NUM_SPK + emo_new, np.arange(N)] = 1.0

    # adjacency in permuted coords
    s_new = old2new[g_e, s_l]
    t_new = old2new[g_e, t_l]
    A = np.zeros((B, P, P), np.float32)
    np.add.at(A, (g_e, t_new, s_new), 1.0)

    # exactness checks: every in-edge of slots {0,1} originates within
    # prefix C2, and every in-edge of a true-T2 slot within prefix C3
    assert not A[:, :2, C2:].any()
    t2cnt = S2.sum(1)
    used = np.arange(C2)[None, :] < t2cnt[:, None]
    assert not (A[:, :C2, C3:].any(-1) & used).any()

    # layer-1 AT tiles: full-layout sources (4 graphs per 128-row block),
    # packed-C3 targets: [block, 128 src, 4*C3 tgt]
    G = B // 4
    W1 = 4 * C3
    atb1 = np.zeros((G, 128, W1), np.float32)
    Ar = A.reshape(G, 4, P, P)
    for i in range(4):
        atb1[:, 32 * i:32 * i + 32, C3 * i:C3 * i + C3] = (
            Ar[:, i][:, :C3, :].transpose(0, 2, 1)
        )
    atb1 = atb1.astype(BF16)

    # layer-2 AT tiles: packed-C3 sources (graphs may straddle 128-row
    # blocks), packed-C2 targets: [block, 128 src, 6*C2 tgt]; the col
    # origin of block b is graph g0 = (128*b_local) // C3 (sg-local)
    AW2 = 6 * C2
    NBsg = (SG * C3) // 128
    Asl = A[:, :C2, :C3]                        # [B, tgt, src]
    glv = np.arange(B) % SG                     # graph within supergroup
    sgg = np.arange(B) // SG
    rows = glv[:, None] * C3 + np.arange(C3)[None, :]       # [B, C3]
    bb = rows // 128
    rr = rows % 128
    g0b = (128 * bb) // C3
    colb = (glv[:, None] - g0b) * C2                        # [B, C3]
    atb2 = np.zeros((B // SG * NBsg, 128, AW2), np.float32)
    blk_i = np.broadcast_to(
        (sgg[:, None, None] * NBsg + bb[:, None, :]), Asl.shape
    )
    row_i = np.broadcast_to(rr[:, None, :], Asl.shape)
    col_i = np.broadcast_to(
        colb[:, None, :] + np.arange(C2)[None, :, None], Asl.shape
    )
    atb2[blk_i, row_i, col_i] = Asl
    atb2 = atb2.astype(BF16)

    # layer-3 AT tiles: [tile, 128 src(packed C2), 2*gp3 tgt(slots 0,1)]
    gp3 = 128 // C2
    G3 = B // gp3
    atb3 = np.zeros((G3, 128, 2 * gp3), np.float32)
    Ar3 = A.reshape(G3, gp3, P, P)
    for i in range(gp3):
        atb3[:, C2 * i:C2 * i + C2, 2 * i:2 * i + 2] = (
            Ar3[:, i][:, :2, :C2].transpose(0, 2, 1)
        )
    atb3 = atb3.astype(BF16)

    cmask = np.tile((c_idx == t_idx).astype(np.uint8)[None, :], (128, 1))

    ohd = np.zeros((P, B), BF16)
    ohd[dist, np.arange(B)] = 1.0

    extT = np.ascontiguousarray(ex.T.astype(BF16))

    embcat = np.concatenate(
        [np.asarray(inputs["spk_emb"], np.float32),
         np.asarray(inputs["emo_emb"], np.float32)], 0
    ).astype(BF16)
    rearr = lambda v: np.ascontiguousarray(
        np.asarray(v, np.float32).reshape(HC, 128).T
    )
    # [K, H] -> [128, (K//128)*H] SBUF-layout slab (contiguous DMA)
    chunk_w = lambda w: np.ascontiguousarray(
        np.asarray(w, np.float32)
        .reshape(-1, 128, w.shape[-1]).transpose(1, 0, 2)
        .reshape(128, -1)
    ).astype(BF16)
    b16 = lambda k: np.asarray(inputs[k], np.float32).astype(BF16)

    shared = dict(
        embcat=embcat,
        wsem=chunk_w(np.asarray(inputs["W_sem"], np.float32)),
        wself=np.stack([
            chunk_w(np.asarray(inputs["gnn_w_self"], np.float32)[l])
            for l in range(L)
        ]),
        wnbr=np.stack([
            chunk_w(np.asarray(inputs["gnn_w_nbr"], np.float32)[l])
            for l in range(L)
        ]),
        demb=b16("dist_emb"),
        wexpl=chunk_w(np.asarray(inputs["W_expl"], np.float32)),
        bexpl=rearr(inputs["b_expl"]),
        wp1=np.ascontiguousarray(
            np.asarray(inputs["W_p1"], np.float32)
            .reshape(36, 128, HC, 128).transpose(2, 1, 0, 3)
            .reshape(HC, 128, 36 * 128)
        ).astype(BF16),
        bp1=rearr(inputs["b_p1"]),
        wp2=rearr(np.asarray(inputs["W_p2"], np.float32)[:, 0]).astype(BF16),
        bp2=np.asarray(inputs["b_p2"], np.float32).reshape(1, 1),
    )

    NC3 = BC * C3
    in_maps = []
    for i in range(NCORES):
        gs = slice(i * BC, (i + 1) * BC)
        ns = slice(i * NCN, (i + 1) * NCN)
        ts = slice(i * (NCN // 128), (i + 1) * (NCN // 128))
        t2 = slice(i * (NC3 // 128), (i + 1) * (NC3 // 128))
        t3 = slice(i * (BC // gp3), (i + 1) * (BC // gp3))
        m = dict(shared)
        m["xt"] = np.ascontiguousarray(
            xtb[:, ns].reshape(8, 128, NT, 512).transpose(2, 1, 0, 3)
            .reshape(NT, 128, 8 * 512)
        )
        m["oh16"] = np.ascontiguousarray(oh16[:, ns])
        m["atb1"] = np.ascontiguousarray(atb1[ts])
        m["atb2"] = np.ascontiguousarray(atb2[t2])
        m["atb3"] = np.ascontiguousarray(atb3[t3])
        m["cmask"] = np.ascontiguousarray(cmask[:, gs])
        m["ohd"] = np.ascontiguousarray(ohd[:, gs])
        m["ext"] = np.ascontiguousarray(extT[:, gs])
        in_maps.append(m)
    return in_maps, (C2, C3)


def kernel(**inputs):
    in_maps, key = _host_prep(inputs)
    if key not in _cache:
        _cache[key] = _build_program(*key)
    from concourse.bass_utils import run_bass_kernel_spmd

    res = run_bass_kernel_spmd(_cache[key], in_maps, list(range(NCORES)))
    out = np.concatenate(
        [res.results[i]["out"].reshape(BC) for i in range(NCORES)]
    )
    return out.astype(np.float32)


# revision 13
# speedup vs baseline: 1.1346x; 1.0367x over previous
"""Trainium2 Bass kernel for nn_CrossTowerCausalModel.

Data-parallel over graphs: each of the 8 NeuronCores handles 128 graphs
(128*32 = 4096 nodes, 128*64 = 8192 edges). Weights/embeddings replicated.

Device activation layout is "transposed" (layout B): hT[feature, node] with
the 768 feature dim split into 6 chunks of 128 partitions. Weight matrices
[in, out] then serve directly as matmul lhsT (stationary) operands.

Receptive-field restriction: the GNN output h_graph is only read at 2 nodes
per graph (c, t). Host permutes each graph's 32 node slots so that
  slot 0 = c, slot 1 = t (filler if c == t),
  slots [0, C2) contain T2 = {c,t} U in({c,t}),
  slots [0, C3) contain T3 = T2 U in(T2),
so layer 3 only computes slots {0,1}, layer 2 only the C2-slot prefix
(C2=16), and layer 1 only the C3-slot prefix (C3=24 when the per-graph
|T3| max allows; else 32). Layer-1 messages still read h_text at all 32
slots, so every value read downstream is identical to the full
computation (values at prefix-C3 are exact; layer-2 junk slots beyond T2
lose out-of-prefix sources but are multiplied by structural zeros in A3).

Packed-C3 layout: 16-graph supergroups of 16*C3 = 384 columns (3 full
128-partition blocks), so layer-2 transposes stay block-aligned; graphs
straddling a 128-block boundary are handled with 2-chain psum
accumulation in the layer-2 adjacency matmuls.

h is stored in bf16 (matmul input dtype); per-layer psum accumulation and
relu stay fp32.

Host-side prep (pure index logic + layout, no heavy math):
  * per-graph node permutation (above) -> final gathers h_c / h_t become
    stride-32 strided copies. (c == t graphs fixed up with copy_predicated.)
  * x passed pre-transposed (feature-major) bf16.
  * dense per-graph adjacency as block-diagonal AT tiles: [128, 4*C3]
    (layer 1, full sources -> packed targets), [128, 6*C2] (layer 2,
    packed sources -> packed targets), [128, 2*GP3] (layer 3).
  * the quirky first-edge/dist logic of the reference (exact int math).
  * speaker/emotion one-hots (16 rows) fused into the input projection.
"""

import numpy as np
import ml_dtypes

B = 1024          # graphs
P = 32            # nodes per graph
N = B * P
H = 768
HC = H // 128     # 6 feature chunks
L = 3
DSEM = 1024
NUM_SPK, NUM_EMO = 9, 7
NCORES = 8
BC = B // NCORES          # graphs per core = 128
NCN = BC * P              # nodes per core = 4096
NT = 8                    # node tiles of 512 (= 16 graphs) per core
GPT = 4                   # blocks (of 128 nodes) per node tile
SG = 16                   # graphs per supergroup (= per full node tile)

BF16 = ml_dtypes.bfloat16

_cache = {}


def _a2_plan(C2, C3):
    """Layer-2 adjacency matmul plan for one supergroup (16 graphs).

    Packed rows live at [g*C3, (g+1)*C3) within the 16*C3-row supergroup;
    graphs may straddle 128-row blocks. Returns merged matmul entries
    (block, rhs_col0, ncols, psm_col0, start, stop) with psum 2-chain
    accumulation for straddlers.
    """
    NB = (SG * C3) // 128
    raw = []
    for b in range(NB):
        g0 = (128 * b) // C3
        g1 = min(SG - 1, (128 * b + 127) // C3)
        for g in range(g0, g1 + 1):
            st = C3 * g >= 128 * b             # graph's rows begin here
            sp = C3 * (g + 1) <= 128 * (b + 1)  # graph's rows end here
            raw.append((b, g, st, sp))
    merged = []
    for b, g, st, sp in raw:
        if (merged and merged[-1][0] == b and merged[-1][2] == (st, sp)
                and merged[-1][1][-1] == g - 1):
            merged[-1][1].append(g)
        else:
            merged.append([b, [g], (st, sp)])
    plan = []
    for b, gs, (st, sp) in merged:
        g0 = (128 * b) // C3
        plan.append((b, (gs[0] - g0) * C2, len(gs) * C2, gs[0] * C2, st, sp))
    return plan


def _build_program(C2, C3):
    from contextlib import ExitStack

    import concourse.bacc as bacc
    import concourse.mybir as mybir
    import concourse.tile as tile
    from concourse.masks import make_identity

    f32 = mybir.dt.float32
    bf16 = mybir.dt.bfloat16
    AF = mybir.ActivationFunctionType

    NC2 = BC * C2             # layer-2 packed cols per core
    NT2 = NC2 // 512          # layer-2 node tiles of 512
    RT3 = NC2 // 128          # layer-3 source row tiles
    GP3 = 128 // C2           # graphs per layer-3 source row tile
    NC3 = BC * C3             # layer-1 packed cols per core
    SGW = SG * C3             # packed cols per supergroup
    W1 = 4 * C3               # layer-1 A-matmul target cols per src block
    AW2 = 6 * C2              # layer-2 AT tile col capacity
    NB = SGW // 128           # packed blocks per supergroup
    SGS2 = 512 // (SG * C2)   # supergroups per layer-2 target tile

    nc = bacc.Bacc(
        "TRN2", target_bir_lowering=False, debug=False, num_devices=NCORES
    )

    dram = lambda name, shape, dt: nc.dram_tensor(
        name, shape, dt, kind="ExternalInput"
    ).ap()

    xt = dram("xt", [NT, 8, 128, 512], bf16)
    oh16 = dram("oh16", [NT, 16, 512], bf16)
    embcat = dram("embcat", [16, H], bf16)
    wsem = dram("wsem", [128, 8 * H], bf16)
    wself = dram("wself", [L, 128, HC * H], bf16)
    wnbr = dram("wnbr", [L, 128, HC * H], bf16)
    atb1 = dram("atb1", [NCN // 128, 128, W1], bf16)
    atb2 = dram("atb2", [NC3 // 128, 128, AW2], bf16)
    atb3 = dram("atb3", [RT3, 128, 2 * GP3], bf16)
    cmask = dram("cmask", [128, BC], mybir.dt.uint8)
    ohd = dram("ohd", [P, BC], bf16)
    demb = dram("demb", [P, H], bf16)
    wexpl = dram("wexpl", [128, HC * H], bf16)
    bexpl = dram("bexpl", [128, HC], f32)
    ext = dram("ext", [H, BC], bf16)
    wp1 = dram("wp1", [HC, 128, 36 * 128], bf16)
    bp1 = dram("bp1", [128, HC], f32)
    wp2 = dram("wp2", [128, HC], bf16)
    bp2 = dram("bp2", [1, 1], f32)
    out_ap = nc.dram_tensor("out", [1, BC], f32, kind="ExternalOutput").ap()

    # [C*128, J] dram AP -> [128, C, J] (partition-major chunked view)
    def chunked(ap, J):
        return ap.rearrange("(c p) j -> c p j", p=128).transpose([1, 0, 2])

    # SBUF tile [128, C*J] -> [128, C, J]
    def sb3(t, J):
        return t[:].rearrange("p (c j) -> p c j", j=J)

    with tile.TileContext(nc) as tc, ExitStack() as ctx:
        erpool = ctx.enter_context(tc.tile_pool(name="er", bufs=1))
        cpool = ctx.enter_context(tc.tile_pool(name="const", bufs=1))

        # pools close in LIFO order: hA (after layer 1), then h1s (after
        # layer 2); h2 lives until the end.
        h2pool = ctx.enter_context(tc.tile_pool(name="h2", bufs=1))
        hsB = ctx.enter_context(ExitStack())     # closes after layer 2
        hBpool = hsB.enter_context(tc.tile_pool(name="hB", bufs=1))
        hsA = ctx.enter_context(ExitStack())     # closes after layer 1
        hApool = hsA.enter_context(tc.tile_pool(name="hA", bufs=1))

        ident = cpool.tile([128, 128], bf16)
        make_identity(nc, ident)
        cmask_t = cpool.tile([128, BC], mybir.dt.uint8)
        nc.sync.dma_start(cmask_t[:], cmask[:])

        # transposed activations: hA = h_text (layer-1 input), full layout,
        # [jc][nt] tiles of [128, 512]; h1s = h1 (layer-2 input), packed-C3
        # slabs of [128, NC3] per jc
        hA = [
            [
                hApool.tile(
                    [128, 512], bf16, tag=f"hA_{jc}_{nt}", name=f"hA_{jc}_{nt}"
                )
                for nt in range(NT)
            ]
            for jc in range(HC)
        ]
        h1s = [
            hBpool.tile([128, NC3], bf16, tag=f"h1_{jc}", name=f"h1_{jc}")
            for jc in range(HC)
        ]
        # layer-2 packed activations: h2T[jc][nt2] is [128, 512] bf16
        h2T = [
            [
                h2pool.tile(
                    [128, 512], bf16, tag=f"h2_{jc}_{nt2}", name=f"h2_{jc}_{nt2}"
                )
                for nt2 in range(NT2)
            ]
            for jc in range(HC)
        ]
        # edge_repr^T, 36 chunks of 128 rows: [h_graph_c, h_text_c, h_graph_t,
        # h_text_t, h_dist, z] each HC chunks wide
        erT = erpool.tile([128, 36 * 128], bf16)

        # ---------------- phase 1: text projection ----------------
        with ExitStack() as p1:
            xtpool = p1.enter_context(tc.tile_pool(name="xt", bufs=3))
            wsem_pool = p1.enter_context(tc.tile_pool(name="wsem", bufs=1))
            oh_pool = p1.enter_context(tc.tile_pool(name="oh16", bufs=3))
            ps_a = p1.enter_context(tc.tile_pool(name="ps_a", bufs=6, space="PSUM"))

            # chunked startup DMAs: the DMA ring only comes up ~8us into the
            # program, so the first matmul chain must need as little data as
            # possible -- interleave per-kc wsem/xt chunks for tile 0.
            wsem_ts = []
            xt0_t = xtpool.tile([128, 8 * 512], bf16, tag="xt0")
            for kc in range(8):
                w = wsem_pool.tile([128, H], bf16, tag=f"wsem{kc}")
                nc.sync.dma_start(w[:], wsem[:, kc * H:][:, :H])
                wsem_ts.append(w)
                nc.sync.dma_start(xt0_t[:, kc * 512:][:, :512], xt[0, kc])
            emb_t = wsem_pool.tile([128, H], bf16)
            nc.sync.dma_start(emb_t[:16, :], embcat[:])
            for nt in range(NT):
                oh16_t = oh_pool.tile([128, 512], bf16)
                nc.sync.dma_start(oh16_t[:16, :], oh16[nt])
                if nt == 0:
                    xt_t = xt0_t
                else:
                    xt_t = xtpool.tile([128, 8 * 512], bf16)
                    nc.sync.dma_start(
                        xt_t[:].rearrange("p (k j) -> p k j", j=512),
                        xt[nt].transpose([1, 0, 2]),
                    )
                for jc in range(HC):
                    acc = ps_a.tile([128, 512], f32)
                    for kc in range(8):
                        nc.tensor.matmul(
                            acc[:],
                            wsem_ts[kc][:, jc * 128:][:, :128],
                            xt_t[:, kc * 512:][:, :512],
                            start=(kc == 0),
                            stop=False,
                        )
                    nc.tensor.matmul(
                        acc[:],
                        emb_t[:16, jc * 128:][:, :128],
                        oh16_t[:16, :],
                        start=False,
                        stop=True,
                    )
                    nc.scalar.activation(hA[jc][nt][:], acc[:], AF.Relu)

            # h_text gathers (chunks 6-11 = h_text_c, 18-23 = h_text_t)
            for jc in range(HC):
                for nt in range(NT):
                    src = hA[jc][nt].rearrange("p (b u) -> p b u", u=P)
                    nc.vector.tensor_copy(
                        erT[:, (6 + jc) * 128 + nt * 16:][:, :16],
                        src[:, :, 0],
                    )
                    nc.vector.tensor_copy(
                        erT[:, (18 + jc) * 128 + nt * 16:][:, :16],
                        src[:, :, 1],
                    )
                nc.vector.copy_predicated(
                    erT[:, (18 + jc) * 128:][:, :BC],
                    cmask_t[:],
                    erT[:, (6 + jc) * 128:][:, :BC],
                )

        # -------- phase 2: GNN layer 1 (full sources -> packed-C3) --------
        with ExitStack() as p2:
            wpool = p2.enter_context(tc.tile_pool(name="w", bufs=2))
            a1pool = p2.enter_context(tc.tile_pool(name="a1", bufs=1))
            hapool = p2.enter_context(tc.tile_pool(name="ha", bufs=3))
            msgpool = p2.enter_context(tc.tile_pool(name="msg", bufs=2))
            tmppool = p2.enter_context(tc.tile_pool(name="tmp", bufs=3))
            ps_t2 = p2.enter_context(tc.tile_pool(name="ps_t2", bufs=3, space="PSUM"))
            ps_m = p2.enter_context(tc.tile_pool(name="ps_m", bufs=2, space="PSUM"))
            ps_a2 = p2.enter_context(tc.tile_pool(name="ps_a2", bufs=2, space="PSUM"))

            atb1_t = a1pool.tile([128, (NCN // 128) * W1], bf16)
            nc.sync.dma_start(sb3(atb1_t, W1), atb1.transpose([1, 0, 2]))

            ws_t = wpool.tile([128, HC * H], bf16, tag="ws")
            nc.sync.dma_start(ws_t[:], wself[0])
            wn_t = wpool.tile([128, HC * H], bf16, tag="wn")
            nc.sync.dma_start(wn_t[:], wnbr[0])
            # full node tile nt == supergroup sg (512 cols = 16 graphs)
            for sg in range(NT):
                msg_t = msgpool.tile([128, HC * SGW], bf16)
                has = [None] * HC

                # transpose/copy units lead the A-matmul units by 2 so the
                # psum->sbuf scalar copy hides under later transposes
                def emit_t(jc, sg=sg, has=has):
                    pst = ps_t2.tile([128, 512], bf16, name="pst1", tag="pst1")
                    for g4 in range(GPT):
                        nc.tensor.transpose(
                            pst[:, g4 * 128:][:, :128],
                            hA[jc][sg][:, g4 * 128:][:, :128],
                            ident[:],
                        )
                    ha = hapool.tile([128, 512], bf16, name="ha1", tag="ha1")
                    nc.scalar.activation(ha[:], pst[:], AF.Copy)
                    has[jc] = ha

                def emit_a(jc, sg=sg, has=has, msg_t=msg_t):
                    psm = ps_m.tile([128, SGW], f32, name="psm1", tag="psm1")
                    for g4 in range(GPT):
                        nc.tensor.matmul(
                            psm[:, g4 * W1:][:, :W1],
                            has[jc][:, g4 * 128:][:, :128],
                            atb1_t[:, (sg * GPT + g4) * W1:][:, :W1],
                            start=True,
                            stop=True,
                        )
                    nc.vector.tensor_copy(msg_t[:, jc * SGW:][:, :SGW], psm[:])

                for jc in range(HC + 2):
                    if jc < HC:
                        emit_t(jc)
                    if jc >= 2:
                        emit_a(jc - 2)
                for jc in range(HC):
                    acc = ps_a2.tile([128, SGW], f32)
                    for kc in range(HC):
                        nc.tensor.matmul(
                            acc[:],
                            ws_t[:, kc * H + jc * 128:][:, :128],
                            hA[kc][sg].rearrange(
                                "p (b u) -> p b u", u=P
                            )[:, :, :C3],
                            start=(kc == 0),
                            stop=False,
                        )
                    for kc in range(HC):
                        nc.tensor.matmul(
                            acc[:],
                            wn_t[:, kc * H + jc * 128:][:, :128],
                            msg_t[:, kc * SGW:][:, :SGW],
                            start=False,
                            stop=(kc == HC - 1),
                        )
                    tmp = tmppool.tile([128, SGW], f32)
                    nc.scalar.activation(tmp[:], acc[:], AF.Relu)
                    nc.vector.tensor_add(
                        out=h1s[jc][:, sg * SGW:][:, :SGW].rearrange(
                            "p (b u) -> p b u", u=C3
                        ),
                        in0=tmp[:].rearrange("p (b u) -> p b u", u=C3),
                        in1=hA[jc][sg].rearrange(
                            "p (b u) -> p b u", u=P
                        )[:, :, :C3],
                    )

        # hA (h_text) no longer needed
        hsA.close()

        # ------- phase 3: GNN layer 2 (packed-C3 sources -> packed-C2) ----
        with ExitStack() as p3:
            wpool = p3.enter_context(tc.tile_pool(name="w2", bufs=2))
            a2pool = p3.enter_context(tc.tile_pool(name="a2", bufs=1))
            hapool = p3.enter_context(tc.tile_pool(name="ha2", bufs=3))
            msgpool = p3.enter_context(tc.tile_pool(name="msg2", bufs=2))
            tmppool = p3.enter_context(tc.tile_pool(name="tmp2", bufs=3))
            ps_t2 = p3.enter_context(tc.tile_pool(name="ps_t3", bufs=3, space="PSUM"))
            ps_m = p3.enter_context(tc.tile_pool(name="ps_m3", bufs=2, space="PSUM"))
            ps_a2 = p3.enter_context(tc.tile_pool(name="ps_a3", bufs=2, space="PSUM"))

            atb2_t = a2pool.tile([128, (NC3 // 128) * AW2], bf16)
            nc.sync.dma_start(sb3(atb2_t, AW2), atb2.transpose([1, 0, 2]))

            zpool = p3.enter_context(tc.tile_pool(name="z", bufs=1))
            ps_z = p3.enter_context(tc.tile_pool(name="ps_z", bufs=1, space="PSUM"))

            ws_t = wpool.tile([128, HC * H], bf16, tag="ws2")
            nc.sync.dma_start(ws_t[:], wself[1])
            wn_t = wpool.tile([128, HC * H], bf16, tag="wn2")
            nc.sync.dma_start(wn_t[:], wnbr[1])

            ohd_t = zpool.tile([128, BC], bf16)
            nc.sync.dma_start(ohd_t[:P, :], ohd[:])
            demb_t = zpool.tile([128, H], bf16)
            nc.sync.dma_start(demb_t[:P, :], demb[:])
            bexpl_t = zpool.tile([128, HC], f32)
            nc.sync.dma_start(bexpl_t[:], bexpl[:])
            ext_t = zpool.tile([128, HC * BC], bf16)
            nc.sync.dma_start(sb3(ext_t, BC), chunked(ext, BC))
            wexpl_t = zpool.tile([128, HC * H], bf16)
            nc.sync.dma_start(wexpl_t[:], wexpl[:])

            # h_dist (erT chunks 24-29) and z_teacher (30-35): independent of
            # the GNN; interleave psum-groups per nt2 so drains hide under
            # the layer-2 matmul streams.
            def emit_zdist(zi):
                jc = zi % HC
                if zi < HC:
                    psd = ps_z.tile([128, BC], f32, tag="zz")
                    nc.tensor.matmul(
                        psd[:],
                        demb_t[:P, jc * 128:][:, :128],
                        ohd_t[:P, :],
                        start=True,
                        stop=True,
                    )
                    nc.scalar.activation(
                        erT[:, (24 + jc) * 128:][:, :BC], psd[:], AF.Copy
                    )
                else:
                    psz = ps_z.tile([128, BC], f32, tag="zz")
                    for kc in range(HC):
                        nc.tensor.matmul(
                            psz[:],
                            wexpl_t[:, kc * H + jc * 128:][:, :128],
                            ext_t[:, kc * BC:][:, :BC],
                            start=(kc == 0),
                            stop=(kc == HC - 1),
                        )
                    nc.scalar.activation(
                        erT[:, (30 + jc) * 128:][:, :BC],
                        psz[:],
                        AF.Relu,
                        bias=bexpl_t[:, jc:jc + 1],
                    )

            plan2 = _a2_plan(C2, C3)
            for nt2 in range(NT2):
                # message phase over the packed-C3 source supergroups;
                # transpose/copy units lead A-matmul units by 2 (skew)
                msg_t = msgpool.tile([128, HC * 512], bf16)
                NU = HC * SGS2
                has = [None] * NU
                psms = [None] * HC

                def emit_t(u, nt2=nt2, has=has):
                    jc, half = divmod(u, SGS2)
                    sg = nt2 * SGS2 + half
                    pst = ps_t2.tile(
                        [128, NB * 128], bf16, name="pst2", tag="pst2"
                    )
                    for b in range(NB):
                        nc.tensor.transpose(
                            pst[:, b * 128:][:, :128],
                            h1s[jc][:, sg * SGW + b * 128:][:, :128],
                            ident[:],
                        )
                    ha = hapool.tile(
                        [128, NB * 128], bf16, name="ha2", tag="ha2"
                    )
                    nc.scalar.activation(ha[:], pst[:], AF.Copy)
                    has[u] = ha

                def emit_a(u, nt2=nt2, has=has, psms=psms, msg_t=msg_t):
                    jc, half = divmod(u, SGS2)
                    sg = nt2 * SGS2 + half
                    if half == 0:
                        psms[jc] = ps_m.tile(
                            [128, 512], f32, name="psm2", tag="psm2"
                        )
                    psm = psms[jc]
                    ha = has[u]
                    for (b, rc0, ncol, pc0, st, sp) in plan2:
                        nc.tensor.matmul(
                            psm[:, half * SG * C2 + pc0:][:, :ncol],
                            ha[:, b * 128:][:, :128],
                            atb2_t[:, (sg * NB + b) * AW2 + rc0:][:, :ncol],
                            start=st,
                            stop=sp,
                        )
                    if half == SGS2 - 1:
                        nc.vector.tensor_copy(
                            msg_t[:, jc * 512:][:, :512], psm[:]
                        )

                for u in range(NU + 2):
                    if u < NU:
                        emit_t(u)
                    if u >= 2:
                        emit_a(u - 2)
                # packed-C2 views of h1 (residual + self rhs)
                hpv = lambda kc: h1s[kc][
                    :, nt2 * SGS2 * SGW:
                ][:, :SGS2 * SGW].rearrange("p (b u) -> p b u", u=C3)[:, :, :C2]
                for jc in range(HC):
                    acc = ps_a2.tile([128, 512], f32)
                    for kc in range(HC):
                        nc.tensor.matmul(
                            acc[:],
                            ws_t[:, kc * H + jc * 128:][:, :128],
                            hpv(kc),
                            start=(kc == 0),
                            stop=False,
                        )
                    for kc in range(HC):
                        nc.tensor.matmul(
                            acc[:],
                            wn_t[:, kc * H + jc * 128:][:, :128],
                            msg_t[:, kc * 512:][:, :512],
                            start=False,
                            stop=(kc == HC - 1),
                        )
                    tmp = tmppool.tile([128, 512], f32)
                    nc.scalar.activation(tmp[:], acc[:], AF.Relu)
                    nc.vector.tensor_add(
                        out=h2T[jc][nt2][:].rearrange("p (b u) -> p b u", u=C2),
                        in0=tmp[:].rearrange("p (b u) -> p b u", u=C2),
                        in1=hpv(jc),
                    )
                for zi in range(
                    nt2 * 12 // NT2, (nt2 + 1) * 12 // NT2
                ):
                    emit_zdist(zi)

        # h1s no longer needed
        hsB.close()

        # ---------------- phase 4: GNN layer 3 (slots 0,1) + predictor ----
        with ExitStack() as p4:
            ppool = p4.enter_context(tc.tile_pool(name="pred", bufs=1))
            w3pool = p4.enter_context(tc.tile_pool(name="w3", bufs=1))
            a3pool = p4.enter_context(tc.tile_pool(name="a3", bufs=1))
            hapool = p4.enter_context(tc.tile_pool(name="ha3", bufs=3))
            tmppool = p4.enter_context(tc.tile_pool(name="tmp3", bufs=2))
            ps_t2 = p4.enter_context(tc.tile_pool(name="ps_t4", bufs=3, space="PSUM"))
            ps_m = p4.enter_context(tc.tile_pool(name="ps_m4", bufs=1, space="PSUM"))
            ps_c3 = p4.enter_context(tc.tile_pool(name="ps_c3", bufs=2, space="PSUM"))
            ps_p = p4.enter_context(tc.tile_pool(name="ps_p", bufs=2, space="PSUM"))

            # small layer-3 DMAs first -- the 7 MB wp1 prefetch must not
            # block them in the DMA queue (layer 3 needs these immediately;
            # wp1 is consumed ~25 us later by the predictor).
            atb3_t = a3pool.tile([128, RT3 * 2 * GP3], bf16)
            nc.sync.dma_start(sb3(atb3_t, 2 * GP3), atb3.transpose([1, 0, 2]))
            ws3_t = w3pool.tile([128, HC * H], bf16, tag="ws3")
            nc.sync.dma_start(ws3_t[:], wself[2])
            wn3_t = w3pool.tile([128, HC * H], bf16, tag="wn3")
            nc.sync.dma_start(wn3_t[:], wnbr[2])
            bp1_t = ppool.tile([128, HC], f32)
            nc.sync.dma_start(bp1_t[:], bp1[:])
            wp2_t = ppool.tile([128, HC], bf16)
            nc.sync.dma_start(wp2_t[:], wp2[:])
            bp2_t = ppool.tile([1, 1], f32)
            nc.sync.dma_start(bp2_t[:], bp2[:])
            wp1_t = []
            for jc in range(HC):
                w1s = ppool.tile(
                    [128, 36 * 128], bf16, tag=f"wp1_{jc}", name=f"wp1_{jc}"
                )
                nc.sync.dma_start(w1s[:], wp1[jc])
                wp1_t.append(w1s)

            # --- layer 3 message + gather at slots {0,1} ---
            W3 = 2 * GP3  # target cols per source row tile
            msg3_t = ppool.tile([128, HC * 2 * BC], bf16)
            h2p_t = ppool.tile([128, HC * 2 * BC], bf16)
            for jc in range(HC):
                psm = ps_m.tile([128, 2 * BC], f32)
                has = [None] * NT2

                def emit_t(nt2, jc=jc, has=has):
                    pst = ps_t2.tile([128, 512], bf16, name="pst3", tag="pst3")
                    for g4 in range(GPT):
                        nc.tensor.transpose(
                            pst[:, g4 * 128:][:, :128],
                            h2T[jc][nt2][:, g4 * 128:][:, :128],
                            ident[:],
                        )
                    ha = hapool.tile([128, 512], bf16, name="ha3", tag="ha3")
                    nc.scalar.activation(ha[:], pst[:], AF.Copy)
                    has[nt2] = ha

                def emit_a(nt2, has=has, psm=psm):
                    for g4 in range(GPT):
                        rt = nt2 * GPT + g4
                        nc.tensor.matmul(
                            psm[:, rt * W3:][:, :W3],
                            has[nt2][:, g4 * 128:][:, :128],
                            atb3_t[:, rt * W3:][:, :W3],
                            start=True,
                            stop=True,
                        )

                for u in range(NT2 + 2):
                    if u < NT2:
                        emit_t(u)
                    if u >= 2:
                        emit_a(u - 2)
                nc.vector.tensor_copy(msg3_t[:, jc * 2 * BC:][:, :2 * BC], psm[:])
                PW3 = 2 * BC // NT2
                for nt2 in range(NT2):
                    src = h2T[jc][nt2].rearrange("p (b u) -> p b u", u=C2)
                    dst = h2p_t[
                        :, jc * 2 * BC + nt2 * PW3:
                    ][:, :PW3].rearrange("p (b u) -> p b u", u=2)
                    nc.vector.tensor_copy(dst, src[:, :, :2])

            # --- layer 3 W-matmuls -> h3 -> erT chunks 0-5 (c), 12-17 (t) ---
            for jc in range(HC):
                acc = ps_c3.tile([128, 2 * BC], f32)
                for kc in range(HC):
                    nc.tensor.matmul(
                        acc[:],
                        ws3_t[:, kc * H + jc * 128:][:, :128],
                        h2p_t[:, kc * 2 * BC:][:, :2 * BC],
                        start=(kc == 0),
                        stop=False,
                    )
                for kc in range(HC):
                    nc.tensor.matmul(
                        acc[:],
                        wn3_t[:, kc * H + jc * 128:][:, :128],
                        msg3_t[:, kc * 2 * BC:][:, :2 * BC],
                        start=False,
                        stop=(kc == HC - 1),
                    )
                tmp = tmppool.tile([128, 2 * BC], f32)
                nc.scalar.activation(tmp[:], acc[:], AF.Relu)
                h3 = tmppool.tile([128, 2 * BC], bf16)
                nc.vector.tensor_add(
                    out=h3[:], in0=tmp[:], in1=h2p_t[:, jc * 2 * BC:][:, :2 * BC]
                )
                h3v = h3.rearrange("p (b u) -> p b u", u=2)
                nc.vector.tensor_copy(erT[:, (0 + jc) * 128:][:, :BC], h3v[:, :, 0])
                nc.vector.tensor_copy(erT[:, (12 + jc) * 128:][:, :BC], h3v[:, :, 1])
                nc.vector.copy_predicated(
                    erT[:, (12 + jc) * 128:][:, :BC],
                    cmask_t[:],
                    erT[:, (0 + jc) * 128:][:, :BC],
                )

            hid_t = ppool.tile([128, HC * BC], bf16)
            for jc in range(HC):
                psp = ps_p.tile([128, BC], f32, tag="pp")
                for kc in range(36):
                    nc.tensor.matmul(
                        psp[:],
                        wp1_t[jc][:, kc * 128:][:, :128],
                        erT[:, kc * 128:][:, :128],
                        start=(kc == 0),
                        stop=(kc == 35),
                    )
                nc.scalar.activation(
                    hid_t[:, jc * BC:][:, :BC],
                    psp[:],
                    AF.Relu,
                    bias=bp1_t[:, jc:jc + 1],
                )

            psl = ps_p.tile([128, BC], f32, tag="pp")
            for jc in range(HC):
                nc.tensor.matmul(
                    psl[:1, :],
                    wp2_t[:, jc:jc + 1],
                    hid_t[:, jc * BC:][:, :BC],
                    start=(jc == 0),
                    stop=(jc == HC - 1),
                )
            logit_t = ppool.tile([128, BC], f32)
            nc.vector.tensor_scalar_add(
                out=logit_t[:1, :], in0=psl[:1, :], scalar1=bp2_t[:1, :1]
            )
            nc.sync.dma_start(out_ap[:], logit_t[:1, :])

    nc.compile()
    return nc


def _host_prep(inputs):
    x = np.asarray(inputs["x"], np.float32)
    spk = np.asarray(inputs["speaker_ids"], np.int64)
    emo = np.asarray(inputs["emotion_ids"], np.int64)
    ei = np.asarray(inputs["edge_index"], np.int64)
    tni = np.asarray(inputs["target_node_indices"], np.int64)
    ex = np.asarray(inputs["expl_space_vec"], np.float32)

    E = ei.shape[1]
    edge_src, edge_tgt = ei[0], ei[1]
    c_idx, t_idx = tni[:, 0], tni[:, 1]

    # reference first-edge/dist logic (exact)
    fe = np.full(N, E, np.int64)
    np.minimum.at(fe, edge_src, np.arange(E, dtype=np.int64))

    def first_tgt(q):
        feq = fe[q]
        return np.where(feq < E, edge_tgt[np.minimum(feq, E - 1)], q)

    dist = np.clip(np.abs(first_tgt(c_idx) - first_tgt(t_idx)), 0, P - 1)

    # slot-1 node: t, or a filler distinct from c when c == t
    t_eff = np.where(c_idx == t_idx, (t_idx + 1) % P, t_idx)

    # per-graph receptive-field sets (old coords):
    # T2 = {c,t} U in({c,t}); T3 = T2 U in(T2)
    g_e = edge_src // P
    s_l, t_l = edge_src % P, edge_tgt % P
    innb = np.zeros((B, P, P), np.int8)
    innb[g_e, t_l, s_l] = 1
    sel = np.zeros((B, P), bool)
    bidx = np.arange(B)
    sel[bidx, c_idx] = True
    sel[bidx, t_eff] = True
    grow = lambda X: X | (np.einsum("bts,bt->bs", innb, X.astype(np.int8)) > 0)
    S2 = grow(sel)
    S3 = grow(S2)
    s2_max = int(S2.sum(1).max())
    s3_max = int(S3.sum(1).max())
    C2 = 16 if s2_max <= 16 else 32
    C3 = max(C2, 24 if s3_max <= 24 else 32)

    # per-graph permutation: slot 0 = c, slot 1 = t_eff, T2 within prefix
    # C2, T3 within prefix C3
    prio = np.full((B, P), 8, np.int64)
    prio[S3] = 3
    prio[S2] = 2
    prio[bidx, t_eff] = 1
    prio[bidx, c_idx] = 0
    new2old = np.argsort(prio, axis=1, kind="stable")
    old2new = np.argsort(new2old, axis=1)
    perm_global = (np.arange(B)[:, None] * P + new2old).reshape(-1)

    xtb = np.ascontiguousarray(x[perm_global].T.astype(BF16))  # [DSEM, N]
    spk_new = spk[perm_global]
    emo_new = emo[perm_global]

    oh16 = np.zeros((16, N), BF16)
    oh16[spk_new, np.arange(N)] = 1.0
    oh16[NUM_SPK + emo_new, np.arange(N)] = 1.0

    # adjacency in permuted coords
    s_new = old2new[g_e, s_l]
    t_new = old2new[g_e, t_l]
    A = np.zeros((B, P, P), np.float32)
    np.add.at(A, (g_e, t_new, s_new), 1.0)

    # exactness checks: every in-edge of slots {0,1} originates within
    # prefix C2, and every in-edge of a true-T2 slot within prefix C3
    assert not A[:, :2, C2:].any()
    t2cnt = S2.sum(1)
    used = np.arange(C2)[None, :] < t2cnt[:, None]
    assert not (A[:, :C2, C3:].any(-1) & used).any()

    # layer-1 AT tiles: full-layout sources (4 graphs per 128-row block),
    # packed-C3 targets: [block, 128 src, 4*C3 tgt]
    G = B // 4
    W1 = 4 * C3
    atb1 = np.zeros((G, 128, W1), np.float32)
    Ar = A.reshape(G, 4, P, P)
    for i in range(4):
        atb1[:, 32 * i:32 * i + 32, C3 * i:C3 * i + C3] = (
            Ar[:, i][:, :C3, :].transpose(0, 2, 1)
        )
    atb1 = atb1.astype(BF16)

    # layer-2 AT tiles: packed-C3 sources (graphs may straddle 128-row
    # blocks), packed-C2 targets: [block, 128 src, 6*C2 tgt]; the col
    # origin of block b is graph g0 = (128*b_local) // C3 (sg-local)
    AW2 = 6 * C2
    NBsg = (SG * C3) // 128
    Asl = A[:, :C2, :C3]                        # [B, tgt, src]
    glv = np.arange(B) % SG                     # graph within supergroup
    sgg = np.arange(B) // SG
    rows = glv[:, None] * C3 + np.arange(C3)[None, :]       # [B, C3]
    bb = rows // 128
    rr = rows % 128
    g0b = (128 * bb) // C3
    colb = (glv[:, None] - g0b) * C2                        # [B, C3]
    atb2 = np.zeros((B // SG * NBsg, 128, AW2), np.float32)
    blk_i = np.broadcast_to(
        (sgg[:, None, None] * NBsg + bb[:, None, :]), Asl.shape
    )
    row_i = np.broadcast_to(rr[:, None, :], Asl.shape)
    col_i = np.broadcast_to(
        colb[:, None, :] + np.arange(C2)[None, :, None], Asl.shape
    )
    atb2[blk_i, row_i, col_i] = Asl
    atb2 = atb2.astype(BF16)

    # layer-3 AT tiles: [tile, 128 src(packed C2), 2*gp3 tgt(slots 0,1)]
    gp3 = 128 // C2
    G3 = B // gp3
    atb3 = np.zeros((G3, 128, 2 * gp3), np.float32)
    Ar3 = A.reshape(G3, gp3, P, P)
    for i in range(gp3):
        atb3[:, C2 * i:C2 * i + C2, 2 * i:2 * i + 2] = (
            Ar3[:, i][:, :2, :C2].transpose(0, 2, 1)
        )
    atb3 = atb3.astype(BF16)

    cmask = np.tile((c_idx == t_idx).astype(np.uint8)[None, :], (128, 1))

    ohd = np.zeros((P, B), BF16)
    ohd[dist, np.arange(B)] = 1.0

    extT = np.ascontiguousarray(ex.T.astype(BF16))

    embcat = np.concatenate(
        [np.asarray(inputs["spk_emb"], np.float32),
         np.asarray(inputs["emo_emb"], np.float32)], 0
    ).astype(BF16)
    rearr = lambda v: np.ascontiguousarray(
        np.asarray(v, np.float32).reshape(HC, 128).T
    )
    # [K, H] -> [128, (K//128)*H] SBUF-layout slab (contiguous DMA)
    chunk_w = lambda w: np.ascontiguousarray(
        np.asarray(w, np.float32)
        .reshape(-1, 128, w.shape[-1]).transpose(1, 0, 2)
        .reshape(128, -1)
    ).astype(BF16)
    b16 = lambda k: np.asarray(inputs[k], np.float32).astype(BF16)

    shared = dict(
        embcat=embcat,
        wsem=chunk_w(np.asarray(inputs["W_sem"], np.float32)),
        wself=np.stack([
            chunk_w(np.asarray(inputs["gnn_w_self"], np.float32)[l])
            for l in range(L)
        ]),
        wnbr=np.stack([
            chunk_w(np.asarray(inputs["gnn_w_nbr"], np.float32)[l])
            for l in range(L)
        ]),
        demb=b16("dist_emb"),
        wexpl=chunk_w(np.asarray(inputs["W_expl"], np.float32)),
        bexpl=rearr(inputs["b_expl"]),
        wp1=np.ascontiguousarray(
            np.asarray(inputs["W_p1"], np.float32)
            .reshape(36, 128, HC, 128).transpose(2, 1, 0, 3)
            .reshape(HC, 128, 36 * 128)
        ).astype(BF16),
        bp1=rearr(inputs["b_p1"]),
        wp2=rearr(np.asarray(inputs["W_p2"], np.float32)[:, 0]).astype(BF16),
        bp2=np.asarray(inputs["b_p2"], np.float32).reshape(1, 1),
    )

    NC3 = BC * C3
    in_maps = []
    for i in range(NCORES):
        gs = slice(i * BC, (i + 1) * BC)
        ns = slice(i * NCN, (i + 1) * NCN)
        ts = slice(i * (NCN // 128), (i + 1) * (NCN // 128))
        t2 = slice(i * (NC3 // 128), (i + 1) * (NC3 // 128))
        t3 = slice(i * (BC // gp3), (i + 1) * (BC // gp3))
        m = dict(shared)
        m["xt"] = np.ascontiguousarray(
            xtb[:, ns].reshape(8, 128, NT, 512).transpose(2, 0, 1, 3)
        )
        m["oh16"] = np.ascontiguousarray(
            oh16[:, ns].reshape(16, NT, 512).transpose(1, 0, 2)
        )
        m["atb1"] = np.ascontiguousarray(atb1[ts])
        m["atb2"] = np.ascontiguousarray(atb2[t2])
        m["atb3"] = np.ascontiguousarray(atb3[t3])
        m["cmask"] = np.ascontiguousarray(cmask[:, gs])
        m["ohd"] = np.ascontiguousarray(ohd[:, gs])
        m["ext"] = np.ascontiguousarray(extT[:, gs])
        in_maps.append(m)
    return in_maps, (C2, C3)


def kernel(**inputs):
    in_maps, key = _host_prep(inputs)
    if key not in _cache:
        _cache[key] = _build_program(*key)
    from concourse.bass_utils import run_bass_kernel_spmd

    res = run_bass_kernel_spmd(_cache[key], in_maps, list(range(NCORES)))
    out = np.concatenate(
        [res.results[i]["out"].reshape(BC) for i in range(NCORES)]
    )
    return out.astype(np.float32)


# revision 24
# speedup vs baseline: 1.2688x; 1.1183x over previous
"""Trainium2 Bass kernel for nn_CrossTowerCausalModel.

Data-parallel over graphs: each of the 8 NeuronCores handles 128 graphs
(128*32 = 4096 nodes, 128*64 = 8192 edges). Weights/embeddings replicated.

Device activation layout is "transposed" (layout B): hT[feature, node] with
the 768 feature dim split into 6 chunks of 128 partitions. Weight matrices
[in, out] then serve directly as matmul lhsT (stationary) operands.

Receptive-field restriction: the GNN output h_graph is only read at 2 nodes
per graph (c, t). Host permutes each graph's 32 node slots so that
  slot 0 = c, slot 1 = t (filler if c == t),
  slots [0, C2) contain T2 = {c,t} U in({c,t}),
  slots [0, C3) contain T3 = T2 U in(T2),
so layer 3 only computes slots {0,1}, layer 2 only the C2-slot prefix and
layer 1 only the C3-slot prefix. Layer-1 messages still read h_text at all
32 slots, so every value read downstream is identical to the full
computation (values at prefix-C3 are exact; layer-2 junk slots beyond T2
lose out-of-prefix sources but are multiplied by structural zeros in A3).

Region specialization: graphs are reordered within each core (restored on
output) so that 16-graph supergroups get individual (C2, C3) prefixes --
typically [(8,16)]*4 + [(16,16)]*3 + [(16,24)] -- sized on the host from
the actual per-graph |T2|/|T3|; uniform fallbacks cover adversarial
inputs. Supergroup packed widths (16*C2, 16*C3) are multiples of 128, so
all 128-partition blocks stay supergroup-aligned; graphs straddling a
128-block inside a C3=24 supergroup use 2-chain psum accumulation in the
layer-2 adjacency matmuls.

h is stored in bf16 (matmul input dtype); per-layer psum accumulation and
relu stay fp32.

Host-side prep (pure index logic + layout, no heavy math):
  * per-graph node permutation (above) -> final gathers h_c / h_t become
    strided copies. (c == t graphs fixed up with copy_predicated.)
  * x passed pre-transposed (feature-major) bf16, per-kc chunks so the
    first matmul chain starts as soon as the DMA ring comes up.
  * dense per-graph adjacency as block-diagonal AT tiles (layers 1-3).
  * the quirky first-edge/dist logic of the reference (exact int math).
  * speaker/emotion one-hots (16 rows) fused into the input projection.
"""

import numpy as np
import ml_dtypes

B = 1024          # graphs
P = 32            # nodes per graph
N = B * P
H = 768
HC = H // 128     # 6 feature chunks
L = 3
DSEM = 1024
NUM_SPK, NUM_EMO = 9, 7
NCORES = 8
BC = B // NCORES          # graphs per core = 128
NCN = BC * P              # nodes per core = 4096
NT = 8                    # node tiles of 512 (= 16 graphs) per core
GPT = 4                   # blocks (of 128 nodes) per full node tile
SG = 16                   # graphs per supergroup (= per full node tile)
NSG = BC // SG            # supergroups per core = 8

BF16 = ml_dtypes.bfloat16

_cache = {}


def _a2_plan(C2, C3):
    """Layer-2 adjacency matmul plan for one supergroup (16 graphs).

    Packed rows live at [g*C3, (g+1)*C3) within the 16*C3-row supergroup;
    graphs may straddle 128-row blocks. Returns merged matmul entries
    (block, rhs_col0, ncols, psm_col0, start, stop) with psum 2-chain
    accumulation for straddlers.
    """
    NB = (SG * C3) // 128
    raw = []
    for b in range(NB):
        g0 = (128 * b) // C3
        g1 = min(SG - 1, (128 * b + 127) // C3)
        for g in range(g0, g1 + 1):
            st = C3 * g >= 128 * b             # graph's rows begin here
            sp = C3 * (g + 1) <= 128 * (b + 1)  # graph's rows end here
            raw.append((b, g, st, sp))
    merged = []
    for b, g, st, sp in raw:
        if (merged and merged[-1][0] == b and merged[-1][2] == (st, sp)
                and merged[-1][1][-1] == g - 1):
            merged[-1][1].append(g)
        else:
            merged.append([b, [g], (st, sp)])
    plan = []
    for b, gs, (st, sp) in merged:
        g0 = (128 * b) // C3
        plan.append((b, (gs[0] - g0) * C2, len(gs) * C2, gs[0] * C2, st, sp))
    return plan


def _build_program(cfg):
    from contextlib import ExitStack

    import concourse.bacc as bacc
    import concourse.mybir as mybir
    import concourse.tile as tile
    from concourse.masks import make_identity

    f32 = mybir.dt.float32
    bf16 = mybir.dt.bfloat16
    AF = mybir.ActivationFunctionType

    c2s = [c[0] for c in cfg]
    c3s = [c[1] for c in cfg]
    SGW2 = [SG * c for c in c2s]      # packed-C2 cols per supergroup
    SGW3 = [SG * c for c in c3s]      # packed-C3 cols per supergroup
    off2 = np.concatenate([[0], np.cumsum(SGW2)]).tolist()
    off3 = np.concatenate([[0], np.cumsum(SGW3)]).tolist()
    NC2, NC3 = off2[-1], off3[-1]
    assert NC2 % 512 == 0
    NT2 = NC2 // 512                  # layer-2 target tiles
    # target tile -> supergroups (each sg fully inside one tile)
    tiles2 = [[] for _ in range(NT2)]
    for sg in range(NSG):
        assert off2[sg] // 512 == (off2[sg + 1] - 1) // 512
        tiles2[off2[sg] // 512].append(sg)
    W1M = 4 * max(c3s)                # layer-1 AT tile col capacity
    plans = [_a2_plan(c2s[sg], c3s[sg]) for sg in range(NSG)]
    AW2 = max(e[1] + e[2] for p in plans for e in p)
    NBLK3 = NC3 // 128                # packed-C3 blocks per core
    NBLK2 = NC2 // 128                # packed-C2 blocks per core
    # layer-3 source blocks: (sg, graphs-per-block, first-graph)
    blocks3 = []
    for sg in range(NSG):
        gpb = 128 // c2s[sg]
        for k in range(SGW2[sg] // 128):
            blocks3.append((sg, gpb, sg * SG + k * gpb))
    W3M = 2 * max(128 // c for c in c2s)

    nc = bacc.Bacc(
        "TRN2", target_bir_lowering=False, debug=False, num_devices=NCORES
    )

    dram = lambda name, shape, dt: nc.dram_tensor(
        name, shape, dt, kind="ExternalInput"
    ).ap()

    xt = dram("xt", [NT, 8, 128, 512], bf16)
    oh16 = dram("oh16", [NT, 16, 512], bf16)
    embcat = dram("embcat", [16, H], bf16)
    wsem = dram("wsem", [128, 8 * H], bf16)
    wself = dram("wself", [L, 128, HC * H], bf16)
    wnbr = dram("wnbr", [L, 128, HC * H], bf16)
    atb1 = dram("atb1", [NCN // 128, 128, W1M], bf16)
    atb2 = dram("atb2", [NBLK3, 128, AW2], bf16)
    atb3 = dram("atb3", [NBLK2, 128, W3M], bf16)
    cmask = dram("cmask", [128, BC], mybir.dt.uint8)
    ohd = dram("ohd", [P, BC], bf16)
    demb = dram("demb", [P, H], bf16)
    wexpl = dram("wexpl", [128, HC * H], bf16)
    bexpl = dram("bexpl", [128, HC], f32)
    ext = dram("ext", [H, BC], bf16)
    wp1 = dram("wp1", [HC, 128, 36 * 128], bf16)
    bp1 = dram("bp1", [128, HC], f32)
    wp2 = dram("wp2", [128, HC], bf16)
    bp2 = dram("bp2", [1, 1], f32)
    out_ap = nc.dram_tensor("out", [1, BC], f32, kind="ExternalOutput").ap()
    import os
    _dbg = os.environ.get("KDBG") == "1"
    if _dbg:
        dbg_ap = nc.dram_tensor(
            "dbg", [HC, 128, NC2], bf16, kind="ExternalOutput"
        ).ap()
        dbg1_ap = nc.dram_tensor(
            "dbg1", [HC, 128, NC3], bf16, kind="ExternalOutput"
        ).ap()
        dbgm_ap = nc.dram_tensor(
            "dbgm", [HC, 128, NC2], bf16, kind="ExternalOutput"
        ).ap()

    # [C*128, J] dram AP -> [128, C, J] (partition-major chunked view)
    def chunked(ap, J):
        return ap.rearrange("(c p) j -> c p j", p=128).transpose([1, 0, 2])

    # SBUF tile [128, C*J] -> [128, C, J]
    def sb3(t, J):
        return t[:].rearrange("p (c j) -> p c j", j=J)

    with tile.TileContext(nc) as tc, ExitStack() as ctx:
        erpool = ctx.enter_context(tc.tile_pool(name="er", bufs=1))
        cpool = ctx.enter_context(tc.tile_pool(name="const", bufs=1))

        # pools close in LIFO order: hA (after layer 1), then h1s (after
        # layer 2); h2 lives until the end.
        h2pool = ctx.enter_context(tc.tile_pool(name="h2", bufs=1))
        hsB = ctx.enter_context(ExitStack())     # closes after layer 2
        hBpool = hsB.enter_context(tc.tile_pool(name="hB", bufs=1))
        hsA = ctx.enter_context(ExitStack())     # closes after layer 1
        hApool = hsA.enter_context(tc.tile_pool(name="hA", bufs=1))

        ident = cpool.tile([128, 128], bf16)
        make_identity(nc, ident)
        cmask_t = cpool.tile([128, BC], mybir.dt.uint8)
        nc.sync.dma_start(cmask_t[:], cmask[:])

        # transposed activations: hA = h_text (layer-1 input), full layout,
        # [jc][nt] tiles of [128, 512]; h1s = h1 packed-C3 slab per jc;
        # h2s = h2 packed-C2 slab per jc
        hA = [
            [
                hApool.tile(
                    [128, 512], bf16, tag=f"hA_{jc}_{nt}", name=f"hA_{jc}_{nt}"
                )
                for nt in range(NT)
            ]
            for jc in range(HC)
        ]
        h1s = [
            hBpool.tile([128, NC3], bf16, tag=f"h1_{jc}", name=f"h1_{jc}")
            for jc in range(HC)
        ]
        h2s = [
            h2pool.tile([128, NC2], bf16, tag=f"h2_{jc}", name=f"h2_{jc}")
            for jc in range(HC)
        ]
        # edge_repr^T, 36 chunks of 128 rows: [h_graph_c, h_text_c, h_graph_t,
        # h_text_t, h_dist, z] each HC chunks wide
        erT = erpool.tile([128, 36 * 128], bf16)

        # ---------------- phase 1: text projection ----------------
        with ExitStack() as p1:
            xtpool = p1.enter_context(tc.tile_pool(name="xt", bufs=3))
            wsem_pool = p1.enter_context(tc.tile_pool(name="wsem", bufs=1))
            oh_pool = p1.enter_context(tc.tile_pool(name="oh16", bufs=3))
            ps_a = p1.enter_context(tc.tile_pool(name="ps_a", bufs=6, space="PSUM"))

            # chunked startup DMAs: the DMA ring only comes up ~8us into the
            # program, so the first matmul chain must need as little data as
            # possible -- interleave per-kc wsem/xt chunks for tile 0.
            wsem_ts = []
            xt0_t = xtpool.tile([128, 8 * 512], bf16, tag="xt0")
            for kc in range(8):
                w = wsem_pool.tile([128, H], bf16, tag=f"wsem{kc}")
                nc.sync.dma_start(w[:], wsem[:, kc * H:][:, :H])
                wsem_ts.append(w)
                nc.sync.dma_start(xt0_t[:, kc * 512:][:, :512], xt[0, kc])
            emb_t = wsem_pool.tile([128, H], bf16)
            nc.sync.dma_start(emb_t[:16, :], embcat[:])
            for nt in range(NT):
                oh16_t = oh_pool.tile([128, 512], bf16)
                nc.sync.dma_start(oh16_t[:16, :], oh16[nt])
                if nt == 0:
                    xt_t = xt0_t
                else:
                    xt_t = xtpool.tile([128, 8 * 512], bf16)
                    nc.sync.dma_start(
                        xt_t[:].rearrange("p (k j) -> p k j", j=512),
                        xt[nt].transpose([1, 0, 2]),
                    )
                for jc in range(HC):
                    acc = ps_a.tile([128, 512], f32)
                    for kc in range(8):
                        nc.tensor.matmul(
                            acc[:],
                            wsem_ts[kc][:, jc * 128:][:, :128],
                            xt_t[:, kc * 512:][:, :512],
                            start=(kc == 0),
                            stop=False,
                        )
                    nc.tensor.matmul(
                        acc[:],
                        emb_t[:16, jc * 128:][:, :128],
                        oh16_t[:16, :],
                        start=False,
                        stop=True,
                    )
                    nc.scalar.activation(hA[jc][nt][:], acc[:], AF.Relu)

            # h_text gathers (chunks 6-11 = h_text_c, 18-23 = h_text_t)
            for jc in range(HC):
                for nt in range(NT):
                    src = hA[jc][nt].rearrange("p (b u) -> p b u", u=P)
                    nc.vector.tensor_copy(
                        erT[:, (6 + jc) * 128 + nt * 16:][:, :16],
                        src[:, :, 0],
                    )
                    nc.vector.tensor_copy(
                        erT[:, (18 + jc) * 128 + nt * 16:][:, :16],
                        src[:, :, 1],
                    )
                nc.vector.copy_predicated(
                    erT[:, (18 + jc) * 128:][:, :BC],
                    cmask_t[:],
                    erT[:, (6 + jc) * 128:][:, :BC],
                )

        # -------- phase 2: GNN layer 1 (full sources -> packed-C3) --------
        with ExitStack() as p2:
            wpool = p2.enter_context(tc.tile_pool(name="w", bufs=2))
            a1pool = p2.enter_context(tc.tile_pool(name="a1", bufs=1))
            hapool = p2.enter_context(tc.tile_pool(name="ha", bufs=6))
            msgpool = p2.enter_context(tc.tile_pool(name="msg", bufs=2))
            tmppool = p2.enter_context(tc.tile_pool(name="tmp", bufs=3))
            ps_t2 = p2.enter_context(tc.tile_pool(name="ps_t2", bufs=3, space="PSUM"))
            ps_m = p2.enter_context(tc.tile_pool(name="ps_m", bufs=2, space="PSUM"))
            ps_a2 = p2.enter_context(tc.tile_pool(name="ps_a2", bufs=2, space="PSUM"))

            atb1_t = a1pool.tile([128, (NCN // 128) * W1M], bf16)
            nc.sync.dma_start(sb3(atb1_t, W1M), atb1.transpose([1, 0, 2]))

            ws_t = wpool.tile([128, HC * H], bf16, tag="ws")
            nc.sync.dma_start(ws_t[:], wself[0])
            wn_t = wpool.tile([128, HC * H], bf16, tag="wn")
            nc.sync.dma_start(wn_t[:], wnbr[0])
            # full node tile nt == supergroup sg (512 cols = 16 graphs)
            for sg in range(NSG):
                c3 = c3s[sg]
                sgw = SGW3[sg]
                w1 = 4 * c3
                msg_t = msgpool.tile(
                    [128, HC * sgw], bf16, name="msg1", tag="msg1"
                )
                has = [None] * HC

                # transpose/copy units lead the A-matmul units by 2 so the
                # psum->sbuf scalar copy hides under later transposes
                def emit_t(jc, sg=sg, has=has):
                    pst = ps_t2.tile([128, 512], bf16, name="pst1", tag="pst1")
                    for g4 in range(GPT):
                        nc.tensor.transpose(
                            pst[:, g4 * 128:][:, :128],
                            hA[jc][sg][:, g4 * 128:][:, :128],
                            ident[:],
                        )
                    ha = hapool.tile([128, 512], bf16, name="ha1", tag="ha1")
                    nc.scalar.activation(ha[:], pst[:], AF.Copy)
                    has[jc] = ha

                def emit_a(jc, sg=sg, sgw=sgw, w1=w1, has=has, msg_t=msg_t):
                    psm = ps_m.tile([128, sgw], f32, name="psm1", tag="psm1")
                    for g4 in range(GPT):
                        nc.tensor.matmul(
                            psm[:, g4 * w1:][:, :w1],
                            has[jc][:, g4 * 128:][:, :128],
                            atb1_t[:, (sg * GPT + g4) * W1M:][:, :w1],
                            start=True,
                            stop=True,
                        )
                    nc.vector.tensor_copy(msg_t[:, jc * sgw:][:, :sgw], psm[:])

                for jc in range(HC + 2):
                    if jc < HC:
                        emit_t(jc)
                    if jc >= 2:
                        emit_a(jc - 2)

                for jc in range(HC):
                    acc = ps_a2.tile([128, sgw], f32, name="acc1", tag="acc1")
                    for kc in range(HC):
                        nc.tensor.matmul(
                            acc[:],
                            ws_t[:, kc * H + jc * 128:][:, :128],
                            hA[kc][sg].rearrange(
                                "p (b u) -> p b u", u=P
                            )[:, :, :c3],
                            start=(kc == 0),
                            stop=False,
                        )
                    for kc in range(HC):
                        nc.tensor.matmul(
                            acc[:],
                            wn_t[:, kc * H + jc * 128:][:, :128],
                            msg_t[:, kc * sgw:][:, :sgw],
                            start=False,
                            stop=(kc == HC - 1),
                        )
                    tmp = tmppool.tile([128, sgw], f32, name="tmp1", tag="tmp1")
                    nc.scalar.activation(tmp[:], acc[:], AF.Relu)
                    nc.vector.tensor_add(
                        out=h1s[jc][:, off3[sg]:][:, :sgw].rearrange(
                            "p (b u) -> p b u", u=c3
                        ),
                        in0=tmp[:].rearrange("p (b u) -> p b u", u=c3),
                        in1=hA[jc][sg].rearrange(
                            "p (b u) -> p b u", u=P
                        )[:, :, :c3],
                    )

        # hA (h_text) no longer needed
        hsA.close()

        # ------- phase 3: GNN layer 2 (packed-C3 sources -> packed-C2) ----
        with ExitStack() as p3:
            wpool = p3.enter_context(tc.tile_pool(name="w2", bufs=2))
            a2pool = p3.enter_context(tc.tile_pool(name="a2", bufs=1))
            hapool = p3.enter_context(tc.tile_pool(name="ha2", bufs=6))
            msgpool = p3.enter_context(tc.tile_pool(name="msg2", bufs=2))
            tmppool = p3.enter_context(tc.tile_pool(name="tmp2", bufs=3))
            ps_t2 = p3.enter_context(tc.tile_pool(name="ps_t3", bufs=3, space="PSUM"))
            ps_m = p3.enter_context(tc.tile_pool(name="ps_m3", bufs=2, space="PSUM"))
            ps_a2 = p3.enter_context(tc.tile_pool(name="ps_a3", bufs=2, space="PSUM"))

            atb2_t = a2pool.tile([128, NBLK3 * AW2], bf16)
            nc.sync.dma_start(sb3(atb2_t, AW2), atb2.transpose([1, 0, 2]))

            zpool = p3.enter_context(tc.tile_pool(name="z", bufs=1))
            ps_z = p3.enter_context(tc.tile_pool(name="ps_z", bufs=1, space="PSUM"))

            ws_t = wpool.tile([128, HC * H], bf16, tag="ws2")
            nc.sync.dma_start(ws_t[:], wself[1])
            wn_t = wpool.tile([128, HC * H], bf16, tag="wn2")
            nc.sync.dma_start(wn_t[:], wnbr[1])

            ohd_t = zpool.tile([128, BC], bf16)
            nc.sync.dma_start(ohd_t[:P, :], ohd[:])
            demb_t = zpool.tile([128, H], bf16)
            nc.sync.dma_start(demb_t[:P, :], demb[:])
            bexpl_t = zpool.tile([128, HC], f32)
            nc.sync.dma_start(bexpl_t[:], bexpl[:])
            ext_t = zpool.tile([128, HC * BC], bf16)
            nc.sync.dma_start(sb3(ext_t, BC), chunked(ext, BC))
            wexpl_t = zpool.tile([128, HC * H], bf16)
            nc.sync.dma_start(wexpl_t[:], wexpl[:])

            # h_dist (erT chunks 24-29) and z_teacher (30-35): independent of
            # the GNN; interleave psum-groups per nt2 so drains hide under
            # the layer-2 matmul streams.
            def emit_zdist(zi):
                jc = zi % HC
                if zi < HC:
                    psd = ps_z.tile([128, BC], f32, tag="zz", name="zz")
                    nc.tensor.matmul(
                        psd[:],
                        demb_t[:P, jc * 128:][:, :128],
                        ohd_t[:P, :],
                        start=True,
                        stop=True,
                    )
                    nc.scalar.activation(
                        erT[:, (24 + jc) * 128:][:, :BC], psd[:], AF.Copy
                    )
                else:
                    psz = ps_z.tile([128, BC], f32, tag="zz", name="zz")
                    for kc in range(HC):
                        nc.tensor.matmul(
                            psz[:],
                            wexpl_t[:, kc * H + jc * 128:][:, :128],
                            ext_t[:, kc * BC:][:, :BC],
                            start=(kc == 0),
                            stop=(kc == HC - 1),
                        )
                    nc.scalar.activation(
                        erT[:, (30 + jc) * 128:][:, :BC],
                        psz[:],
                        AF.Relu,
                        bias=bexpl_t[:, jc:jc + 1],
                    )

            for t2 in range(NT2):
                # message phase over this tile's supergroups; transpose/copy
                # units lead A-matmul units by 2 (skew)
                sgs = tiles2[t2]
                msg_t = msgpool.tile(
                    [128, HC * 512], bf16, name="msg2t", tag="msg2t"
                )
                NU = HC * len(sgs)
                has = [None] * NU
                psms = [None] * HC

                def emit_t(u, sgs=sgs, has=has):
                    jc, si = divmod(u, len(sgs))
                    sg = sgs[si]
                    nb = SGW3[sg] // 128
                    pst = ps_t2.tile(
                        [128, nb * 128], bf16, name="pst2", tag="pst2"
                    )
                    for b in range(nb):
                        nc.tensor.transpose(
                            pst[:, b * 128:][:, :128],
                            h1s[jc][:, off3[sg] + b * 128:][:, :128],
                            ident[:],
                        )
                    ha = hapool.tile(
                        [128, nb * 128], bf16, name="ha2", tag="ha2"
                    )
                    nc.scalar.activation(ha[:], pst[:], AF.Copy)
                    has[u] = ha

                def emit_a(u, t2=t2, sgs=sgs, has=has, psms=psms, msg_t=msg_t):
                    jc, si = divmod(u, len(sgs))
                    sg = sgs[si]
                    if si == 0:
                        psms[jc] = ps_m.tile(
                            [128, 512], f32, name="psm2", tag="psm2"
                        )
                    psm = psms[jc]
                    ha = has[u]
                    base = off2[sg] - 512 * t2
                    blk0 = off3[sg] // 128
                    for (b, rc0, ncol, pc0, st, sp) in plans[sg]:
                        nc.tensor.matmul(
                            psm[:, base + pc0:][:, :ncol],
                            ha[:, b * 128:][:, :128],
                            atb2_t[:, (blk0 + b) * AW2 + rc0:][:, :ncol],
                            start=st,
                            stop=sp,
                        )
                    if si == len(sgs) - 1:
                        nc.vector.tensor_copy(
                            msg_t[:, jc * 512:][:, :512], psm[:]
                        )

                for u in range(NU + 2):
                    if u < NU:
                        emit_t(u)
                    if u >= 2:
                        emit_a(u - 2)

                # W-matmuls: self over per-sg packed-C2 views, nbr over msg
                # per-(jc, sg) full-width psum chains: multi-instruction
                # accumulation must cover the whole psum tile (sub-region
                # chains mis-accumulate on HW)
                for jc in range(HC):
                    for sg in sgs:
                        base = off2[sg] - 512 * t2
                        sw2 = SGW2[sg]
                        acc = ps_a2.tile(
                            [128, sw2], f32, name="acc2", tag="acc2"
                        )
                        for kc in range(HC):
                            nc.tensor.matmul(
                                acc[:],
                                ws_t[:, kc * H + jc * 128:][:, :128],
                                h1s[kc][:, off3[sg]:][:, :SGW3[sg]].rearrange(
                                    "p (b u) -> p b u", u=c3s[sg]
                                )[:, :, :c2s[sg]],
                                start=(kc == 0),
                                stop=False,
                            )
                        for kc in range(HC):
                            nc.tensor.matmul(
                                acc[:],
                                wn_t[:, kc * H + jc * 128:][:, :128],
                                msg_t[:, kc * 512 + base:][:, :sw2],
                                start=False,
                                stop=(kc == HC - 1),
                            )
                        tmp = tmppool.tile(
                            [128, sw2], f32, name="tmp2", tag="tmp2"
                        )
                        nc.scalar.activation(tmp[:], acc[:], AF.Relu)
                        nc.vector.tensor_add(
                            out=h2s[jc][:, off2[sg]:][:, :sw2].rearrange(
                                "p (b u) -> p b u", u=c2s[sg]
                            ),
                            in0=tmp[:].rearrange("p (b u) -> p b u", u=c2s[sg]),
                            in1=h1s[jc][:, off3[sg]:][:, :SGW3[sg]].rearrange(
                                "p (b u) -> p b u", u=c3s[sg]
                            )[:, :, :c2s[sg]],
                        )
                if _dbg:
                    for jc in range(HC):
                        nc.sync.dma_start(
                            dbgm_ap[jc][:, t2 * 512:][:, :512],
                            msg_t[:, jc * 512:][:, :512],
                        )
                for zi in range(
                    t2 * 12 // NT2, (t2 + 1) * 12 // NT2
                ):
                    emit_zdist(zi)

        if _dbg:
            for jc in range(HC):
                nc.sync.dma_start(dbg_ap[jc], h2s[jc][:])
                nc.sync.dma_start(dbg1_ap[jc], h1s[jc][:])

        # h1s no longer needed
        hsB.close()

        # ---------------- phase 4: GNN layer 3 (slots 0,1) + predictor ----
        with ExitStack() as p4:
            ppool = p4.enter_context(tc.tile_pool(name="pred", bufs=1))
            w3pool = p4.enter_context(tc.tile_pool(name="w3", bufs=1))
            a3pool = p4.enter_context(tc.tile_pool(name="a3", bufs=1))
            hapool = p4.enter_context(tc.tile_pool(name="ha3", bufs=6))
            tmppool = p4.enter_context(tc.tile_pool(name="tmp3", bufs=2))
            ps_t2 = p4.enter_context(tc.tile_pool(name="ps_t4", bufs=3, space="PSUM"))
            ps_m = p4.enter_context(tc.tile_pool(name="ps_m4", bufs=1, space="PSUM"))
            ps_c3 = p4.enter_context(tc.tile_pool(name="ps_c3", bufs=2, space="PSUM"))
            ps_p = p4.enter_context(tc.tile_pool(name="ps_p", bufs=2, space="PSUM"))

            # small layer-3 DMAs first -- the 7 MB wp1 prefetch must not
            # block them in the DMA queue (layer 3 needs these immediately;
            # wp1 is consumed ~25 us later by the predictor).
            atb3_t = a3pool.tile([128, NBLK2 * W3M], bf16)
            nc.sync.dma_start(sb3(atb3_t, W3M), atb3.transpose([1, 0, 2]))
            ws3_t = w3pool.tile([128, HC * H], bf16, tag="ws3")
            nc.sync.dma_start(ws3_t[:], wself[2])
            wn3_t = w3pool.tile([128, HC * H], bf16, tag="wn3")
            nc.sync.dma_start(wn3_t[:], wnbr[2])
            bp1_t = ppool.tile([128, HC], f32)
            nc.sync.dma_start(bp1_t[:], bp1[:])
            wp2_t = ppool.tile([128, HC], bf16)
            nc.sync.dma_start(wp2_t[:], wp2[:])
            bp2_t = ppool.tile([1, 1], f32)
            nc.sync.dma_start(bp2_t[:], bp2[:])
            wp1_t = []
            for jc in range(HC):
                w1s = ppool.tile(
                    [128, 36 * 128], bf16, tag=f"wp1_{jc}", name=f"wp1_{jc}"
                )
                nc.sync.dma_start(w1s[:], wp1[jc])
                wp1_t.append(w1s)

            # --- layer 3 message + gather at slots {0,1} ---
            msg3_t = ppool.tile([128, HC * 2 * BC], bf16)
            h2p_t = ppool.tile([128, HC * 2 * BC], bf16)
            for jc in range(HC):
                psm = ps_m.tile([128, 2 * BC], f32, name="psm3", tag="psm3")
                has = [None] * NBLK2

                def emit_t(blk, jc=jc, has=has):
                    pst = ps_t2.tile([128, 128], bf16, name="pst3", tag="pst3")
                    nc.tensor.transpose(
                        pst[:], h2s[jc][:, blk * 128:][:, :128], ident[:]
                    )
                    ha = hapool.tile([128, 128], bf16, name="ha3", tag="ha3")
                    nc.scalar.activation(ha[:], pst[:], AF.Copy)
                    has[blk] = ha

                def emit_a(blk, has=has, psm=psm):
                    _, gpb, g0 = blocks3[blk]
                    nc.tensor.matmul(
                        psm[:, 2 * g0:][:, :2 * gpb],
                        has[blk][:],
                        atb3_t[:, blk * W3M:][:, :2 * gpb],
                        start=True,
                        stop=True,
                    )

                for u in range(NBLK2 + 3):
                    if u < NBLK2:
                        emit_t(u)
                    if u >= 3:
                        emit_a(u - 3)

                nc.vector.tensor_copy(msg3_t[:, jc * 2 * BC:][:, :2 * BC], psm[:])
                for sg in range(NSG):
                    src = h2s[jc][:, off2[sg]:][:, :SGW2[sg]].rearrange(
                        "p (b u) -> p b u", u=c2s[sg]
                    )
                    dst = h2p_t[
                        :, jc * 2 * BC + sg * 2 * SG:
                    ][:, :2 * SG].rearrange("p (b u) -> p b u", u=2)
                    nc.vector.tensor_copy(dst, src[:, :, :2])

            # --- layer 3 W-matmuls -> h3 -> erT chunks 0-5 (c), 12-17 (t) ---
            for jc in range(HC):
                acc = ps_c3.tile([128, 2 * BC], f32, name="acc3", tag="acc3")
                for kc in range(HC):
                    nc.tensor.matmul(
                        acc[:],
                        ws3_t[:, kc * H + jc * 128:][:, :128],
                        h2p_t[:, kc * 2 * BC:][:, :2 * BC],
                        start=(kc == 0),
                        stop=False,
                    )
                for kc in range(HC):
                    nc.tensor.matmul(
                        acc[:],
                        wn3_t[:, kc * H + jc * 128:][:, :128],
                        msg3_t[:, kc * 2 * BC:][:, :2 * BC],
                        start=False,
                        stop=(kc == HC - 1),
                    )
                tmp = tmppool.tile([128, 2 * BC], f32, name="tmp3", tag="tmp3")
                nc.scalar.activation(tmp[:], acc[:], AF.Relu)
                h3 = tmppool.tile([128, 2 * BC], bf16, name="h3", tag="h3")
                nc.vector.tensor_add(
                    out=h3[:], in0=tmp[:], in1=h2p_t[:, jc * 2 * BC:][:, :2 * BC]
                )
                h3v = h3.rearrange("p (b u) -> p b u", u=2)
                nc.vector.tensor_copy(erT[:, (0 + jc) * 128:][:, :BC], h3v[:, :, 0])
                nc.vector.tensor_copy(erT[:, (12 + jc) * 128:][:, :BC], h3v[:, :, 1])
                nc.vector.copy_predicated(
                    erT[:, (12 + jc) * 128:][:, :BC],
                    cmask_t[:],
                    erT[:, (0 + jc) * 128:][:, :BC],
                )

            hid_t = ppool.tile([128, HC * BC], bf16)
            for jc in range(HC):
                psp = ps_p.tile([128, BC], f32, tag="pp", name="pp")
                for kc in range(36):
                    nc.tensor.matmul(
                        psp[:],
                        wp1_t[jc][:, kc * 128:][:, :128],
                        erT[:, kc * 128:][:, :128],
                        start=(kc == 0),
                        stop=(kc == 35),
                    )
                nc.scalar.activation(
                    hid_t[:, jc * BC:][:, :BC],
                    psp[:],
                    AF.Relu,
                    bias=bp1_t[:, jc:jc + 1],
                )

            psl = ps_p.tile([128, BC], f32, tag="pp", name="psl")
            for jc in range(HC):
                nc.tensor.matmul(
                    psl[:1, :],
                    wp2_t[:, jc:jc + 1],
                    hid_t[:, jc * BC:][:, :BC],
                    start=(jc == 0),
                    stop=(jc == HC - 1),
                )
            logit_t = ppool.tile([128, BC], f32)
            nc.vector.tensor_scalar_add(
                out=logit_t[:1, :], in0=psl[:1, :], scalar1=bp2_t[:1, :1]
            )
            nc.sync.dma_start(out_ap[:], logit_t[:1, :])

    nc.compile()
    return nc


def _host_prep(inputs):
    x = np.asarray(inputs["x"], np.float32)
    spk = np.asarray(inputs["speaker_ids"], np.int64)
    emo = np.asarray(inputs["emotion_ids"], np.int64)
    ei = np.asarray(inputs["edge_index"], np.int64)
    tni = np.asarray(inputs["target_node_indices"], np.int64)
    ex = np.asarray(inputs["expl_space_vec"], np.float32)

    E = ei.shape[1]
    edge_src, edge_tgt = ei[0], ei[1]
    c_idx, t_idx = tni[:, 0], tni[:, 1]

    # reference first-edge/dist logic (exact)
    fe = np.full(N, E, np.int64)
    np.minimum.at(fe, edge_src, np.arange(E, dtype=np.int64))

    def first_tgt(q):
        feq = fe[q]
        return np.where(feq < E, edge_tgt[np.minimum(feq, E - 1)], q)

    dist = np.clip(np.abs(first_tgt(c_idx) - first_tgt(t_idx)), 0, P - 1)

    # slot-1 node: t, or a filler distinct from c when c == t
    t_eff = np.where(c_idx == t_idx, (t_idx + 1) % P, t_idx)

    # per-graph receptive-field sets (old coords):
    # T2 = {c,t} U in({c,t}); T3 = T2 U in(T2)
    g_e = edge_src // P
    s_l, t_l = edge_src % P, edge_tgt % P
    innb = np.zeros((B, P, P), np.int8)
    innb[g_e, t_l, s_l] = 1
    sel = np.zeros((B, P), bool)
    bidx = np.arange(B)
    sel[bidx, c_idx] = True
    sel[bidx, t_eff] = True
    grow = lambda X: X | (np.einsum("bts,bt->bs", innb, X.astype(np.int8)) > 0)
    S2 = grow(sel)
    S3 = grow(S2)
    t2cnt = S2.sum(1)
    t3cnt = S3.sum(1)

    # per-graph node permutation: slot 0 = c, slot 1 = t_eff, T2 within
    # prefix C2, T3 within prefix C3
    prio = np.full((B, P), 8, np.int64)
    prio[S3] = 3
    prio[S2] = 2
    prio[bidx, t_eff] = 1
    prio[bidx, c_idx] = 0
    new2old = np.argsort(prio, axis=1, kind="stable")
    old2new = np.argsort(new2old, axis=1)

    # adjacency in permuted coords (original graph order)
    s_new = old2new[g_e, s_l]
    t_new = old2new[g_e, t_l]
    A = np.zeros((B, P, P), np.float32)
    np.add.at(A, (g_e, t_new, s_new), 1.0)

    # region config: reorder graphs within each core so light graphs
    # (small T2/T3) land in supergroups with small C2/C3 prefixes
    kcls = np.where(t3cnt > 16, 2, np.where(t2cnt > 8, 1, 0)).reshape(
        NCORES, BC
    )
    n_k2 = (kcls == 2).sum(1).max()
    n_k12 = (kcls >= 1).sum(1).max()
    t2max, t3max = int(t2cnt.max()), int(t3cnt.max())
    import os
    force = os.environ.get("KCFG", "")
    if t2max <= 16 and t3max <= 24 and n_k2 <= SG and n_k12 <= 4 * SG:
        cfg = ((8, 16),) * 4 + ((16, 16),) * 3 + ((16, 24),)
        if force == "c3only":
            cfg = ((16, 16),) * 7 + ((16, 24),)
        elif force == "uniform":
            cfg = ((16, 24),) * 8
        gperm = (
            np.argsort(kcls, axis=1, kind="stable")
            + np.arange(NCORES)[:, None] * BC
        ).reshape(-1)
    else:
        c2u = 16 if t2max <= 16 else 32
        c3u = max(c2u, 24 if t3max <= 24 else 32)
        cfg = ((c2u, c3u),) * NSG
        gperm = np.arange(B)
    c2g = np.tile(np.repeat([c[0] for c in cfg], SG), NCORES)  # per position
    c3g = np.tile(np.repeat([c[1] for c in cfg], SG), NCORES)

    # reorder all per-graph data into position order
    A = A[gperm]
    dist = dist[gperm]
    new2old = new2old[gperm]
    ceqt = (c_idx == t_idx)[gperm]
    ex = ex[gperm]
    t2cnt = t2cnt[gperm]

    # exactness checks: every in-edge of slots {0,1} originates within the
    # graph's prefix C2, and of a true-T2 slot within prefix C3
    srcbad2 = np.arange(P)[None, :] >= c2g[:, None]         # [B, P]
    assert not (A[:, :2, :].any(1) & srcbad2).any()
    usedmask = np.arange(P)[None, :] < t2cnt[:, None]       # true-T2 slots
    srcbad3 = np.arange(P)[None, :] >= c3g[:, None]
    assert not (A * usedmask[:, :, None] * srcbad3[:, None, :]).any()

    perm_global = (gperm[:, None] * P + new2old).reshape(-1)

    xtb = np.ascontiguousarray(x[perm_global].T.astype(BF16))  # [DSEM, N]
    spk_new = spk[perm_global]
    emo_new = emo[perm_global]

    oh16 = np.zeros((16, N), BF16)
    oh16[spk_new, np.arange(N)] = 1.0
    oh16[NUM_SPK + emo_new, np.arange(N)] = 1.0

    # geometry (mirrors _build_program)
    c2s = [c[0] for c in cfg]
    c3s = [c[1] for c in cfg]
    SGW2 = [SG * c for c in c2s]
    SGW3 = [SG * c for c in c3s]
    off2 = np.concatenate([[0], np.cumsum(SGW2)])
    off3 = np.concatenate([[0], np.cumsum(SGW3)])
    NC2, NC3 = int(off2[-1]), int(off3[-1])
    W1M = 4 * max(c3s)
    plans = [_a2_plan(c2s[sg], c3s[sg]) for sg in range(NSG)]
    AW2 = max(e[1] + e[2] for p in plans for e in p)
    NBLK3 = NC3 // 128
    NBLK2 = NC2 // 128
    W3M = 2 * max(128 // c for c in c2s)

    # layer-1 AT tiles: full-layout sources (4 graphs per 128-row block),
    # packed-C3 targets
    nfb = B * P // 128
    atb1 = np.zeros((nfb, 128, W1M), np.float32)
    for i in range(4):
        for sg in range(NSG):          # same cfg on every core
            c3 = c3s[sg]
            fb = (np.arange(B // 4).reshape(NCORES, NSG, 4)[:, sg, :]).ravel()
            gl = fb * 4 + i
            atb1[fb, 32 * i:32 * i + 32, c3 * i:c3 * i + c3] = (
                A[gl][:, :c3, :].transpose(0, 2, 1)
            )
    atb1 = atb1.astype(BF16)

    # layer-2 AT tiles: packed-C3 sources -> packed-C2 targets, blocks are
    # supergroup-aligned; col origin of block b is its first graph
    atb2 = np.zeros((NCORES * NBLK3, 128, AW2), np.float32)
    for core in range(NCORES):
        for sg in range(NSG):
            c2, c3 = c2s[sg], c3s[sg]
            blk0 = core * NBLK3 + int(off3[sg]) // 128
            for gl in range(SG):
                g = core * BC + sg * SG + gl
                rows = gl * c3 + np.arange(c3)
                bb = rows // 128
                rr = rows % 128
                g0b = (128 * bb) // c3
                for t in range(c2):
                    atb2[blk0 + bb, rr, (gl - g0b) * c2 + t] = A[g, t, :c3]
    atb2 = atb2.astype(BF16)

    # layer-3 AT tiles: packed-C2 sources -> slots {0,1}
    atb3 = np.zeros((NCORES * NBLK2, 128, W3M), np.float32)
    for core in range(NCORES):
        for sg in range(NSG):
            c2 = c2s[sg]
            gpb = 128 // c2
            nb = SGW2[sg] // 128
            for k in range(nb):
                blk = core * NBLK2 + int(off2[sg]) // 128 + k
                for i in range(gpb):
                    g = core * BC + sg * SG + k * gpb + i
                    atb3[blk, c2 * i:c2 * i + c2, 2 * i:2 * i + 2] = (
                        A[g, :2, :c2].T
                    )
    atb3 = atb3.astype(BF16)

    cmask = np.tile(ceqt.astype(np.uint8)[None, :], (128, 1))

    ohd = np.zeros((P, B), BF16)
    ohd[dist, np.arange(B)] = 1.0

    extT = np.ascontiguousarray(ex.T.astype(BF16))

    embcat = np.concatenate(
        [np.asarray(inputs["spk_emb"], np.float32),
         np.asarray(inputs["emo_emb"], np.float32)], 0
    ).astype(BF16)
    rearr = lambda v: np.ascontiguousarray(
        np.asarray(v, np.float32).reshape(HC, 128).T
    )
    # [K, H] -> [128, (K//128)*H] SBUF-layout slab (contiguous DMA)
    chunk_w = lambda w: np.ascontiguousarray(
        np.asarray(w, np.float32)
        .reshape(-1, 128, w.shape[-1]).transpose(1, 0, 2)
        .reshape(128, -1)
    ).astype(BF16)
    b16 = lambda k: np.asarray(inputs[k], np.float32).astype(BF16)

    shared = dict(
        embcat=embcat,
        wsem=chunk_w(np.asarray(inputs["W_sem"], np.float32)),
        wself=np.stack([
            chunk_w(np.asarray(inputs["gnn_w_self"], np.float32)[l])
            for l in range(L)
        ]),
        wnbr=np.stack([
            chunk_w(np.asarray(inputs["gnn_w_nbr"], np.float32)[l])
            for l in range(L)
        ]),
        demb=b16("dist_emb"),
        wexpl=chunk_w(np.asarray(inputs["W_expl"], np.float32)),
        bexpl=rearr(inputs["b_expl"]),
        wp1=np.ascontiguousarray(
            np.asarray(inputs["W_p1"], np.float32)
            .reshape(36, 128, HC, 128).transpose(2, 1, 0, 3)
            .reshape(HC, 128, 36 * 128)
        ).astype(BF16),
        bp1=rearr(inputs["b_p1"]),
        wp2=rearr(np.asarray(inputs["W_p2"], np.float32)[:, 0]).astype(BF16),
        bp2=np.asarray(inputs["b_p2"], np.float32).reshape(1, 1),
    )

    in_maps = []
    for i in range(NCORES):
        gs = slice(i * BC, (i + 1) * BC)
        ns = slice(i * NCN, (i + 1) * NCN)
        m = dict(shared)
        m["xt"] = np.ascontiguousarray(
            xtb[:, ns].reshape(8, 128, NT, 512).transpose(2, 0, 1, 3)
        )
        m["oh16"] = np.ascontiguousarray(
            oh16[:, ns].reshape(16, NT, 512).transpose(1, 0, 2)
        )
        m["atb1"] = np.ascontiguousarray(atb1[i * (NCN // 128):][:NCN // 128])
        m["atb2"] = np.ascontiguousarray(atb2[i * NBLK3:][:NBLK3])
        m["atb3"] = np.ascontiguousarray(atb3[i * NBLK2:][:NBLK2])
        m["cmask"] = np.ascontiguousarray(cmask[:, gs])
        m["ohd"] = np.ascontiguousarray(ohd[:, gs])
        m["ext"] = np.ascontiguousarray(extT[:, gs])
        in_maps.append(m)
    return in_maps, cfg, gperm


def kernel(**inputs):
    in_maps, cfg, gperm = _host_prep(inputs)
    if cfg not in _cache:
        _cache[cfg] = _build_program(cfg)
    from concourse.bass_utils import run_bass_kernel_spmd

    res = run_bass_kernel_spmd(_cache[cfg], in_maps, list(range(NCORES)))
    out = np.empty(B, np.float32)
    out[gperm] = np.concatenate(
        [res.results[i]["out"].reshape(BC) for i in range(NCORES)]
    ).astype(np.float32)
    return out


# revision 31
# speedup vs baseline: 1.3288x; 1.0473x over previous
"""Trainium2 Bass kernel for nn_CrossTowerCausalModel.

Data-parallel over graphs: each of the 8 NeuronCores handles 128 graphs
(128*32 = 4096 nodes, 128*64 = 8192 edges). Weights/embeddings replicated.

Device activation layout is "transposed" (layout B): hT[feature, node] with
the 768 feature dim split into 6 chunks of 128 partitions. Weight matrices
[in, out] then serve directly as matmul lhsT (stationary) operands.

Receptive-field restriction: the GNN output h_graph is only read at 2 nodes
per graph (c, t). Host permutes each graph's 32 node slots so that
  slot 0 = c, slot 1 = t (filler if c == t),
  slots [0, C2) contain T2 = {c,t} U in({c,t}),
  slots [0, C3) contain T3 = T2 U in(T2),
so layer 3 only computes slots {0,1}, layer 2 only the C2-slot prefix and
layer 1 only the C3-slot prefix. Layer-1 messages still read h_text at all
32 slots, so every value read downstream is identical to the full
computation (values at prefix-C3 are exact; layer-2 junk slots beyond T2
lose out-of-prefix sources but are multiplied by structural zeros in A3).

Region specialization: graphs are reordered within each core (restored on
output) so that 16-graph supergroups get individual (C2, C3) prefixes --
typically [(8,16)]*4 + [(16,16)]*3 + [(16,24)] -- sized on the host from
the actual per-graph |T2|/|T3|; uniform fallbacks cover adversarial
inputs. Supergroup packed widths (16*C2, 16*C3) are multiples of 128, so
all 128-partition blocks stay supergroup-aligned; graphs straddling a
128-block inside a C3=24 supergroup use 2-chain psum accumulation in the
layer-2 adjacency matmuls.

h is stored in bf16 (matmul input dtype); per-layer psum accumulation and
relu stay fp32.

Host-side prep (pure index logic + layout, no heavy math):
  * per-graph node permutation (above) -> final gathers h_c / h_t become
    strided copies. (c == t graphs fixed up with copy_predicated.)
  * x passed pre-transposed (feature-major) bf16, per-kc chunks so the
    first matmul chain starts as soon as the DMA ring comes up.
  * dense per-graph adjacency as block-diagonal AT tiles (layers 1-3).
  * the quirky first-edge/dist logic of the reference (exact int math).
  * speaker/emotion one-hots (16 rows) fused into the input projection.
"""

import numpy as np
import ml_dtypes

B = 1024          # graphs
P = 32            # nodes per graph
N = B * P
H = 768
HC = H // 128     # 6 feature chunks
L = 3
DSEM = 1024
NUM_SPK, NUM_EMO = 9, 7
NCORES = 8
BC = B // NCORES          # graphs per core = 128
NCN = BC * P              # nodes per core = 4096
NT = 8                    # node tiles of 512 (= 16 graphs) per core
GPT = 4                   # blocks (of 128 nodes) per full node tile
SG = 16                   # graphs per supergroup (= per full node tile)
NSG = BC // SG            # supergroups per core = 8

BF16 = ml_dtypes.bfloat16

_cache = {}


def _a2_plan(C2, C3):
    """Layer-2 adjacency matmul plan for one supergroup (16 graphs).

    Packed rows live at [g*C3, (g+1)*C3) within the 16*C3-row supergroup;
    graphs may straddle 128-row blocks. Returns merged matmul entries
    (block, rhs_col0, ncols, psm_col0, start, stop) with psum 2-chain
    accumulation for straddlers.
    """
    NB = (SG * C3) // 128
    raw = []
    for b in range(NB):
        g0 = (128 * b) // C3
        g1 = min(SG - 1, (128 * b + 127) // C3)
        for g in range(g0, g1 + 1):
            st = C3 * g >= 128 * b             # graph's rows begin here
            sp = C3 * (g + 1) <= 128 * (b + 1)  # graph's rows end here
            raw.append((b, g, st, sp))
    merged = []
    for b, g, st, sp in raw:
        if (merged and merged[-1][0] == b and merged[-1][2] == (st, sp)
                and merged[-1][1][-1] == g - 1):
            merged[-1][1].append(g)
        else:
            merged.append([b, [g], (st, sp)])
    plan = []
    for b, gs, (st, sp) in merged:
        g0 = (128 * b) // C3
        plan.append((b, (gs[0] - g0) * C2, len(gs) * C2, gs[0] * C2, st, sp))
    return plan


def _build_program(cfg):
    from contextlib import ExitStack

    import concourse.bacc as bacc
    import concourse.mybir as mybir
    import concourse.tile as tile
    from concourse.masks import make_identity

    f32 = mybir.dt.float32
    bf16 = mybir.dt.bfloat16
    AF = mybir.ActivationFunctionType

    c2s = [c[0] for c in cfg]
    c3s = [c[1] for c in cfg]
    SGW2 = [SG * c for c in c2s]      # packed-C2 cols per supergroup
    SGW3 = [SG * c for c in c3s]      # packed-C3 cols per supergroup
    off2 = np.concatenate([[0], np.cumsum(SGW2)]).tolist()
    off3 = np.concatenate([[0], np.cumsum(SGW3)]).tolist()
    NC2, NC3 = off2[-1], off3[-1]
    assert NC2 % 512 == 0
    NT2 = NC2 // 512                  # layer-2 target tiles
    # target tile -> supergroups (each sg fully inside one tile)
    tiles2 = [[] for _ in range(NT2)]
    for sg in range(NSG):
        assert off2[sg] // 512 == (off2[sg + 1] - 1) // 512
        tiles2[off2[sg] // 512].append(sg)
    W1M = 4 * max(c3s)                # layer-1 AT tile col capacity
    plans = [_a2_plan(c2s[sg], c3s[sg]) for sg in range(NSG)]
    AW2 = max(e[1] + e[2] for p in plans for e in p)
    NBLK3 = NC3 // 128                # packed-C3 blocks per core
    NBLK2 = NC2 // 128                # packed-C2 blocks per core
    # layer-3 source blocks: (sg, graphs-per-block, first-graph)
    blocks3 = []
    for sg in range(NSG):
        gpb = 128 // c2s[sg]
        for k in range(SGW2[sg] // 128):
            blocks3.append((sg, gpb, sg * SG + k * gpb))
    W3M = 2 * max(128 // c for c in c2s)

    nc = bacc.Bacc(
        "TRN2", target_bir_lowering=False, debug=False, num_devices=NCORES
    )

    dram = lambda name, shape, dt: nc.dram_tensor(
        name, shape, dt, kind="ExternalInput"
    ).ap()

    xt = dram("xt", [NT, 8, 128, 512], bf16)
    embt = dram("embt", [NT, HC, 128, 512], bf16)
    wsem = dram("wsem", [128, 8 * H], bf16)
    wself = dram("wself", [L, 128, HC * H], bf16)
    wnbr = dram("wnbr", [L, 128, HC * H], bf16)
    atb1 = dram("atb1", [NCN // 128, 128, W1M], bf16)
    atb2 = dram("atb2", [NBLK3, 128, AW2], bf16)
    atb3 = dram("atb3", [NBLK2, 128, W3M], bf16)
    cmask = dram("cmask", [128, BC], mybir.dt.uint8)
    ohd = dram("ohd", [P, BC], bf16)
    demb = dram("demb", [P, H], bf16)
    wexpl = dram("wexpl", [128, HC * H], bf16)
    bexpl = dram("bexpl", [128, HC], f32)
    ext = dram("ext", [H, BC], bf16)
    wp1 = dram("wp1", [HC, 128, 36 * 128], bf16)
    bp1 = dram("bp1", [128, HC], f32)
    wp2 = dram("wp2", [128, HC], bf16)
    bp2 = dram("bp2", [1, 1], f32)
    out_ap = nc.dram_tensor("out", [1, BC], f32, kind="ExternalOutput").ap()
    import os
    _dbg = os.environ.get("KDBG") == "1"
    if _dbg:
        dbg_ap = nc.dram_tensor(
            "dbg", [HC, 128, NC2], bf16, kind="ExternalOutput"
        ).ap()
        dbg1_ap = nc.dram_tensor(
            "dbg1", [HC, 128, NC3], bf16, kind="ExternalOutput"
        ).ap()
        dbgm_ap = nc.dram_tensor(
            "dbgm", [HC, 128, NC2], bf16, kind="ExternalOutput"
        ).ap()

    # [C*128, J] dram AP -> [128, C, J] (partition-major chunked view)
    def chunked(ap, J):
        return ap.rearrange("(c p) j -> c p j", p=128).transpose([1, 0, 2])

    # SBUF tile [128, C*J] -> [128, C, J]
    def sb3(t, J):
        return t[:].rearrange("p (c j) -> p c j", j=J)

    with tile.TileContext(nc) as tc, ExitStack() as ctx:
        erpool = ctx.enter_context(tc.tile_pool(name="er", bufs=1))
        cpool = ctx.enter_context(tc.tile_pool(name="const", bufs=1))

        # pools close in LIFO order: hA (after layer 1), then h1s (after
        # layer 2); h2 lives until the end.
        h2pool = ctx.enter_context(tc.tile_pool(name="h2", bufs=1))
        hsB = ctx.enter_context(ExitStack())     # closes after layer 2
        hBpool = hsB.enter_context(tc.tile_pool(name="hB", bufs=1))
        hsA = ctx.enter_context(ExitStack())     # closes after layer 1
        hApool = hsA.enter_context(tc.tile_pool(name="hA", bufs=1))

        ident = cpool.tile([128, 128], bf16)
        make_identity(nc, ident)
        cmask_t = cpool.tile([128, BC], mybir.dt.uint8)
        nc.sync.dma_start(cmask_t[:], cmask[:])

        # transposed activations: hA = h_text (layer-1 input), full layout,
        # [jc][nt] tiles of [128, 512]; h1s = h1 packed-C3 slab per jc;
        # h2s = h2 packed-C2 slab per jc
        hA = [
            [
                hApool.tile(
                    [128, 512], bf16, tag=f"hA_{jc}_{nt}", name=f"hA_{jc}_{nt}"
                )
                for nt in range(NT)
            ]
            for jc in range(HC)
        ]
        h1s = [
            hBpool.tile([128, NC3], bf16, tag=f"h1_{jc}", name=f"h1_{jc}")
            for jc in range(HC)
        ]
        h2s = [
            h2pool.tile([128, NC2], bf16, tag=f"h2_{jc}", name=f"h2_{jc}")
            for jc in range(HC)
        ]
        # edge_repr^T, 36 chunks of 128 rows: [h_graph_c, h_text_c, h_graph_t,
        # h_text_t, h_dist, z] each HC chunks wide
        erT = erpool.tile([128, 36 * 128], bf16)

        # ---------------- phase 1: text projection ----------------
        with ExitStack() as p1:
            xtpool = p1.enter_context(tc.tile_pool(name="xt", bufs=3))
            wsem_pool = p1.enter_context(tc.tile_pool(name="wsem", bufs=1))
            embpool = p1.enter_context(tc.tile_pool(name="embt", bufs=3))
            ps_a = p1.enter_context(tc.tile_pool(name="ps_a", bufs=8, space="PSUM"))

            # chunked startup DMAs: the DMA ring only comes up ~8us into the
            # program, so the first matmul chain must need as little data as
            # possible -- interleave per-kc wsem/xt chunks for tile 0.
            wsem_ts = []
            xt0_t = xtpool.tile([128, 8 * 512], bf16, tag="xt0")
            for kc in range(8):
                w = wsem_pool.tile([128, H], bf16, tag=f"wsem{kc}")
                nc.sync.dma_start(w[:], wsem[:, kc * H:][:, :H])
                wsem_ts.append(w)
                nc.sync.dma_start(xt0_t[:, kc * 512:][:, :512], xt[0, kc])
            for nt in range(NT):
                emb_t = embpool.tile([128, HC * 512], bf16)
                nc.sync.dma_start(
                    emb_t[:].rearrange("p (k j) -> p k j", j=512),
                    embt[nt].transpose([1, 0, 2]),
                )
                if nt == 0:
                    xt_t = xt0_t
                else:
                    xt_t = xtpool.tile([128, 8 * 512], bf16)
                    nc.sync.dma_start(
                        xt_t[:].rearrange("p (k j) -> p k j", j=512),
                        xt[nt].transpose([1, 0, 2]),
                    )
                for jc in range(HC):
                    acc = ps_a.tile([128, 512], f32)
                    for kc in range(8):
                        nc.tensor.matmul(
                            acc[:],
                            wsem_ts[kc][:, jc * 128:][:, :128],
                            xt_t[:, kc * 512:][:, :512],
                            start=(kc == 0),
                            stop=(kc == 7),
                        )
                    # speaker/emotion embedding sum added on the vector
                    # engine (in-place psum update) instead of a K=16
                    # one-hot matmul
                    nc.vector.tensor_add(
                        out=acc[:],
                        in0=acc[:],
                        in1=emb_t[:, jc * 512:][:, :512],
                    )
                    nc.scalar.activation(hA[jc][nt][:], acc[:], AF.Relu)

            # h_text gathers (chunks 6-11 = h_text_c, 18-23 = h_text_t)
            for jc in range(HC):
                for nt in range(NT):
                    src = hA[jc][nt].rearrange("p (b u) -> p b u", u=P)
                    nc.vector.tensor_copy(
                        erT[:, (6 + jc) * 128 + nt * 16:][:, :16],
                        src[:, :, 0],
                    )
                    nc.vector.tensor_copy(
                        erT[:, (18 + jc) * 128 + nt * 16:][:, :16],
                        src[:, :, 1],
                    )
                nc.vector.copy_predicated(
                    erT[:, (18 + jc) * 128:][:, :BC],
                    cmask_t[:],
                    erT[:, (6 + jc) * 128:][:, :BC],
                )

        # -------- phase 2: GNN layer 1 (full sources -> packed-C3) --------
        with ExitStack() as p2:
            wpool = p2.enter_context(tc.tile_pool(name="w", bufs=2))
            a1pool = p2.enter_context(tc.tile_pool(name="a1", bufs=1))
            hapool = p2.enter_context(tc.tile_pool(name="ha", bufs=6))
            msgpool = p2.enter_context(tc.tile_pool(name="msg", bufs=2))
            tmppool = p2.enter_context(tc.tile_pool(name="tmp", bufs=3))
            ps_t2 = p2.enter_context(tc.tile_pool(name="ps_t2", bufs=3, space="PSUM"))
            ps_m = p2.enter_context(tc.tile_pool(name="ps_m", bufs=2, space="PSUM"))
            ps_a2 = p2.enter_context(tc.tile_pool(name="ps_a2", bufs=2, space="PSUM"))

            atb1_t = a1pool.tile([128, (NCN // 128) * W1M], bf16)
            nc.sync.dma_start(sb3(atb1_t, W1M), atb1.transpose([1, 0, 2]))

            ws_t = wpool.tile([128, HC * H], bf16, tag="ws")
            nc.sync.dma_start(ws_t[:], wself[0])
            wn_t = wpool.tile([128, HC * H], bf16, tag="wn")
            nc.sync.dma_start(wn_t[:], wnbr[0])
            # full node tile nt == supergroup sg (512 cols = 16 graphs)
            for sg in range(NSG):
                c3 = c3s[sg]
                sgw = SGW3[sg]
                w1 = 4 * c3
                msg_t = msgpool.tile(
                    [128, HC * sgw], bf16, name="msg1", tag="msg1"
                )
                has = [None] * HC

                # transpose/copy units lead the A-matmul units by 2 so the
                # psum->sbuf scalar copy hides under later transposes
                def emit_t(jc, sg=sg, has=has):
                    pst = ps_t2.tile([128, 512], bf16, name="pst1", tag="pst1")
                    for g4 in range(GPT):
                        nc.tensor.transpose(
                            pst[:, g4 * 128:][:, :128],
                            hA[jc][sg][:, g4 * 128:][:, :128],
                            ident[:],
                        )
                    ha = hapool.tile([128, 512], bf16, name="ha1", tag="ha1")
                    nc.scalar.activation(ha[:], pst[:], AF.Copy)
                    has[jc] = ha

                def emit_a(jc, sg=sg, sgw=sgw, w1=w1, has=has, msg_t=msg_t):
                    psm = ps_m.tile([128, sgw], f32, name="psm1", tag="psm1")
                    for g4 in range(GPT):
                        nc.tensor.matmul(
                            psm[:, g4 * w1:][:, :w1],
                            has[jc][:, g4 * 128:][:, :128],
                            atb1_t[:, (sg * GPT + g4) * W1M:][:, :w1],
                            start=True,
                            stop=True,
                        )
                    nc.vector.tensor_copy(msg_t[:, jc * sgw:][:, :sgw], psm[:])

                for jc in range(HC + 2):
                    if jc < HC:
                        emit_t(jc)
                    if jc >= 2:
                        emit_a(jc - 2)

                for jc in range(HC):
                    acc = ps_a2.tile([128, sgw], f32, name="acc1", tag="acc1")
                    for kc in range(HC):
                        nc.tensor.matmul(
                            acc[:],
                            ws_t[:, kc * H + jc * 128:][:, :128],
                            hA[kc][sg].rearrange(
                                "p (b u) -> p b u", u=P
                            )[:, :, :c3],
                            start=(kc == 0),
                            stop=False,
                        )
                    for kc in range(HC):
                        nc.tensor.matmul(
                            acc[:],
                            wn_t[:, kc * H + jc * 128:][:, :128],
                            msg_t[:, kc * sgw:][:, :sgw],
                            start=False,
                            stop=(kc == HC - 1),
                        )
                    tmp = tmppool.tile([128, sgw], f32, name="tmp1", tag="tmp1")
                    nc.scalar.activation(tmp[:], acc[:], AF.Relu)
                    nc.vector.tensor_add(
                        out=h1s[jc][:, off3[sg]:][:, :sgw].rearrange(
                            "p (b u) -> p b u", u=c3
                        ),
                        in0=tmp[:].rearrange("p (b u) -> p b u", u=c3),
                        in1=hA[jc][sg].rearrange(
                            "p (b u) -> p b u", u=P
                        )[:, :, :c3],
                    )

        # hA (h_text) no longer needed
        hsA.close()

        # ------- phase 3: GNN layer 2 (packed-C3 sources -> packed-C2) ----
        with ExitStack() as p3:
            wpool = p3.enter_context(tc.tile_pool(name="w2", bufs=2))
            a2pool = p3.enter_context(tc.tile_pool(name="a2", bufs=1))
            hapool = p3.enter_context(tc.tile_pool(name="ha2", bufs=6))
            msgpool = p3.enter_context(tc.tile_pool(name="msg2", bufs=2))
            tmppool = p3.enter_context(tc.tile_pool(name="tmp2", bufs=3))
            ps_t2 = p3.enter_context(tc.tile_pool(name="ps_t3", bufs=3, space="PSUM"))
            ps_m = p3.enter_context(tc.tile_pool(name="ps_m3", bufs=2, space="PSUM"))
            ps_a2 = p3.enter_context(tc.tile_pool(name="ps_a3", bufs=2, space="PSUM"))

            atb2_t = a2pool.tile([128, NBLK3 * AW2], bf16)
            nc.sync.dma_start(sb3(atb2_t, AW2), atb2.transpose([1, 0, 2]))

            zpool = p3.enter_context(tc.tile_pool(name="z", bufs=1))
            ps_z = p3.enter_context(tc.tile_pool(name="ps_z", bufs=1, space="PSUM"))

            ws_t = wpool.tile([128, HC * H], bf16, tag="ws2")
            nc.sync.dma_start(ws_t[:], wself[1])
            wn_t = wpool.tile([128, HC * H], bf16, tag="wn2")
            nc.sync.dma_start(wn_t[:], wnbr[1])

            ohd_t = zpool.tile([128, BC], bf16)
            nc.sync.dma_start(ohd_t[:P, :], ohd[:])
            demb_t = zpool.tile([128, H], bf16)
            nc.sync.dma_start(demb_t[:P, :], demb[:])
            bexpl_t = zpool.tile([128, HC], f32)
            nc.sync.dma_start(bexpl_t[:], bexpl[:])
            ext_t = zpool.tile([128, HC * BC], bf16)
            nc.sync.dma_start(sb3(ext_t, BC), chunked(ext, BC))
            wexpl_t = zpool.tile([128, HC * H], bf16)
            nc.sync.dma_start(wexpl_t[:], wexpl[:])

            # h_dist (erT chunks 24-29) and z_teacher (30-35): independent of
            # the GNN; interleave psum-groups per nt2 so drains hide under
            # the layer-2 matmul streams.
            def emit_zdist(zi):
                jc = zi % HC
                if zi < HC:
                    psd = ps_z.tile([128, BC], f32, tag="zz", name="zz")
                    nc.tensor.matmul(
                        psd[:],
                        demb_t[:P, jc * 128:][:, :128],
                        ohd_t[:P, :],
                        start=True,
                        stop=True,
                    )
                    nc.scalar.activation(
                        erT[:, (24 + jc) * 128:][:, :BC], psd[:], AF.Copy
                    )
                else:
                    psz = ps_z.tile([128, BC], f32, tag="zz", name="zz")
                    for kc in range(HC):
                        nc.tensor.matmul(
                            psz[:],
                            wexpl_t[:, kc * H + jc * 128:][:, :128],
                            ext_t[:, kc * BC:][:, :BC],
                            start=(kc == 0),
                            stop=(kc == HC - 1),
                        )
                    nc.scalar.activation(
                        erT[:, (30 + jc) * 128:][:, :BC],
                        psz[:],
                        AF.Relu,
                        bias=bexpl_t[:, jc:jc + 1],
                    )

            for t2 in range(NT2):
                # message phase over this tile's supergroups; transpose/copy
                # units lead A-matmul units by 2 (skew)
                sgs = tiles2[t2]
                msg_t = msgpool.tile(
                    [128, HC * 512], bf16, name="msg2t", tag="msg2t"
                )
                NU = HC * len(sgs)
                has = [None] * NU
                psms = [None] * HC

                def emit_t(u, sgs=sgs, has=has):
                    jc, si = divmod(u, len(sgs))
                    sg = sgs[si]
                    nb = SGW3[sg] // 128
                    pst = ps_t2.tile(
                        [128, nb * 128], bf16, name="pst2", tag="pst2"
                    )
                    for b in range(nb):
                        nc.tensor.transpose(
                            pst[:, b * 128:][:, :128],
                            h1s[jc][:, off3[sg] + b * 128:][:, :128],
                            ident[:],
                        )
                    ha = hapool.tile(
                        [128, nb * 128], bf16, name="ha2", tag="ha2"
                    )
                    nc.scalar.activation(ha[:], pst[:], AF.Copy)
                    has[u] = ha

                def emit_a(u, t2=t2, sgs=sgs, has=has, psms=psms, msg_t=msg_t):
                    jc, si = divmod(u, len(sgs))
                    sg = sgs[si]
                    if si == 0:
                        psms[jc] = ps_m.tile(
                            [128, 512], f32, name="psm2", tag="psm2"
                        )
                    psm = psms[jc]
                    ha = has[u]
                    base = off2[sg] - 512 * t2
                    blk0 = off3[sg] // 128
                    for (b, rc0, ncol, pc0, st, sp) in plans[sg]:
                        nc.tensor.matmul(
                            psm[:, base + pc0:][:, :ncol],
                            ha[:, b * 128:][:, :128],
                            atb2_t[:, (blk0 + b) * AW2 + rc0:][:, :ncol],
                            start=st,
                            stop=sp,
                        )
                    if si == len(sgs) - 1:
                        nc.vector.tensor_copy(
                            msg_t[:, jc * 512:][:, :512], psm[:]
                        )

                for u in range(NU + 2):
                    if u < NU:
                        emit_t(u)
                    if u >= 2:
                        emit_a(u - 2)

                # W-matmuls: self over per-sg packed-C2 views, nbr over msg
                # per-(jc, sg) full-width psum chains: multi-instruction
                # accumulation must cover the whole psum tile (sub-region
                # chains mis-accumulate on HW)
                for jc in range(HC):
                    for sg in sgs:
                        base = off2[sg] - 512 * t2
                        sw2 = SGW2[sg]
                        acc = ps_a2.tile(
                            [128, sw2], f32, name="acc2", tag="acc2"
                        )
                        for kc in range(HC):
                            nc.tensor.matmul(
                                acc[:],
                                ws_t[:, kc * H + jc * 128:][:, :128],
                                h1s[kc][:, off3[sg]:][:, :SGW3[sg]].rearrange(
                                    "p (b u) -> p b u", u=c3s[sg]
                                )[:, :, :c2s[sg]],
                                start=(kc == 0),
                                stop=False,
                            )
                        for kc in range(HC):
                            nc.tensor.matmul(
                                acc[:],
                                wn_t[:, kc * H + jc * 128:][:, :128],
                                msg_t[:, kc * 512 + base:][:, :sw2],
                                start=False,
                                stop=(kc == HC - 1),
                            )
                        tmp = tmppool.tile(
                            [128, sw2], f32, name="tmp2", tag="tmp2"
                        )
                        nc.scalar.activation(tmp[:], acc[:], AF.Relu)
                        nc.vector.tensor_add(
                            out=h2s[jc][:, off2[sg]:][:, :sw2].rearrange(
                                "p (b u) -> p b u", u=c2s[sg]
                            ),
                            in0=tmp[:].rearrange("p (b u) -> p b u", u=c2s[sg]),
                            in1=h1s[jc][:, off3[sg]:][:, :SGW3[sg]].rearrange(
                                "p (b u) -> p b u", u=c3s[sg]
                            )[:, :, :c2s[sg]],
                        )
                if _dbg:
                    for jc in range(HC):
                        nc.sync.dma_start(
                            dbgm_ap[jc][:, t2 * 512:][:, :512],
                            msg_t[:, jc * 512:][:, :512],
                        )
                for zi in range(
                    t2 * 12 // NT2, (t2 + 1) * 12 // NT2
                ):
                    emit_zdist(zi)

        if _dbg:
            for jc in range(HC):
                nc.sync.dma_start(dbg_ap[jc], h2s[jc][:])
                nc.sync.dma_start(dbg1_ap[jc], h1s[jc][:])

        # h1s no longer needed
        hsB.close()

        # ---------------- phase 4: GNN layer 3 (slots 0,1) + predictor ----
        with ExitStack() as p4:
            ppool = p4.enter_context(tc.tile_pool(name="pred", bufs=1))
            w3pool = p4.enter_context(tc.tile_pool(name="w3", bufs=1))
            a3pool = p4.enter_context(tc.tile_pool(name="a3", bufs=1))
            hapool = p4.enter_context(tc.tile_pool(name="ha3", bufs=6))
            tmppool = p4.enter_context(tc.tile_pool(name="tmp3", bufs=2))
            ps_t2 = p4.enter_context(tc.tile_pool(name="ps_t4", bufs=3, space="PSUM"))
            ps_m = p4.enter_context(tc.tile_pool(name="ps_m4", bufs=1, space="PSUM"))
            ps_c3 = p4.enter_context(tc.tile_pool(name="ps_c3", bufs=2, space="PSUM"))
            ps_p = p4.enter_context(tc.tile_pool(name="ps_p", bufs=2, space="PSUM"))

            # small layer-3 DMAs first -- the 7 MB wp1 prefetch must not
            # block them in the DMA queue (layer 3 needs these immediately;
            # wp1 is consumed ~25 us later by the predictor).
            atb3_t = a3pool.tile([128, NBLK2 * W3M], bf16)
            nc.sync.dma_start(sb3(atb3_t, W3M), atb3.transpose([1, 0, 2]))
            ws3_t = w3pool.tile([128, HC * H], bf16, tag="ws3")
            nc.sync.dma_start(ws3_t[:], wself[2])
            wn3_t = w3pool.tile([128, HC * H], bf16, tag="wn3")
            nc.sync.dma_start(wn3_t[:], wnbr[2])
            bp1_t = ppool.tile([128, HC], f32)
            nc.sync.dma_start(bp1_t[:], bp1[:])
            wp2_t = ppool.tile([128, HC], bf16)
            nc.sync.dma_start(wp2_t[:], wp2[:])
            bp2_t = ppool.tile([1, 1], f32)
            nc.sync.dma_start(bp2_t[:], bp2[:])
            wp1_t = []
            for jc in range(HC):
                w1s = ppool.tile(
                    [128, 36 * 128], bf16, tag=f"wp1_{jc}", name=f"wp1_{jc}"
                )
                nc.sync.dma_start(w1s[:], wp1[jc])
                wp1_t.append(w1s)

            # --- layer 3 message + gather at slots {0,1} ---
            msg3_t = ppool.tile([128, HC * 2 * BC], bf16)
            h2p_t = ppool.tile([128, HC * 2 * BC], bf16)
            for jc in range(HC):
                psm = ps_m.tile([128, 2 * BC], f32, name="psm3", tag="psm3")
                has = [None] * NBLK2

                def emit_t(blk, jc=jc, has=has):
                    pst = ps_t2.tile([128, 128], bf16, name="pst3", tag="pst3")
                    nc.tensor.transpose(
                        pst[:], h2s[jc][:, blk * 128:][:, :128], ident[:]
                    )
                    ha = hapool.tile([128, 128], bf16, name="ha3", tag="ha3")
                    nc.scalar.activation(ha[:], pst[:], AF.Copy)
                    has[blk] = ha

                def emit_a(blk, has=has, psm=psm):
                    _, gpb, g0 = blocks3[blk]
                    nc.tensor.matmul(
                        psm[:, 2 * g0:][:, :2 * gpb],
                        has[blk][:],
                        atb3_t[:, blk * W3M:][:, :2 * gpb],
                        start=True,
                        stop=True,
                    )

                for u in range(NBLK2 + 3):
                    if u < NBLK2:
                        emit_t(u)
                    if u >= 3:
                        emit_a(u - 3)

                nc.vector.tensor_copy(msg3_t[:, jc * 2 * BC:][:, :2 * BC], psm[:])
                for sg in range(NSG):
                    src = h2s[jc][:, off2[sg]:][:, :SGW2[sg]].rearrange(
                        "p (b u) -> p b u", u=c2s[sg]
                    )
                    dst = h2p_t[
                        :, jc * 2 * BC + sg * 2 * SG:
                    ][:, :2 * SG].rearrange("p (b u) -> p b u", u=2)
                    nc.vector.tensor_copy(dst, src[:, :, :2])

            # --- layer 3 W-matmuls -> h3 -> erT chunks 0-5 (c), 12-17 (t) ---
            for jc in range(HC):
                acc = ps_c3.tile([128, 2 * BC], f32, name="acc3", tag="acc3")
                for kc in range(HC):
                    nc.tensor.matmul(
                        acc[:],
                        ws3_t[:, kc * H + jc * 128:][:, :128],
                        h2p_t[:, kc * 2 * BC:][:, :2 * BC],
                        start=(kc == 0),
                        stop=False,
                    )
                for kc in range(HC):
                    nc.tensor.matmul(
                        acc[:],
                        wn3_t[:, kc * H + jc * 128:][:, :128],
                        msg3_t[:, kc * 2 * BC:][:, :2 * BC],
                        start=False,
                        stop=(kc == HC - 1),
                    )
                tmp = tmppool.tile([128, 2 * BC], f32, name="tmp3", tag="tmp3")
                nc.scalar.activation(tmp[:], acc[:], AF.Relu)
                h3 = tmppool.tile([128, 2 * BC], bf16, name="h3", tag="h3")
                nc.vector.tensor_add(
                    out=h3[:], in0=tmp[:], in1=h2p_t[:, jc * 2 * BC:][:, :2 * BC]
                )
                h3v = h3.rearrange("p (b u) -> p b u", u=2)
                nc.vector.tensor_copy(erT[:, (0 + jc) * 128:][:, :BC], h3v[:, :, 0])
                nc.vector.tensor_copy(erT[:, (12 + jc) * 128:][:, :BC], h3v[:, :, 1])
                nc.vector.copy_predicated(
                    erT[:, (12 + jc) * 128:][:, :BC],
                    cmask_t[:],
                    erT[:, (0 + jc) * 128:][:, :BC],
                )

            hid_t = ppool.tile([128, HC * BC], bf16)
            for jc in range(HC):
                psp = ps_p.tile([128, BC], f32, tag="pp", name="pp")
                for kc in range(36):
                    nc.tensor.matmul(
                        psp[:],
                        wp1_t[jc][:, kc * 128:][:, :128],
                        erT[:, kc * 128:][:, :128],
                        start=(kc == 0),
                        stop=(kc == 35),
                    )
                nc.scalar.activation(
                    hid_t[:, jc * BC:][:, :BC],
                    psp[:],
                    AF.Relu,
                    bias=bp1_t[:, jc:jc + 1],
                )

            psl = ps_p.tile([128, BC], f32, tag="pp", name="psl")
            for jc in range(HC):
                nc.tensor.matmul(
                    psl[:1, :],
                    wp2_t[:, jc:jc + 1],
                    hid_t[:, jc * BC:][:, :BC],
                    start=(jc == 0),
                    stop=(jc == HC - 1),
                )
            logit_t = ppool.tile([128, BC], f32)
            nc.vector.tensor_scalar_add(
                out=logit_t[:1, :], in0=psl[:1, :], scalar1=bp2_t[:1, :1]
            )
            nc.sync.dma_start(out_ap[:], logit_t[:1, :])

    nc.compile()
    return nc


def _host_prep(inputs):
    x = np.asarray(inputs["x"], np.float32)
    spk = np.asarray(inputs["speaker_ids"], np.int64)
    emo = np.asarray(inputs["emotion_ids"], np.int64)
    ei = np.asarray(inputs["edge_index"], np.int64)
    tni = np.asarray(inputs["target_node_indices"], np.int64)
    ex = np.asarray(inputs["expl_space_vec"], np.float32)

    E = ei.shape[1]
    edge_src, edge_tgt = ei[0], ei[1]
    c_idx, t_idx = tni[:, 0], tni[:, 1]

    # reference first-edge/dist logic (exact)
    fe = np.full(N, E, np.int64)
    np.minimum.at(fe, edge_src, np.arange(E, dtype=np.int64))

    def first_tgt(q):
        feq = fe[q]
        return np.where(feq < E, edge_tgt[np.minimum(feq, E - 1)], q)

    dist = np.clip(np.abs(first_tgt(c_idx) - first_tgt(t_idx)), 0, P - 1)

    # slot-1 node: t, or a filler distinct from c when c == t
    t_eff = np.where(c_idx == t_idx, (t_idx + 1) % P, t_idx)

    # per-graph receptive-field sets (old coords):
    # T2 = {c,t} U in({c,t}); T3 = T2 U in(T2)
    g_e = edge_src // P
    s_l, t_l = edge_src % P, edge_tgt % P
    innb = np.zeros((B, P, P), np.int8)
    innb[g_e, t_l, s_l] = 1
    sel = np.zeros((B, P), bool)
    bidx = np.arange(B)
    sel[bidx, c_idx] = True
    sel[bidx, t_eff] = True
    grow = lambda X: X | (np.einsum("bts,bt->bs", innb, X.astype(np.int8)) > 0)
    S2 = grow(sel)
    S3 = grow(S2)
    t2cnt = S2.sum(1)
    t3cnt = S3.sum(1)

    # per-graph node permutation: slot 0 = c, slot 1 = t_eff, T2 within
    # prefix C2, T3 within prefix C3
    prio = np.full((B, P), 8, np.int64)
    prio[S3] = 3
    prio[S2] = 2
    prio[bidx, t_eff] = 1
    prio[bidx, c_idx] = 0
    new2old = np.argsort(prio, axis=1, kind="stable")
    old2new = np.argsort(new2old, axis=1)

    # adjacency in permuted coords (original graph order)
    s_new = old2new[g_e, s_l]
    t_new = old2new[g_e, t_l]
    A = np.zeros((B, P, P), np.float32)
    np.add.at(A, (g_e, t_new, s_new), 1.0)

    # region config: reorder graphs within each core so light graphs
    # (small T2/T3) land in supergroups with small C2/C3 prefixes
    kcls = np.where(t3cnt > 16, 2, np.where(t2cnt > 8, 1, 0)).reshape(
        NCORES, BC
    )
    n_k2 = (kcls == 2).sum(1).max()
    n_k12 = (kcls >= 1).sum(1).max()
    t2max, t3max = int(t2cnt.max()), int(t3cnt.max())
    import os
    force = os.environ.get("KCFG", "")
    if t2max <= 16 and t3max <= 24 and n_k2 <= SG and n_k12 <= 4 * SG:
        cfg = ((8, 16),) * 4 + ((16, 16),) * 3 + ((16, 24),)
        if force == "c3only":
            cfg = ((16, 16),) * 7 + ((16, 24),)
        elif force == "uniform":
            cfg = ((16, 24),) * 8
        gperm = (
            np.argsort(kcls, axis=1, kind="stable")
            + np.arange(NCORES)[:, None] * BC
        ).reshape(-1)
    else:
        c2u = 16 if t2max <= 16 else 32
        c3u = max(c2u, 24 if t3max <= 24 else 32)
        cfg = ((c2u, c3u),) * NSG
        gperm = np.arange(B)
    c2g = np.tile(np.repeat([c[0] for c in cfg], SG), NCORES)  # per position
    c3g = np.tile(np.repeat([c[1] for c in cfg], SG), NCORES)

    # reorder all per-graph data into position order
    A = A[gperm]
    dist = dist[gperm]
    new2old = new2old[gperm]
    ceqt = (c_idx == t_idx)[gperm]
    ex = ex[gperm]
    t2cnt = t2cnt[gperm]

    # exactness checks: every in-edge of slots {0,1} originates within the
    # graph's prefix C2, and of a true-T2 slot within prefix C3
    srcbad2 = np.arange(P)[None, :] >= c2g[:, None]         # [B, P]
    assert not (A[:, :2, :].any(1) & srcbad2).any()
    usedmask = np.arange(P)[None, :] < t2cnt[:, None]       # true-T2 slots
    srcbad3 = np.arange(P)[None, :] >= c3g[:, None]
    assert not (A * usedmask[:, :, None] * srcbad3[:, None, :]).any()

    perm_global = (gperm[:, None] * P + new2old).reshape(-1)

    xtb = np.ascontiguousarray(x[perm_global].T.astype(BF16))  # [DSEM, N]
    spk_new = spk[perm_global]
    emo_new = emo[perm_global]

    # speaker+emotion embedding sums, feature-major [H, N]
    embsum = (
        np.asarray(inputs["spk_emb"], np.float32)[spk_new]
        + np.asarray(inputs["emo_emb"], np.float32)[emo_new]
    ).T.astype(BF16)

    # geometry (mirrors _build_program)
    c2s = [c[0] for c in cfg]
    c3s = [c[1] for c in cfg]
    SGW2 = [SG * c for c in c2s]
    SGW3 = [SG * c for c in c3s]
    off2 = np.concatenate([[0], np.cumsum(SGW2)])
    off3 = np.concatenate([[0], np.cumsum(SGW3)])
    NC2, NC3 = int(off2[-1]), int(off3[-1])
    W1M = 4 * max(c3s)
    plans = [_a2_plan(c2s[sg], c3s[sg]) for sg in range(NSG)]
    AW2 = max(e[1] + e[2] for p in plans for e in p)
    NBLK3 = NC3 // 128
    NBLK2 = NC2 // 128
    W3M = 2 * max(128 // c for c in c2s)

    # layer-1 AT tiles: full-layout sources (4 graphs per 128-row block),
    # packed-C3 targets
    nfb = B * P // 128
    atb1 = np.zeros((nfb, 128, W1M), np.float32)
    for i in range(4):
        for sg in range(NSG):          # same cfg on every core
            c3 = c3s[sg]
            fb = (np.arange(B // 4).reshape(NCORES, NSG, 4)[:, sg, :]).ravel()
            gl = fb * 4 + i
            atb1[fb, 32 * i:32 * i + 32, c3 * i:c3 * i + c3] = (
                A[gl][:, :c3, :].transpose(0, 2, 1)
            )
    atb1 = atb1.astype(BF16)

    # layer-2 AT tiles: packed-C3 sources -> packed-C2 targets, blocks are
    # supergroup-aligned; col origin of block b is its first graph
    atb2 = np.zeros((NCORES * NBLK3, 128, AW2), np.float32)
    for core in range(NCORES):
        for sg in range(NSG):
            c2, c3 = c2s[sg], c3s[sg]
            blk0 = core * NBLK3 + int(off3[sg]) // 128
            for gl in range(SG):
                g = core * BC + sg * SG + gl
                rows = gl * c3 + np.arange(c3)
                bb = rows // 128
                rr = rows % 128
                g0b = (128 * bb) // c3
                for t in range(c2):
                    atb2[blk0 + bb, rr, (gl - g0b) * c2 + t] = A[g, t, :c3]
    atb2 = atb2.astype(BF16)

    # layer-3 AT tiles: packed-C2 sources -> slots {0,1}
    atb3 = np.zeros((NCORES * NBLK2, 128, W3M), np.float32)
    for core in range(NCORES):
        for sg in range(NSG):
            c2 = c2s[sg]
            gpb = 128 // c2
            nb = SGW2[sg] // 128
            for k in range(nb):
                blk = core * NBLK2 + int(off2[sg]) // 128 + k
                for i in range(gpb):
                    g = core * BC + sg * SG + k * gpb + i
                    atb3[blk, c2 * i:c2 * i + c2, 2 * i:2 * i + 2] = (
                        A[g, :2, :c2].T
                    )
    atb3 = atb3.astype(BF16)

    cmask = np.tile(ceqt.astype(np.uint8)[None, :], (128, 1))

    ohd = np.zeros((P, B), BF16)
    ohd[dist, np.arange(B)] = 1.0

    extT = np.ascontiguousarray(ex.T.astype(BF16))

    rearr = lambda v: np.ascontiguousarray(
        np.asarray(v, np.float32).reshape(HC, 128).T
    )
    # [K, H] -> [128, (K//128)*H] SBUF-layout slab (contiguous DMA)
    chunk_w = lambda w: np.ascontiguousarray(
        np.asarray(w, np.float32)
        .reshape(-1, 128, w.shape[-1]).transpose(1, 0, 2)
        .reshape(128, -1)
    ).astype(BF16)
    b16 = lambda k: np.asarray(inputs[k], np.float32).astype(BF16)

    shared = dict(
        wsem=chunk_w(np.asarray(inputs["W_sem"], np.float32)),
        wself=np.stack([
            chunk_w(np.asarray(inputs["gnn_w_self"], np.float32)[l])
            for l in range(L)
        ]),
        wnbr=np.stack([
            chunk_w(np.asarray(inputs["gnn_w_nbr"], np.float32)[l])
            for l in range(L)
        ]),
        demb=b16("dist_emb"),
        wexpl=chunk_w(np.asarray(inputs["W_expl"], np.float32)),
        bexpl=rearr(inputs["b_expl"]),
        wp1=np.ascontiguousarray(
            np.asarray(inputs["W_p1"], np.float32)
            .reshape(36, 128, HC, 128).transpose(2, 1, 0, 3)
            .reshape(HC, 128, 36 * 128)
        ).astype(BF16),
        bp1=rearr(inputs["b_p1"]),
        wp2=rearr(np.asarray(inputs["W_p2"], np.float32)[:, 0]).astype(BF16),
        bp2=np.asarray(inputs["b_p2"], np.float32).reshape(1, 1),
    )

    in_maps = []
    for i in range(NCORES):
        gs = slice(i * BC, (i + 1) * BC)
        ns = slice(i * NCN, (i + 1) * NCN)
        m = dict(shared)
        m["xt"] = np.ascontiguousarray(
            xtb[:, ns].reshape(8, 128, NT, 512).transpose(2, 0, 1, 3)
        )
        m["embt"] = np.ascontiguousarray(
            embsum[:, ns].reshape(HC, 128, NT, 512).transpose(2, 0, 1, 3)
        )
        m["atb1"] = np.ascontiguousarray(atb1[i * (NCN // 128):][:NCN // 128])
        m["atb2"] = np.ascontiguousarray(atb2[i * NBLK3:][:NBLK3])
        m["atb3"] = np.ascontiguousarray(atb3[i * NBLK2:][:NBLK2])
        m["cmask"] = np.ascontiguousarray(cmask[:, gs])
        m["ohd"] = np.ascontiguousarray(ohd[:, gs])
        m["ext"] = np.ascontiguousarray(extT[:, gs])
        in_maps.append(m)
    return in_maps, cfg, gperm


def kernel(**inputs):
    in_maps, cfg, gperm = _host_prep(inputs)
    if cfg not in _cache:
        _cache[cfg] = _build_program(cfg)
    from concourse.bass_utils import run_bass_kernel_spmd

    res = run_bass_kernel_spmd(_cache[cfg], in_maps, list(range(NCORES)))
    out = np.empty(B, np.float32)
    out[gperm] = np.concatenate(
        [res.results[i]["out"].reshape(BC) for i in range(NCORES)]
    ).astype(np.float32)
    return out


# revision 35
# speedup vs baseline: 1.3409x; 1.0091x over previous
"""Trainium2 Bass kernel for nn_CrossTowerCausalModel.

Data-parallel over graphs: each of the 8 NeuronCores handles 128 graphs
(128*32 = 4096 nodes, 128*64 = 8192 edges). Weights/embeddings replicated.

Device activation layout is "transposed" (layout B): hT[feature, node] with
the 768 feature dim split into 6 chunks of 128 partitions. Weight matrices
[in, out] then serve directly as matmul lhsT (stationary) operands.

Receptive-field restriction: the GNN output h_graph is only read at 2 nodes
per graph (c, t). Host permutes each graph's 32 node slots so that
  slot 0 = c, slot 1 = t (filler if c == t),
  slots [0, C2) contain T2 = {c,t} U in({c,t}),
  slots [0, C3) contain T3 = T2 U in(T2),
so layer 3 only computes slots {0,1}, layer 2 only the C2-slot prefix and
layer 1 only the C3-slot prefix. Layer-1 messages still read h_text at all
32 slots, so every value read downstream is identical to the full
computation (values at prefix-C3 are exact; layer-2 junk slots beyond T2
lose out-of-prefix sources but are multiplied by structural zeros in A3).

Region specialization: graphs are reordered within each core (restored on
output) so that 16-graph supergroups get individual (C2, C3) prefixes --
typically [(8,16)]*4 + [(16,16)]*3 + [(16,24)] -- sized on the host from
the actual per-graph |T2|/|T3|; uniform fallbacks cover adversarial
inputs. Supergroup packed widths (16*C2, 16*C3) are multiples of 128, so
all 128-partition blocks stay supergroup-aligned; graphs straddling a
128-block inside a C3=24 supergroup use 2-chain psum accumulation in the
layer-2 adjacency matmuls.

h is stored in bf16 (matmul input dtype); per-layer psum accumulation and
relu stay fp32.

Host-side prep (pure index logic + layout, no heavy math):
  * per-graph node permutation (above) -> final gathers h_c / h_t become
    strided copies. (c == t graphs fixed up with copy_predicated.)
  * x passed pre-transposed (feature-major) bf16, per-kc chunks so the
    first matmul chain starts as soon as the DMA ring comes up.
  * dense per-graph adjacency as block-diagonal AT tiles (layers 1-3).
  * the quirky first-edge/dist logic of the reference (exact int math).
  * speaker/emotion one-hots (16 rows) fused into the input projection.
"""

import numpy as np
import ml_dtypes

B = 1024          # graphs
P = 32            # nodes per graph
N = B * P
H = 768
HC = H // 128     # 6 feature chunks
L = 3
DSEM = 1024
NUM_SPK, NUM_EMO = 9, 7
NCORES = 8
BC = B // NCORES          # graphs per core = 128
NCN = BC * P              # nodes per core = 4096
NT = 8                    # node tiles of 512 (= 16 graphs) per core
GPT = 4                   # blocks (of 128 nodes) per full node tile
SG = 16                   # graphs per supergroup (= per full node tile)
NSG = BC // SG            # supergroups per core = 8

BF16 = ml_dtypes.bfloat16

_cache = {}


def _a2_plan(C2, C3):
    """Layer-2 adjacency matmul plan for one supergroup (16 graphs).

    Packed rows live at [g*C3, (g+1)*C3) within the 16*C3-row supergroup;
    graphs may straddle 128-row blocks. Returns merged matmul entries
    (block, rhs_col0, ncols, psm_col0, start, stop) with psum 2-chain
    accumulation for straddlers.
    """
    NB = (SG * C3) // 128
    raw = []
    for b in range(NB):
        g0 = (128 * b) // C3
        g1 = min(SG - 1, (128 * b + 127) // C3)
        for g in range(g0, g1 + 1):
            st = C3 * g >= 128 * b             # graph's rows begin here
            sp = C3 * (g + 1) <= 128 * (b + 1)  # graph's rows end here
            raw.append((b, g, st, sp))
    merged = []
    for b, g, st, sp in raw:
        if (merged and merged[-1][0] == b and merged[-1][2] == (st, sp)
                and merged[-1][1][-1] == g - 1):
            merged[-1][1].append(g)
        else:
            merged.append([b, [g], (st, sp)])
    plan = []
    for b, gs, (st, sp) in merged:
        g0 = (128 * b) // C3
        plan.append((b, (gs[0] - g0) * C2, len(gs) * C2, gs[0] * C2, st, sp))
    return plan


def _build_program(cfg):
    from contextlib import ExitStack

    import concourse.bacc as bacc
    import concourse.mybir as mybir
    import concourse.tile as tile
    from concourse.masks import make_identity

    f32 = mybir.dt.float32
    bf16 = mybir.dt.bfloat16
    AF = mybir.ActivationFunctionType

    c2s = [c[0] for c in cfg]
    c3s = [c[1] for c in cfg]
    SGW2 = [SG * c for c in c2s]      # packed-C2 cols per supergroup
    SGW3 = [SG * c for c in c3s]      # packed-C3 cols per supergroup
    off2 = np.concatenate([[0], np.cumsum(SGW2)]).tolist()
    off3 = np.concatenate([[0], np.cumsum(SGW3)]).tolist()
    NC2, NC3 = off2[-1], off3[-1]
    assert NC2 % 512 == 0
    NT2 = NC2 // 512                  # layer-2 target tiles
    # target tile -> supergroups (each sg fully inside one tile)
    tiles2 = [[] for _ in range(NT2)]
    for sg in range(NSG):
        assert off2[sg] // 512 == (off2[sg + 1] - 1) // 512
        tiles2[off2[sg] // 512].append(sg)
    W1M = 4 * max(c3s)                # layer-1 AT tile col capacity
    plans = [_a2_plan(c2s[sg], c3s[sg]) for sg in range(NSG)]
    AW2 = max(e[1] + e[2] for p in plans for e in p)
    NBLK3 = NC3 // 128                # packed-C3 blocks per core
    NBLK2 = NC2 // 128                # packed-C2 blocks per core
    # layer-3 source blocks: (sg, graphs-per-block, first-graph)
    blocks3 = []
    for sg in range(NSG):
        gpb = 128 // c2s[sg]
        for k in range(SGW2[sg] // 128):
            blocks3.append((sg, gpb, sg * SG + k * gpb))
    W3M = 2 * max(128 // c for c in c2s)

    nc = bacc.Bacc(
        "TRN2", target_bir_lowering=False, debug=False, num_devices=NCORES
    )

    dram = lambda name, shape, dt: nc.dram_tensor(
        name, shape, dt, kind="ExternalInput"
    ).ap()

    xt = dram("xt", [NT, 8, 128, 512], bf16)
    embt = dram("embt", [NT, HC, 128, 512], bf16)
    wsem = dram("wsem", [128, 8 * H], bf16)
    wself = dram("wself", [L, 128, HC * H], bf16)
    wnbr = dram("wnbr", [L, 128, HC * H], bf16)
    atb1 = dram("atb1", [NCN // 128, 128, W1M], bf16)
    atb2 = dram("atb2", [NBLK3, 128, AW2], bf16)
    atb3 = dram("atb3", [NBLK2, 128, W3M], bf16)
    cmask = dram("cmask", [128, BC], mybir.dt.uint8)
    ohd = dram("ohd", [P, BC], bf16)
    demb = dram("demb", [P, H], bf16)
    wexpl = dram("wexpl", [128, HC * H], bf16)
    bexpl = dram("bexpl", [128, HC], f32)
    ext = dram("ext", [H, BC], bf16)
    wp1 = dram("wp1", [HC, 128, 36 * 128], bf16)
    bp1 = dram("bp1", [128, HC], f32)
    wp2 = dram("wp2", [128, HC], bf16)
    bp2 = dram("bp2", [1, 1], f32)
    out_ap = nc.dram_tensor("out", [1, BC], f32, kind="ExternalOutput").ap()
    import os
    _dbg = os.environ.get("KDBG") == "1"
    if _dbg:
        dbg_ap = nc.dram_tensor(
            "dbg", [HC, 128, NC2], bf16, kind="ExternalOutput"
        ).ap()
        dbg1_ap = nc.dram_tensor(
            "dbg1", [HC, 128, NC3], bf16, kind="ExternalOutput"
        ).ap()
        dbgm_ap = nc.dram_tensor(
            "dbgm", [HC, 128, NC2], bf16, kind="ExternalOutput"
        ).ap()

    # [C*128, J] dram AP -> [128, C, J] (partition-major chunked view)
    def chunked(ap, J):
        return ap.rearrange("(c p) j -> c p j", p=128).transpose([1, 0, 2])

    # SBUF tile [128, C*J] -> [128, C, J]
    def sb3(t, J):
        return t[:].rearrange("p (c j) -> p c j", j=J)

    with tile.TileContext(nc) as tc, ExitStack() as ctx:
        erpool = ctx.enter_context(tc.tile_pool(name="er", bufs=1))
        cpool = ctx.enter_context(tc.tile_pool(name="const", bufs=1))

        # pools close in LIFO order: hA (after layer 1), then h1s (after
        # layer 2); h2 lives until the end.
        h2pool = ctx.enter_context(tc.tile_pool(name="h2", bufs=1))
        hsB = ctx.enter_context(ExitStack())     # closes after layer 2
        hBpool = hsB.enter_context(tc.tile_pool(name="hB", bufs=1))
        hsA = ctx.enter_context(ExitStack())     # closes after layer 1
        hApool = hsA.enter_context(tc.tile_pool(name="hA", bufs=1))

        ident = cpool.tile([128, 128], bf16)
        make_identity(nc, ident)
        cmask_t = cpool.tile([128, BC], mybir.dt.uint8)
        nc.sync.dma_start(cmask_t[:], cmask[:])

        # transposed activations: hA = h_text (layer-1 input), full layout,
        # [jc][nt] tiles of [128, 512]; h1s = h1 packed-C3 slab per jc;
        # h2s = h2 packed-C2 slab per jc
        hA = [
            [
                hApool.tile(
                    [128, 512], bf16, tag=f"hA_{jc}_{nt}", name=f"hA_{jc}_{nt}"
                )
                for nt in range(NT)
            ]
            for jc in range(HC)
        ]
        h1s = [
            hBpool.tile([128, NC3], bf16, tag=f"h1_{jc}", name=f"h1_{jc}")
            for jc in range(HC)
        ]
        h2s = [
            h2pool.tile([128, NC2], bf16, tag=f"h2_{jc}", name=f"h2_{jc}")
            for jc in range(HC)
        ]
        # edge_repr^T, 36 chunks of 128 rows: [h_graph_c, h_text_c, h_graph_t,
        # h_text_t, h_dist, z] each HC chunks wide
        erT = erpool.tile([128, 36 * 128], bf16)

        # ---------------- phase 1: text projection ----------------
        with ExitStack() as p1:
            xtpool = p1.enter_context(tc.tile_pool(name="xt", bufs=3))
            wsem_pool = p1.enter_context(tc.tile_pool(name="wsem", bufs=1))
            embpool = p1.enter_context(tc.tile_pool(name="embt", bufs=3))
            ps_a = p1.enter_context(tc.tile_pool(name="ps_a", bufs=8, space="PSUM"))

            # chunked startup DMAs: the DMA ring only comes up ~8us into the
            # program, so the first matmul chain must need as little data as
            # possible -- interleave per-kc wsem/xt chunks for tile 0.
            wsem_ts = []
            xt0_t = xtpool.tile([128, 8 * 512], bf16, tag="xt0", bufs=1)
            for kc in range(8):
                w = wsem_pool.tile([128, H], bf16, tag=f"wsem{kc}")
                nc.sync.dma_start(w[:], wsem[:, kc * H:][:, :H])
                wsem_ts.append(w)
                nc.sync.dma_start(xt0_t[:, kc * 512:][:, :512], xt[0, kc])
            # keep the xt DMA one tile ahead of embt: the matmul chain needs
            # xt[nt+1] before the vector engine needs embt[nt]
            xts = [xt0_t] + [None] * (NT - 1)

            def fetch_xt(n):
                t = xtpool.tile(
                    [128, 8 * 512], bf16, name="xt_t", tag="xt_t", bufs=2
                )
                nc.sync.dma_start(
                    t[:].rearrange("p (k j) -> p k j", j=512),
                    xt[n].transpose([1, 0, 2]),
                )
                xts[n] = t

            for nt in range(NT):
                if nt + 1 < NT:
                    fetch_xt(nt + 1)
                emb_t = embpool.tile([128, HC * 512], bf16)
                nc.sync.dma_start(
                    emb_t[:].rearrange("p (k j) -> p k j", j=512),
                    embt[nt].transpose([1, 0, 2]),
                )
                xt_t = xts[nt]
                for jc in range(HC):
                    acc = ps_a.tile([128, 512], f32)
                    for kc in range(8):
                        nc.tensor.matmul(
                            acc[:],
                            wsem_ts[kc][:, jc * 128:][:, :128],
                            xt_t[:, kc * 512:][:, :512],
                            start=(kc == 0),
                            stop=(kc == 7),
                        )
                    # speaker/emotion embedding sum added on the vector
                    # engine (in-place psum update) instead of a K=16
                    # one-hot matmul
                    nc.vector.tensor_add(
                        out=acc[:],
                        in0=acc[:],
                        in1=emb_t[:, jc * 512:][:, :512],
                    )
                    nc.scalar.activation(hA[jc][nt][:], acc[:], AF.Relu)

            # h_text gathers (chunks 6-11 = h_text_c, 18-23 = h_text_t)
            for jc in range(HC):
                for nt in range(NT):
                    src = hA[jc][nt].rearrange("p (b u) -> p b u", u=P)
                    nc.vector.tensor_copy(
                        erT[:, (6 + jc) * 128 + nt * 16:][:, :16],
                        src[:, :, 0],
                    )
                    nc.vector.tensor_copy(
                        erT[:, (18 + jc) * 128 + nt * 16:][:, :16],
                        src[:, :, 1],
                    )
                nc.vector.copy_predicated(
                    erT[:, (18 + jc) * 128:][:, :BC],
                    cmask_t[:],
                    erT[:, (6 + jc) * 128:][:, :BC],
                )

        # -------- phase 2: GNN layer 1 (full sources -> packed-C3) --------
        with ExitStack() as p2:
            wpool = p2.enter_context(tc.tile_pool(name="w", bufs=2))
            a1pool = p2.enter_context(tc.tile_pool(name="a1", bufs=1))
            hapool = p2.enter_context(tc.tile_pool(name="ha", bufs=6))
            msgpool = p2.enter_context(tc.tile_pool(name="msg", bufs=2))
            tmppool = p2.enter_context(tc.tile_pool(name="tmp", bufs=3))
            ps_t2 = p2.enter_context(tc.tile_pool(name="ps_t2", bufs=3, space="PSUM"))
            ps_m = p2.enter_context(tc.tile_pool(name="ps_m", bufs=2, space="PSUM"))
            ps_a2 = p2.enter_context(tc.tile_pool(name="ps_a2", bufs=2, space="PSUM"))

            atb1_t = a1pool.tile([128, (NCN // 128) * W1M], bf16)
            nc.sync.dma_start(sb3(atb1_t, W1M), atb1.transpose([1, 0, 2]))

            ws_t = wpool.tile([128, HC * H], bf16, tag="ws")
            nc.sync.dma_start(ws_t[:], wself[0])
            wn_t = wpool.tile([128, HC * H], bf16, tag="wn")
            nc.sync.dma_start(wn_t[:], wnbr[0])
            # full node tile nt == supergroup sg (512 cols = 16 graphs)
            for sg in range(NSG):
                c3 = c3s[sg]
                sgw = SGW3[sg]
                w1 = 4 * c3
                msg_t = msgpool.tile(
                    [128, HC * sgw], bf16, name="msg1", tag="msg1"
                )
                has = [None] * HC

                # transpose/copy units lead the A-matmul units by 2 so the
                # psum->sbuf scalar copy hides under later transposes
                def emit_t(jc, sg=sg, has=has):
                    pst = ps_t2.tile([128, 512], bf16, name="pst1", tag="pst1")
                    for g4 in range(GPT):
                        nc.tensor.transpose(
                            pst[:, g4 * 128:][:, :128],
                            hA[jc][sg][:, g4 * 128:][:, :128],
                            ident[:],
                        )
                    ha = hapool.tile([128, 512], bf16, name="ha1", tag="ha1")
                    nc.scalar.activation(ha[:], pst[:], AF.Copy)
                    has[jc] = ha

                def emit_a(jc, sg=sg, sgw=sgw, w1=w1, has=has, msg_t=msg_t):
                    psm = ps_m.tile([128, sgw], f32, name="psm1", tag="psm1")
                    for g4 in range(GPT):
                        nc.tensor.matmul(
                            psm[:, g4 * w1:][:, :w1],
                            has[jc][:, g4 * 128:][:, :128],
                            atb1_t[:, (sg * GPT + g4) * W1M:][:, :w1],
                            start=True,
                            stop=True,
                        )
                    nc.vector.tensor_copy(msg_t[:, jc * sgw:][:, :sgw], psm[:])

                for jc in range(HC + 2):
                    if jc < HC:
                        emit_t(jc)
                    if jc >= 2:
                        emit_a(jc - 2)

                for jc in range(HC):
                    acc = ps_a2.tile([128, sgw], f32, name="acc1", tag="acc1")
                    for kc in range(HC):
                        nc.tensor.matmul(
                            acc[:],
                            ws_t[:, kc * H + jc * 128:][:, :128],
                            hA[kc][sg].rearrange(
                                "p (b u) -> p b u", u=P
                            )[:, :, :c3],
                            start=(kc == 0),
                            stop=False,
                        )
                    for kc in range(HC):
                        nc.tensor.matmul(
                            acc[:],
                            wn_t[:, kc * H + jc * 128:][:, :128],
                            msg_t[:, kc * sgw:][:, :sgw],
                            start=False,
                            stop=(kc == HC - 1),
                        )
                    tmp = tmppool.tile([128, sgw], f32, name="tmp1", tag="tmp1")
                    nc.scalar.activation(tmp[:], acc[:], AF.Relu)
                    nc.vector.tensor_add(
                        out=h1s[jc][:, off3[sg]:][:, :sgw].rearrange(
                            "p (b u) -> p b u", u=c3
                        ),
                        in0=tmp[:].rearrange("p (b u) -> p b u", u=c3),
                        in1=hA[jc][sg].rearrange(
                            "p (b u) -> p b u", u=P
                        )[:, :, :c3],
                    )

        # hA (h_text) no longer needed
        hsA.close()

        # ------- phase 3: GNN layer 2 (packed-C3 sources -> packed-C2) ----
        with ExitStack() as p3:
            wpool = p3.enter_context(tc.tile_pool(name="w2", bufs=2))
            a2pool = p3.enter_context(tc.tile_pool(name="a2", bufs=1))
            hapool = p3.enter_context(tc.tile_pool(name="ha2", bufs=6))
            msgpool = p3.enter_context(tc.tile_pool(name="msg2", bufs=2))
            tmppool = p3.enter_context(tc.tile_pool(name="tmp2", bufs=3))
            ps_t2 = p3.enter_context(tc.tile_pool(name="ps_t3", bufs=3, space="PSUM"))
            ps_m = p3.enter_context(tc.tile_pool(name="ps_m3", bufs=2, space="PSUM"))
            ps_a2 = p3.enter_context(tc.tile_pool(name="ps_a3", bufs=2, space="PSUM"))

            atb2_t = a2pool.tile([128, NBLK3 * AW2], bf16)
            nc.sync.dma_start(sb3(atb2_t, AW2), atb2.transpose([1, 0, 2]))

            zpool = p3.enter_context(tc.tile_pool(name="z", bufs=1))
            ps_z = p3.enter_context(tc.tile_pool(name="ps_z", bufs=1, space="PSUM"))

            ws_t = wpool.tile([128, HC * H], bf16, tag="ws2")
            nc.sync.dma_start(ws_t[:], wself[1])
            wn_t = wpool.tile([128, HC * H], bf16, tag="wn2")
            nc.sync.dma_start(wn_t[:], wnbr[1])

            ohd_t = zpool.tile([128, BC], bf16)
            nc.sync.dma_start(ohd_t[:P, :], ohd[:])
            demb_t = zpool.tile([128, H], bf16)
            nc.sync.dma_start(demb_t[:P, :], demb[:])
            bexpl_t = zpool.tile([128, HC], f32)
            nc.sync.dma_start(bexpl_t[:], bexpl[:])
            ext_t = zpool.tile([128, HC * BC], bf16)
            nc.sync.dma_start(sb3(ext_t, BC), chunked(ext, BC))
            wexpl_t = zpool.tile([128, HC * H], bf16)
            nc.sync.dma_start(wexpl_t[:], wexpl[:])

            # h_dist (erT chunks 24-29) and z_teacher (30-35): independent of
            # the GNN; interleave psum-groups per nt2 so drains hide under
            # the layer-2 matmul streams.
            def emit_zdist(zi):
                jc = zi % HC
                if zi < HC:
                    psd = ps_z.tile([128, BC], f32, tag="zz", name="zz")
                    nc.tensor.matmul(
                        psd[:],
                        demb_t[:P, jc * 128:][:, :128],
                        ohd_t[:P, :],
                        start=True,
                        stop=True,
                    )
                    nc.scalar.activation(
                        erT[:, (24 + jc) * 128:][:, :BC], psd[:], AF.Copy
                    )
                else:
                    psz = ps_z.tile([128, BC], f32, tag="zz", name="zz")
                    for kc in range(HC):
                        nc.tensor.matmul(
                            psz[:],
                            wexpl_t[:, kc * H + jc * 128:][:, :128],
                            ext_t[:, kc * BC:][:, :BC],
                            start=(kc == 0),
                            stop=(kc == HC - 1),
                        )
                    nc.scalar.activation(
                        erT[:, (30 + jc) * 128:][:, :BC],
                        psz[:],
                        AF.Relu,
                        bias=bexpl_t[:, jc:jc + 1],
                    )

            for t2 in range(NT2):
                # message phase over this tile's supergroups; transpose/copy
                # units lead A-matmul units by 2 (skew)
                sgs = tiles2[t2]
                msg_t = msgpool.tile(
                    [128, HC * 512], bf16, name="msg2t", tag="msg2t"
                )
                NU = HC * len(sgs)
                has = [None] * NU
                psms = [None] * HC

                def emit_t(u, sgs=sgs, has=has):
                    jc, si = divmod(u, len(sgs))
                    sg = sgs[si]
                    nb = SGW3[sg] // 128
                    pst = ps_t2.tile(
                        [128, nb * 128], bf16, name="pst2", tag="pst2"
                    )
                    for b in range(nb):
                        nc.tensor.transpose(
                            pst[:, b * 128:][:, :128],
                            h1s[jc][:, off3[sg] + b * 128:][:, :128],
                            ident[:],
                        )
                    ha = hapool.tile(
                        [128, nb * 128], bf16, name="ha2", tag="ha2"
                    )
                    nc.scalar.activation(ha[:], pst[:], AF.Copy)
                    has[u] = ha

                def emit_a(u, t2=t2, sgs=sgs, has=has, psms=psms, msg_t=msg_t):
                    jc, si = divmod(u, len(sgs))
                    sg = sgs[si]
                    if si == 0:
                        psms[jc] = ps_m.tile(
                            [128, 512], f32, name="psm2", tag="psm2"
                        )
                    psm = psms[jc]
                    ha = has[u]
                    base = off2[sg] - 512 * t2
                    blk0 = off3[sg] // 128
                    for (b, rc0, ncol, pc0, st, sp) in plans[sg]:
                        nc.tensor.matmul(
                            psm[:, base + pc0:][:, :ncol],
                            ha[:, b * 128:][:, :128],
                            atb2_t[:, (blk0 + b) * AW2 + rc0:][:, :ncol],
                            start=st,
                            stop=sp,
                        )
                    if si == len(sgs) - 1:
                        nc.vector.tensor_copy(
                            msg_t[:, jc * 512:][:, :512], psm[:]
                        )

                for u in range(NU + 2):
                    if u < NU:
                        emit_t(u)
                    if u >= 2:
                        emit_a(u - 2)

                # W-matmuls: self over per-sg packed-C2 views, nbr over msg
                # full-width psum chains over groups of consecutive
                # same-config supergroups (multi-instruction accumulation
                # must cover the whole psum tile: sub-region chains
                # mis-accumulate on HW)
                wgroups = []
                for sg in sgs:
                    if (wgroups and cfg[wgroups[-1][0]] == cfg[sg]
                            and wgroups[-1][0] + wgroups[-1][1] == sg):
                        wgroups[-1][1] += 1
                    else:
                        wgroups.append([sg, 1])
                for jc in range(HC):
                    for s0, ns in wgroups:
                        base = off2[s0] - 512 * t2
                        sw2 = ns * SGW2[s0]
                        sw3 = ns * SGW3[s0]
                        c2, c3 = cfg[s0]
                        hv = lambda kc: h1s[kc][
                            :, off3[s0]:
                        ][:, :sw3].rearrange(
                            "p (b u) -> p b u", u=c3
                        )[:, :, :c2]
                        acc = ps_a2.tile(
                            [128, sw2], f32, name="acc2", tag="acc2"
                        )
                        for kc in range(HC):
                            nc.tensor.matmul(
                                acc[:],
                                ws_t[:, kc * H + jc * 128:][:, :128],
                                hv(kc),
                                start=(kc == 0),
                                stop=False,
                            )
                        for kc in range(HC):
                            nc.tensor.matmul(
                                acc[:],
                                wn_t[:, kc * H + jc * 128:][:, :128],
                                msg_t[:, kc * 512 + base:][:, :sw2],
                                start=False,
                                stop=(kc == HC - 1),
                            )
                        tmp = tmppool.tile(
                            [128, sw2], f32, name="tmp2", tag="tmp2"
                        )
                        nc.scalar.activation(tmp[:], acc[:], AF.Relu)
                        nc.vector.tensor_add(
                            out=h2s[jc][:, off2[s0]:][:, :sw2].rearrange(
                                "p (b u) -> p b u", u=c2
                            ),
                            in0=tmp[:].rearrange("p (b u) -> p b u", u=c2),
                            in1=hv(jc),
                        )
                if _dbg:
                    for jc in range(HC):
                        nc.sync.dma_start(
                            dbgm_ap[jc][:, t2 * 512:][:, :512],
                            msg_t[:, jc * 512:][:, :512],
                        )
                for zi in range(
                    t2 * 12 // NT2, (t2 + 1) * 12 // NT2
                ):
                    emit_zdist(zi)

        if _dbg:
            for jc in range(HC):
                nc.sync.dma_start(dbg_ap[jc], h2s[jc][:])
                nc.sync.dma_start(dbg1_ap[jc], h1s[jc][:])

        # h1s no longer needed
        hsB.close()

        # ---------------- phase 4: GNN layer 3 (slots 0,1) + predictor ----
        with ExitStack() as p4:
            ppool = p4.enter_context(tc.tile_pool(name="pred", bufs=1))
            w3pool = p4.enter_context(tc.tile_pool(name="w3", bufs=1))
            a3pool = p4.enter_context(tc.tile_pool(name="a3", bufs=1))
            hapool = p4.enter_context(tc.tile_pool(name="ha3", bufs=6))
            tmppool = p4.enter_context(tc.tile_pool(name="tmp3", bufs=2))
            ps_t2 = p4.enter_context(tc.tile_pool(name="ps_t4", bufs=3, space="PSUM"))
            ps_m = p4.enter_context(tc.tile_pool(name="ps_m4", bufs=1, space="PSUM"))
            ps_c3 = p4.enter_context(tc.tile_pool(name="ps_c3", bufs=2, space="PSUM"))
            ps_p = p4.enter_context(tc.tile_pool(name="ps_p", bufs=2, space="PSUM"))

            # small layer-3 DMAs first -- the 7 MB wp1 prefetch must not
            # block them in the DMA queue (layer 3 needs these immediately;
            # wp1 is consumed ~25 us later by the predictor).
            atb3_t = a3pool.tile([128, NBLK2 * W3M], bf16)
            nc.sync.dma_start(sb3(atb3_t, W3M), atb3.transpose([1, 0, 2]))
            ws3_t = w3pool.tile([128, HC * H], bf16, tag="ws3")
            nc.sync.dma_start(ws3_t[:], wself[2])
            wn3_t = w3pool.tile([128, HC * H], bf16, tag="wn3")
            nc.sync.dma_start(wn3_t[:], wnbr[2])
            bp1_t = ppool.tile([128, HC], f32)
            nc.sync.dma_start(bp1_t[:], bp1[:])
            wp2_t = ppool.tile([128, HC], bf16)
            nc.sync.dma_start(wp2_t[:], wp2[:])
            bp2_t = ppool.tile([1, 1], f32)
            nc.sync.dma_start(bp2_t[:], bp2[:])
            wp1_t = []
            for jc in range(HC):
                w1s = ppool.tile(
                    [128, 36 * 128], bf16, tag=f"wp1_{jc}", name=f"wp1_{jc}"
                )
                nc.sync.dma_start(w1s[:], wp1[jc])
                wp1_t.append(w1s)

            # --- layer 3 message + gather at slots {0,1} ---
            msg3_t = ppool.tile([128, HC * 2 * BC], bf16)
            h2p_t = ppool.tile([128, HC * 2 * BC], bf16)
            for jc in range(HC):
                psm = ps_m.tile([128, 2 * BC], f32, name="psm3", tag="psm3")
                has = [None] * NBLK2

                def emit_t(blk, jc=jc, has=has):
                    pst = ps_t2.tile([128, 128], bf16, name="pst3", tag="pst3")
                    nc.tensor.transpose(
                        pst[:], h2s[jc][:, blk * 128:][:, :128], ident[:]
                    )
                    ha = hapool.tile([128, 128], bf16, name="ha3", tag="ha3")
                    nc.scalar.activation(ha[:], pst[:], AF.Copy)
                    has[blk] = ha

                def emit_a(blk, has=has, psm=psm):
                    _, gpb, g0 = blocks3[blk]
                    nc.tensor.matmul(
                        psm[:, 2 * g0:][:, :2 * gpb],
                        has[blk][:],
                        atb3_t[:, blk * W3M:][:, :2 * gpb],
                        start=True,
                        stop=True,
                    )

                for u in range(NBLK2 + 3):
                    if u < NBLK2:
                        emit_t(u)
                    if u >= 3:
                        emit_a(u - 3)

                nc.vector.tensor_copy(msg3_t[:, jc * 2 * BC:][:, :2 * BC], psm[:])
                for sg in range(NSG):
                    src = h2s[jc][:, off2[sg]:][:, :SGW2[sg]].rearrange(
                        "p (b u) -> p b u", u=c2s[sg]
                    )
                    dst = h2p_t[
                        :, jc * 2 * BC + sg * 2 * SG:
                    ][:, :2 * SG].rearrange("p (b u) -> p b u", u=2)
                    nc.vector.tensor_copy(dst, src[:, :, :2])

            # --- layer 3 W-matmuls -> h3 -> erT chunks 0-5 (c), 12-17 (t) ---
            for jc in range(HC):
                acc = ps_c3.tile([128, 2 * BC], f32, name="acc3", tag="acc3")
                for kc in range(HC):
                    nc.tensor.matmul(
                        acc[:],
                        ws3_t[:, kc * H + jc * 128:][:, :128],
                        h2p_t[:, kc * 2 * BC:][:, :2 * BC],
                        start=(kc == 0),
                        stop=False,
                    )
                for kc in range(HC):
                    nc.tensor.matmul(
                        acc[:],
                        wn3_t[:, kc * H + jc * 128:][:, :128],
                        msg3_t[:, kc * 2 * BC:][:, :2 * BC],
                        start=False,
                        stop=(kc == HC - 1),
                    )
                tmp = tmppool.tile([128, 2 * BC], f32, name="tmp3", tag="tmp3")
                nc.scalar.activation(tmp[:], acc[:], AF.Relu)
                h3 = tmppool.tile([128, 2 * BC], bf16, name="h3", tag="h3")
                nc.vector.tensor_add(
                    out=h3[:], in0=tmp[:], in1=h2p_t[:, jc * 2 * BC:][:, :2 * BC]
                )
                h3v = h3.rearrange("p (b u) -> p b u", u=2)
                nc.vector.tensor_copy(erT[:, (0 + jc) * 128:][:, :BC], h3v[:, :, 0])
                nc.vector.tensor_copy(erT[:, (12 + jc) * 128:][:, :BC], h3v[:, :, 1])
                nc.vector.copy_predicated(
                    erT[:, (12 + jc) * 128:][:, :BC],
                    cmask_t[:],
                    erT[:, (0 + jc) * 128:][:, :BC],
                )

            hid_t = ppool.tile([128, HC * BC], bf16)
            for jc in range(HC):
                psp = ps_p.tile([128, BC], f32, tag="pp", name="pp")
                for kc in range(36):
                    nc.tensor.matmul(
                        psp[:],
                        wp1_t[jc][:, kc * 128:][:, :128],
                        erT[:, kc * 128:][:, :128],
                        start=(kc == 0),
                        stop=(kc == 35),
                    )
                nc.scalar.activation(
                    hid_t[:, jc * BC:][:, :BC],
                    psp[:],
                    AF.Relu,
                    bias=bp1_t[:, jc:jc + 1],
                )

            psl = ps_p.tile([128, BC], f32, tag="pp", name="psl")
            for jc in range(HC):
                nc.tensor.matmul(
                    psl[:1, :],
                    wp2_t[:, jc:jc + 1],
                    hid_t[:, jc * BC:][:, :BC],
                    start=(jc == 0),
                    stop=(jc == HC - 1),
                )
            logit_t = ppool.tile([128, BC], f32)
            nc.vector.tensor_scalar_add(
                out=logit_t[:1, :], in0=psl[:1, :], scalar1=bp2_t[:1, :1]
            )
            nc.sync.dma_start(out_ap[:], logit_t[:1, :])

    nc.compile()
    return nc


def _host_prep(inputs):
    x = np.asarray(inputs["x"], np.float32)
    spk = np.asarray(inputs["speaker_ids"], np.int64)
    emo = np.asarray(inputs["emotion_ids"], np.int64)
    ei = np.asarray(inputs["edge_index"], np.int64)
    tni = np.asarray(inputs["target_node_indices"], np.int64)
    ex = np.asarray(inputs["expl_space_vec"], np.float32)

    E = ei.shape[1]
    edge_src, edge_tgt = ei[0], ei[1]
    c_idx, t_idx = tni[:, 0], tni[:, 1]

    # reference first-edge/dist logic (exact)
    fe = np.full(N, E, np.int64)
    np.minimum.at(fe, edge_src, np.arange(E, dtype=np.int64))

    def first_tgt(q):
        feq = fe[q]
        return np.where(feq < E, edge_tgt[np.minimum(feq, E - 1)], q)

    dist = np.clip(np.abs(first_tgt(c_idx) - first_tgt(t_idx)), 0, P - 1)

    # slot-1 node: t, or a filler distinct from c when c == t
    t_eff = np.where(c_idx == t_idx, (t_idx + 1) % P, t_idx)

    # per-graph receptive-field sets (old coords):
    # T2 = {c,t} U in({c,t}); T3 = T2 U in(T2)
    g_e = edge_src // P
    s_l, t_l = edge_src % P, edge_tgt % P
    innb = np.zeros((B, P, P), np.int8)
    innb[g_e, t_l, s_l] = 1
    sel = np.zeros((B, P), bool)
    bidx = np.arange(B)
    sel[bidx, c_idx] = True
    sel[bidx, t_eff] = True
    grow = lambda X: X | (np.einsum("bts,bt->bs", innb, X.astype(np.int8)) > 0)
    S2 = grow(sel)
    S3 = grow(S2)
    t2cnt = S2.sum(1)
    t3cnt = S3.sum(1)

    # per-graph node permutation: slot 0 = c, slot 1 = t_eff, T2 within
    # prefix C2, T3 within prefix C3
    prio = np.full((B, P), 8, np.int64)
    prio[S3] = 3
    prio[S2] = 2
    prio[bidx, t_eff] = 1
    prio[bidx, c_idx] = 0
    new2old = np.argsort(prio, axis=1, kind="stable")
    old2new = np.argsort(new2old, axis=1)

    # adjacency in permuted coords (original graph order)
    s_new = old2new[g_e, s_l]
    t_new = old2new[g_e, t_l]
    A = np.zeros((B, P, P), np.float32)
    np.add.at(A, (g_e, t_new, s_new), 1.0)

    # region config: reorder graphs within each core so light graphs
    # (small T2/T3) land in supergroups with small C2/C3 prefixes
    kcls = np.where(t3cnt > 16, 2, np.where(t2cnt > 8, 1, 0)).reshape(
        NCORES, BC
    )
    n_k2 = (kcls == 2).sum(1).max()
    n_k12 = (kcls >= 1).sum(1).max()
    t2max, t3max = int(t2cnt.max()), int(t3cnt.max())
    import os
    force = os.environ.get("KCFG", "")
    if t2max <= 16 and t3max <= 24 and n_k2 <= SG and n_k12 <= 4 * SG:
        cfg = ((8, 16),) * 4 + ((16, 16),) * 3 + ((16, 24),)
        if force == "c3only":
            cfg = ((16, 16),) * 7 + ((16, 24),)
        elif force == "uniform":
            cfg = ((16, 24),) * 8
        gperm = (
            np.argsort(kcls, axis=1, kind="stable")
            + np.arange(NCORES)[:, None] * BC
        ).reshape(-1)
    else:
        c2u = 16 if t2max <= 16 else 32
        c3u = max(c2u, 24 if t3max <= 24 else 32)
        cfg = ((c2u, c3u),) * NSG
        gperm = np.arange(B)
    c2g = np.tile(np.repeat([c[0] for c in cfg], SG), NCORES)  # per position
    c3g = np.tile(np.repeat([c[1] for c in cfg], SG), NCORES)

    # reorder all per-graph data into position order
    A = A[gperm]
    dist = dist[gperm]
    new2old = new2old[gperm]
    ceqt = (c_idx == t_idx)[gperm]
    ex = ex[gperm]
    t2cnt = t2cnt[gperm]

    # exactness checks: every in-edge of slots {0,1} originates within the
    # graph's prefix C2, and of a true-T2 slot within prefix C3
    srcbad2 = np.arange(P)[None, :] >= c2g[:, None]         # [B, P]
    assert not (A[:, :2, :].any(1) & srcbad2).any()
    usedmask = np.arange(P)[None, :] < t2cnt[:, None]       # true-T2 slots
    srcbad3 = np.arange(P)[None, :] >= c3g[:, None]
    assert not (A * usedmask[:, :, None] * srcbad3[:, None, :]).any()

    perm_global = (gperm[:, None] * P + new2old).reshape(-1)

    xtb = np.ascontiguousarray(x[perm_global].T.astype(BF16))  # [DSEM, N]
    spk_new = spk[perm_global]
    emo_new = emo[perm_global]

    # speaker+emotion embedding sums, feature-major [H, N]
    embsum = (
        np.asarray(inputs["spk_emb"], np.float32)[spk_new]
        + np.asarray(inputs["emo_emb"], np.float32)[emo_new]
    ).T.astype(BF16)

    # geometry (mirrors _build_program)
    c2s = [c[0] for c in cfg]
    c3s = [c[1] for c in cfg]
    SGW2 = [SG * c for c in c2s]
    SGW3 = [SG * c for c in c3s]
    off2 = np.concatenate([[0], np.cumsum(SGW2)])
    off3 = np.concatenate([[0], np.cumsum(SGW3)])
    NC2, NC3 = int(off2[-1]), int(off3[-1])
    W1M = 4 * max(c3s)
    plans = [_a2_plan(c2s[sg], c3s[sg]) for sg in range(NSG)]
    AW2 = max(e[1] + e[2] for p in plans for e in p)
    NBLK3 = NC3 // 128
    NBLK2 = NC2 // 128
    W3M = 2 * max(128 // c for c in c2s)

    # layer-1 AT tiles: full-layout sources (4 graphs per 128-row block),
    # packed-C3 targets
    nfb = B * P // 128
    atb1 = np.zeros((nfb, 128, W1M), np.float32)
    for i in range(4):
        for sg in range(NSG):          # same cfg on every core
            c3 = c3s[sg]
            fb = (np.arange(B // 4).reshape(NCORES, NSG, 4)[:, sg, :]).ravel()
            gl = fb * 4 + i
            atb1[fb, 32 * i:32 * i + 32, c3 * i:c3 * i + c3] = (
                A[gl][:, :c3, :].transpose(0, 2, 1)
            )
    atb1 = atb1.astype(BF16)

    # layer-2 AT tiles: packed-C3 sources -> packed-C2 targets, blocks are
    # supergroup-aligned; col origin of block b is its first graph
    atb2 = np.zeros((NCORES * NBLK3, 128, AW2), np.float32)
    for core in range(NCORES):
        for sg in range(NSG):
            c2, c3 = c2s[sg], c3s[sg]
            blk0 = core * NBLK3 + int(off3[sg]) // 128
            for gl in range(SG):
                g = core * BC + sg * SG + gl
                rows = gl * c3 + np.arange(c3)
                bb = rows // 128
                rr = rows % 128
                g0b = (128 * bb) // c3
                for t in range(c2):
                    atb2[blk0 + bb, rr, (gl - g0b) * c2 + t] = A[g, t, :c3]
    atb2 = atb2.astype(BF16)

    # layer-3 AT tiles: packed-C2 sources -> slots {0,1}
    atb3 = np.zeros((NCORES * NBLK2, 128, W3M), np.float32)
    for core in range(NCORES):
        for sg in range(NSG):
            c2 = c2s[sg]
            gpb = 128 // c2
            nb = SGW2[sg] // 128
            for k in range(nb):
                blk = core * NBLK2 + int(off2[sg]) // 128 + k
                for i in range(gpb):
                    g = core * BC + sg * SG + k * gpb + i
                    atb3[blk, c2 * i:c2 * i + c2, 2 * i:2 * i + 2] = (
                        A[g, :2, :c2].T
                    )
    atb3 = atb3.astype(BF16)

    cmask = np.tile(ceqt.astype(np.uint8)[None, :], (128, 1))

    ohd = np.zeros((P, B), BF16)
    ohd[dist, np.arange(B)] = 1.0

    extT = np.ascontiguousarray(ex.T.astype(BF16))

    rearr = lambda v: np.ascontiguousarray(
        np.asarray(v, np.float32).reshape(HC, 128).T
    )
    # [K, H] -> [128, (K//128)*H] SBUF-layout slab (contiguous DMA)
    chunk_w = lambda w: np.ascontiguousarray(
        np.asarray(w, np.float32)
        .reshape(-1, 128, w.shape[-1]).transpose(1, 0, 2)
        .reshape(128, -1)
    ).astype(BF16)
    b16 = lambda k: np.asarray(inputs[k], np.float32).astype(BF16)

    shared = dict(
        wsem=chunk_w(np.asarray(inputs["W_sem"], np.float32)),
        wself=np.stack([
            chunk_w(np.asarray(inputs["gnn_w_self"], np.float32)[l])
            for l in range(L)
        ]),
        wnbr=np.stack([
            chunk_w(np.asarray(inputs["gnn_w_nbr"], np.float32)[l])
            for l in range(L)
        ]),
        demb=b16("dist_emb"),
        wexpl=chunk_w(np.asarray(inputs["W_expl"], np.float32)),
        bexpl=rearr(inputs["b_expl"]),
        wp1=np.ascontiguousarray(
            np.asarray(inputs["W_p1"], np.float32)
            .reshape(36, 128, HC, 128).transpose(2, 1, 0, 3)
            .reshape(HC, 128, 36 * 128)
        ).astype(BF16),
        bp1=rearr(inputs["b_p1"]),
        wp2=rearr(np.asarray(inputs["W_p2"], np.float32)[:, 0]).astype(BF16),
        bp2=np.asarray(inputs["b_p2"], np.float32).reshape(1, 1),
    )

    in_maps = []
    for i in range(NCORES):
        gs = slice(i * BC, (i + 1) * BC)
        ns = slice(i * NCN, (i + 1) * NCN)
        m = dict(shared)
        m["xt"] = np.ascontiguousarray(
            xtb[:, ns].reshape(8, 128, NT, 512).transpose(2, 0, 1, 3)
        )
        m["embt"] = np.ascontiguousarray(
            embsum[:, ns].reshape(HC, 128, NT, 512).transpose(2, 0, 1, 3)
        )
        m["atb1"] = np.ascontiguousarray(atb1[i * (NCN // 128):][:NCN // 128])
        m["atb2"] = np.ascontiguousarray(atb2[i * NBLK3:][:NBLK3])
        m["atb3"] = np.ascontiguousarray(atb3[i * NBLK2:][:NBLK2])
        m["cmask"] = np.ascontiguousarray(cmask[:, gs])
        m["ohd"] = np.ascontiguousarray(ohd[:, gs])
        m["ext"] = np.ascontiguousarray(extT[:, gs])
        in_maps.append(m)
    return in_maps, cfg, gperm


def kernel(**inputs):
    in_maps, cfg, gperm = _host_prep(inputs)
    if cfg not in _cache:
        _cache[cfg] = _build_program(cfg)
    from concourse.bass_utils import run_bass_kernel_spmd

    res = run_bass_kernel_spmd(_cache[cfg], in_maps, list(range(NCORES)))
    out = np.empty(B, np.float32)
    out[gperm] = np.concatenate(
        [res.results[i]["out"].reshape(BC) for i in range(NCORES)]
    ).astype(np.float32)
    return out
